# revision 19
# baseline (speedup 1.0000x reference)
"""DMSTGCN forward on 8 Trainium2 NeuronCores (Bass/Tile).

Sharding: data-parallel over batch B=16 -> 2 batches per core; parameters
replicated. The dynamic adjacency (1024x1024 per batch) is built and kept in
SBUF (bf16); 1x1 convs run as block-diagonal (W (x) I) matmuls in an l-major
"[(time,chan), node]" layout, graph hops in "[node, (time,chan)]" layout with
PE transposes between the two. Trunk math is float32r (TF32-like), graph-hop
operands bf16. The two batches are emitted layer-interleaved, all heavy ops
are sliced per 512 nodes, and PSUM tiles are single-bank so the scheduler can
overlap the two batch streams.
"""
import numpy as np
import ml_dtypes

import concourse.bacc as bacc
import concourse.mybir as mybir
from concourse.tile import TileContext
from concourse.bass_utils import run_bass_kernel_spmd

F32 = mybir.dt.float32
F32R = mybir.dt.float32r
BF16 = mybir.dt.bfloat16
AF = mybir.ActivationFunctionType
ALU = mybir.AluOpType

B, N, T, RF = 16, 2, 1024, 12  # placeholder, fixed below
B, N, T, RF = 16, 1024, 12, 13
RC, SC, DIMS, L = 16, 8, 32, 8
BN_EPS = 1e-5
NCORES = 8
BPC = B // NCORES          # batches per core
CL = RC * RF               # 208 rows in T-layout
SKR = SC * RF              # 104 skip rows
CH = ((0, 128), (128, 80))  # l-major T-layout row chunks
NV_COLS = 4 + L + L * 2 * 3 + 2

_CACHED = None


def _build_nc():
    nc = bacc.Bacc("TRN2", target_bir_lowering=False)

    d = {}
    def din(name, shape, dt=F32R):
        d[name] = nc.dram_tensor(name, list(shape), dt, kind="ExternalInput")

    din("inp", (BPC, 2, RF, N))
    din("adp", (BPC, DIMS, DIMS))
    din("p2T", (DIMS, N))
    din("p3sT", (DIMS, DIMS))
    din("wstart0", (2, RF, 128))
    din("wstart1", (2, RF, 80))
    din("wfc1_0", (128, 128)); din("wfc1_1", (80, 80))
    din("wfc2_0", (128, 128), BF16); din("wfc2_1", (80, 80), BF16)
    din("wskip0", (L, 128, 64), BF16)
    din("wskip1", (L, 80, 40), BF16)
    din("wgc0", (L, 3, 128, 128), BF16)
    din("wgc1", (L, 3, 80, 80), BF16)
    din("we1", (L, SKR, 64), BF16)
    din("we2", (64, 12))
    din("idenb", (128, 128), BF16)
    din("idenr", (128, 128))
    din("wav0", (L, 128, 128), BF16)
    din("wav1", (L, 80, 80), BF16)
    din("vecs", (128, NV_COLS), F32)
    outp = nc.dram_tensor("outp", [BPC, 12, N], F32, kind="ExternalOutput")

    with TileContext(nc) as tc, \
         tc.tile_pool(name="wp", bufs=1) as wp, \
         tc.tile_pool(name="ap", bufs=1) as ap, \
         tc.tile_pool(name="pp", bufs=1, space="PSUM") as pp:

        def wtile(name, src_ap, shape, dt=F32R, eng=None):
            t = wp.tile(shape, dt, tag=name, name=name)
            (eng or nc.sync).dma_start(out=t[:], in_=src_ap)
            return t

        # phase0-critical loads go first on the SP queue; bulk weights on
        # gpsimd so PE can start within ~2us.
        p2T = wtile("p2T", d["p2T"][:], (DIMS, N), eng=nc.sync)
        p3sT = wtile("p3sT", d["p3sT"][:], (DIMS, DIMS), eng=nc.sync)
        adps = [wtile(f"adp{b}", d["adp"][b], (DIMS, DIMS), eng=nc.sync)
                for b in range(BPC)]

        idenb = wtile("idenb", d["idenb"][:], (128, 128), BF16)
        idenr = wtile("idenr", d["idenr"][:], (128, 128))
        wav = [[wtile(f"wav{i}_{c}", d[f"wav{c}"][i],
                      (CH[c][1], CH[c][1]), BF16) for c in range(2)]
               for i in range(L)]
        vecs = wtile("vecs", d["vecs"][:], (128, NV_COLS), F32)
        wstart = [[wtile(f"wst{s}_{c}", d[f"wstart{c}"][s],
                         (RF, CH[c][1])) for c in range(2)] for s in range(2)]
        wfc1 = [wtile(f"wfc1_{c}", d[f"wfc1_{c}"][:],
                      (CH[c][1], CH[c][1])) for c in range(2)]
        wfc2 = [wtile(f"wfc2_{c}", d[f"wfc2_{c}"][:],
                      (CH[c][1], CH[c][1]), BF16) for c in range(2)]
        wskip = [[wtile(f"wsk{i}_{c}", d[f"wskip{c}"][i],
                        (CH[c][1], (64, 40)[c]), BF16) for c in range(2)]
                 for i in range(L)]
        we1 = [wtile(f"we1_{i}", d["we1"][i], (SKR, 64), BF16) for i in range(L)]
        we2 = wtile("we2", d["we2"][:], (64, 12))

        vc = {}
        ci = 0
        for nm in ("sb0", "sb1", "sab0", "sab1"):
            vc[nm] = ci; ci += 1
        for i in range(L):
            vc[f"skb{i}"] = ci; ci += 1
        for i in range(L):
            for c in range(2):
                for nm in ("bns", "bnb", "av"):
                    vc[f"{nm}{i}_{c}"] = ci; ci += 1
        vc["e1b"] = ci; ci += 1
        vc["e2b"] = ci; ci += 1
        assert ci == NV_COLS

        def vcol(nm, rows=128):
            return vecs[:rows, vc[nm]:vc[nm] + 1]

        NS = (slice(0, 512), slice(512, 1024))

        st = [dict() for _ in range(BPC)]

        # ---------------- adjacency (both batches interleaved) ----------
        def phase0_pair():
            BS = range(BPC)
            # L-stack rows: [u; -srcT], R-stack rows: [srcT; u] so that
            # D = x1^T - x1 is ONE K=64 matmul per (v, ns).
            Lst = [ap.tile((64, N), F32R, tag=f"Lst{b}", name=f"Lst{b}")
                   for b in BS]
            Rst = [ap.tile((64, N), F32R, tag=f"Rst{b}", name=f"Rst{b}")
                   for b in BS]
            for nsi, ns in enumerate(NS):
                pss = []
                for b in BS:
                    ps = pp.tile((DIMS, 512), F32, tag="pwork", bufs=3,
                                 name=f"srcTps{b}_{nsi}")
                    nc.tensor.matmul(ps[:], adps[b][:], p2T[:, ns],
                                     start=True, stop=True)
                    pss.append(ps)
                for b in BS:
                    nc.scalar.activation(Rst[b][0:32, ns], pss[b][:], AF.Copy)
                    nc.vector.tensor_scalar(Lst[b][32:64, ns], pss[b][:],
                                            -1.0, None, ALU.mult)
            for nsi, ns in enumerate(NS):
                pss = []
                for b in BS:
                    ps = pp.tile((DIMS, 512), F32, tag="pwork", bufs=3,
                                 name=f"ups{b}_{nsi}")
                    nc.tensor.matmul(ps[:], p3sT[:], Rst[b][0:32, ns],
                                     start=True, stop=True)
                    pss.append(ps)
                for b in BS:
                    nc.scalar.activation(Lst[b][0:32, ns], pss[b][:], AF.Copy)
                    nc.vector.tensor_copy(Rst[b][32:64, ns], pss[b][:])
            ATs = [[ap.tile((128, N), BF16, tag=f"AT{b}_{v}", name=f"AT{b}_{v}")
                    for v in range(8)] for b in BS]
            Dts = [[ap.tile((128, N), BF16, tag=f"Dt{b}",
                            name=f"Dt{b}_{v}") for v in range(8)] for b in BS]
            for v in range(8):
                cs = slice(v * 128, (v + 1) * 128)
                for nsi, ns in enumerate(NS):
                    dpss = []
                    for b in BS:
                        dps = pp.tile((128, 512), F32, tag="pwork", bufs=3,
                                      name=f"dps{b}_{v}_{nsi}")
                        nc.tensor.matmul(dps[:], Lst[b][:, cs], Rst[b][:, ns],
                                         start=True, stop=True)
                        dpss.append(dps)
                    # relu(tanh(D)) == tanh(max(D, 0))
                    for b in BS:
                        if b % 2 == 0:
                            nc.vector.tensor_scalar(Dts[b][v][:, ns], dpss[b][:],
                                                    0.0, None, ALU.max)
                        else:
                            nc.scalar.activation(Dts[b][v][:, ns], dpss[b][:],
                                                 AF.Relu)
                    for b in BS:
                        nc.scalar.activation(ATs[b][v][:, ns], Dts[b][v][:, ns],
                                             AF.Tanh)
            for b in BS:
                st[b]["AT"] = ATs[b]

        # ---------------- start convs ----------------
        def start(b):
            in0 = ap.tile((RF, N), F32R, tag="in0", name=f"in0_{b}")[:]
            in1 = ap.tile((RF, N), F32R, tag="in1", name=f"in1_{b}")[:]
            nc.sync.dma_start(out=in0, in_=d["inp"][b, 0])
            nc.sync.dma_start(out=in1, in_=d["inp"][b, 1])
            xt, xa = [None, None], [None, None]
            for c in range(2):
                rows = CH[c][1]
                xt[c] = ap.tile((rows, N), F32R, tag=f"XT{b}_{c}", bufs=2,
                                name=f"XT{b}_{c}_init")
                xa[c] = ap.tile((rows, N), BF16, tag=f"XA{b}_{c}",
                                name=f"XA{b}_{c}")
                for nsi, ns in enumerate(NS):
                    ps = pp.tile((rows, 512), F32, tag="pwork", bufs=3,
                                 name=f"stp{b}_{c}_{nsi}")
                    nc.tensor.matmul(ps[:], wstart[0][c][:], in0[:, ns],
                                     start=True, stop=True)
                    nc.scalar.activation(xt[c][:, ns], ps[:], AF.Identity,
                                         bias=vcol(f"sb{c}", rows))
                    psa = pp.tile((rows, 512), F32, tag="pwork", bufs=3,
                                  name=f"stpa{b}_{c}_{nsi}")
                    nc.tensor.matmul(psa[:], wstart[1][c][:], in1[:, ns],
                                     start=True, stop=True)
                    nc.scalar.activation(xa[c][:, ns], psa[:], AF.Identity,
                                         bias=vcol(f"sab{c}", rows))
            st[b]["xt"], st[b]["xa"] = xt, xa
            st[b]["end"] = ap.tile((64, N), F32, tag=f"END{b}", name=f"END{b}")

        # ---------------- one layer, both batches stage-interleaved ----------
        def layer_pair(i):
            BS = range(BPC)
            xt = [st[b]["xt"] for b in BS]
            xa = [st[b]["xa"] for b in BS]
            AT = [st[b]["AT"] for b in BS]

            gcw = [[[ap.tile((CH[c][1], CH[c][1]), BF16, tag=f"gcw{b}_{c}_{s}",
                             bufs=2, name=f"gcw{b}_{i}_{c}_{s}")
                     for c in range(2)] for s in range(3)] for b in BS]
            for b in BS:
                for s in range(3):
                    for c in range(2):
                        nc.sync.dma_start(out=gcw[b][s][c][:],
                                          in_=d[f"wgc{c}"][i, s])

            # -- attention + sigmoid
            xn = [[None, None] for b in BS]
            r1 = [[None, None] for b in BS]
            sg = [[None, None] for b in BS]
            for b in BS:
                for c in range(2):
                    rows = CH[c][1]
                    r1[b][c] = ap.tile((rows, N), BF16, tag=f"R1{b}_{c}",
                                       name=f"R1{b}_{i}_{c}")
                    sg[b][c] = ap.tile((rows, N), F32, tag=f"tmp{b}_{c}",
                                       name=f"sg{b}_{i}_{c}")
                    xn[b][c] = ap.tile((rows, N), BF16, tag=f"XN{b}_{c}",
                                       name=f"XN{b}_{i}_{c}")
            for c in range(2):
                rows = CH[c][1]
                for nsi, ns in enumerate(NS):
                    m1s = []
                    for b in BS:
                        m1 = pp.tile((rows, 512), F32, tag="pwork", bufs=3,
                                     name=f"m1_{b}_{i}_{c}_{nsi}")
                        nc.tensor.matmul(m1[:], wfc1[c][:], xt[b][c][:, ns],
                                         start=True, stop=True)
                        m1s.append(m1)
                    for b in BS:
                        if b % 2 == 0:
                            nc.scalar.activation(r1[b][c][:, ns], m1s[b][:],
                                                 AF.Relu)
                        else:
                            nc.vector.tensor_scalar(r1[b][c][:, ns], m1s[b][:],
                                                    0.0, None, ALU.max)
                    aps = []
                    for b in BS:
                        a_ps = pp.tile((rows, 512), F32, tag="pwork", bufs=3,
                                       name=f"aps{b}_{i}_{c}_{nsi}")
                        nc.tensor.matmul(a_ps[:], wfc2[c][:], r1[b][c][:, ns],
                                         start=True, stop=True)
                        aps.append(a_ps)
                    for b in BS:
                        nc.vector.scalar_tensor_tensor(
                            sg[b][c][:, ns], aps[b][:], 2.0,
                            xt[b][c][:, ns].bitcast(F32), ALU.mult, ALU.add)
                    for b in BS:
                        nc.scalar.activation(xn[b][c][:, ns], sg[b][c][:, ns],
                                             AF.Sigmoid)

            # -- skip conv -> relu -> end1 matmul -> SBUF accumulator
            rsk = [ap.tile((SKR, N), BF16, tag=f"rsk{b}", name=f"rsk{b}_{i}")
                   for b in BS]
            for nsi, ns in enumerate(NS):
                sks = []
                for b in BS:
                    sk_ps = pp.tile((SKR, 512), F32, tag="pwork", bufs=3,
                                    name=f"skp{b}_{i}_{nsi}")
                    nc.tensor.matmul(sk_ps[:64], wskip[i][0][:],
                                     xn[b][0][:, ns], start=True, stop=True)
                    nc.tensor.matmul(sk_ps[64:], wskip[i][1][:],
                                     xn[b][1][:, ns], start=True, stop=True)
                    sks.append(sk_ps)
                for b in BS:
                    if b % 2 == 0:
                        nc.vector.tensor_scalar(rsk[b][:, ns], sks[b][:],
                                                vcol(f"skb{i}", SKR), 0.0,
                                                ALU.add, ALU.max)
                    else:
                        nc.scalar.activation(rsk[b][:, ns], sks[b][:], AF.Relu,
                                             bias=vcol(f"skb{i}", SKR))
                for b in BS:
                    e_ps = pp.tile((64, 512), F32, tag="pwork", bufs=3,
                                   name=f"eps{b}_{i}_{nsi}")
                    nc.tensor.matmul(e_ps[:], we1[i][:], rsk[b][:, ns],
                                     start=True, stop=True)
                    if i == 0:
                        nc.vector.tensor_copy(st[b]["end"][:, ns], e_ps[:])
                    else:
                        nc.vector.scalar_tensor_tensor(
                            st[b]["end"][:, ns], e_ps[:], 0.0,
                            st[b]["end"][:, ns], ALU.bypass, ALU.add)

            # -- V-layout of xn via PE transposes
            xv = [[None] * 8 for b in BS]
            for v in range(8):
                cs = slice(v * 128, (v + 1) * 128)
                for b in BS:
                    tp = pp.tile((128, CL), BF16, tag="ptr", bufs=3,
                                 name=f"tpx{b}_{i}_{v}")
                    for c in range(2):
                        o, rows = CH[c]
                        nc.tensor.transpose(tp[:, o:o + rows],
                                            xn[b][c][:, cs],
                                            idenb[:rows, :rows])
                    xv[b][v] = ap.tile((128, CL), BF16, tag=f"XV{b}_{v}",
                                       name=f"XV{b}_{i}_{v}")
                    nc.vector.tensor_copy(xv[b][v][:], tp[:])

            def hop(rv, nm):
                """A-hop (V-orientation, w-pairs) + transpose back, both b."""
                hvp = [[None] * 4 for b in BS]
                for p in range(4):
                    for b in BS:
                        h_ps = pp.tile((128, 2 * CL), F32, tag="ptr", bufs=3,
                                       name=f"hp{nm}{b}_{i}_{p}")
                        for half in range(2):
                            w = 2 * p + half
                            ws = slice(w * 128, (w + 1) * 128)
                            dst = h_ps[:, half * CL:(half + 1) * CL]
                            for k in range(8):
                                nc.tensor.matmul(dst, AT[b][k][:, ws], rv(b, k),
                                                 start=(k == 0), stop=(k == 7))
                        hvp[b][p] = ap.tile((128, 2 * CL), BF16,
                                            tag=f"{nm}V{b}_{p}",
                                            name=f"{nm}V{b}_{i}_{p}")
                        if (b + p) % 2 == 0:
                            nc.vector.tensor_copy(hvp[b][p][:], h_ps[:])
                        else:
                            nc.scalar.activation(hvp[b][p][:], h_ps[:], AF.Copy)

                ht = [[ap.tile((CH[c][1], N), BF16, tag=f"{nm}T{b}_{c}",
                               name=f"{nm}T{b}_{i}_{c}") for c in range(2)]
                      for b in BS]
                for b in BS:
                    tpb = [pp.tile((CH[c][1], N), BF16, tag=f"ptb{c}",
                                   bufs=1, name=f"tpb{nm}{b}_{i}_{c}")
                           for c in range(2)]
                    for w in range(8):
                        src = hvp[b][w // 2][:, (w % 2) * CL:(w % 2) * CL + CL]
                        for c in range(2):
                            o, rows = CH[c]
                            nc.tensor.transpose(
                                tpb[c][:, w * 128:(w + 1) * 128],
                                src[:, o:o + rows], idenb[:, :])
                        if w % 4 == 3:
                            half = slice((w - 3) * 128, (w + 1) * 128)
                            for c in range(2):
                                if (b + c) % 2 == 0:
                                    nc.scalar.activation(ht[b][c][:, half],
                                                         tpb[c][:, half],
                                                         AF.Copy)
                                else:
                                    nc.vector.tensor_copy(ht[b][c][:, half],
                                                          tpb[c][:, half])
                return hvp, ht

            h1vp, h1t = hop(lambda b, k: xv[b][k][:], "H1")
            _, h2t = hop(
                lambda b, k: h1vp[b][k // 2][:, (k % 2) * CL:(k % 2) * CL + CL],
                "H2")

            # -- gconv (block-diag over l); residual x and av*xa are
            # accumulated in PSUM via identity/diagonal matmuls, then a
            # single affine eviction applies the batchnorm.
            for c in range(2):
                rows = CH[c][1]
                nxt = [ap.tile((rows, N), F32R, tag=f"XT{b}_{c}", bufs=2,
                               name=f"XT{b}_{i}_{c}") for b in BS]
                for nsi, ns in enumerate(NS):
                    gps = []
                    for b in BS:
                        g_ps = pp.tile((rows, 512), F32, tag="pwork", bufs=3,
                                       name=f"gp{b}_{i}_{c}_{nsi}")
                        srcs = (xn[b], h1t[b], h2t[b])
                        for s in range(3):
                            nc.tensor.matmul(g_ps[:], gcw[b][s][c][:],
                                             srcs[s][c][:, ns],
                                             start=(s == 0), stop=False)
                        nc.tensor.matmul(g_ps[:], idenr[:rows, :rows],
                                         xt[b][c][:, ns],
                                         start=False, stop=False)
                        nc.tensor.matmul(g_ps[:], wav[i][c][:],
                                         xa[b][c][:, ns],
                                         start=False, stop=True)
                        gps.append(g_ps)
                    for b in BS:
                        if b % 2 == 0:
                            nc.scalar.activation(nxt[b][:, ns], gps[b][:],
                                                 AF.Identity,
                                                 bias=vcol(f"bnb{i}_{c}", rows),
                                                 scale=vcol(f"bns{i}_{c}", rows))
                        else:
                            nc.vector.tensor_scalar(
                                nxt[b][:, ns], gps[b][:],
                                vcol(f"bns{i}_{c}", rows),
                                vcol(f"bnb{i}_{c}", rows), ALU.mult, ALU.add)
                for b in BS:
                    xt[b][c] = nxt[b]

        # ---------------- end convs ----------------
        def tail(b):
            o1 = ap.tile((64, N), F32R, tag="o1", name=f"o1_{b}")
            ob = ap.tile((12, N), F32, tag="ob", name=f"ob{b}")
            for nsi, ns in enumerate(NS):
                nc.scalar.activation(o1[:, ns], st[b]["end"][:, ns], AF.Relu,
                                     bias=vcol("e1b", 64))
                o2_ps = pp.tile((12, 512), F32, tag="pwork", bufs=3,
                                name=f"o2p{b}_{nsi}")
                nc.tensor.matmul(o2_ps[:], we2[:], o1[:, ns],
                                 start=True, stop=True)
                nc.scalar.activation(ob[:, ns], o2_ps[:], AF.Identity,
                                     bias=vcol("e2b", 12))
            nc.sync.dma_start(out=outp[b], in_=ob[:])

        phase0_pair()
        for b in range(BPC):
            start(b)
        for i in range(L):
            layer_pair(i)
        for b in range(BPC):
            tail(b)

    nc.finalize()
    return nc


# ----------------------------------------------------------------------------
# host-side preprocessing
# ----------------------------------------------------------------------------

def _prep_host(inputs):
    f = lambda x: np.asarray(x, dtype=np.float32)
    bf = lambda x: np.ascontiguousarray(x).astype(ml_dtypes.bfloat16)
    x_in = f(inputs["inputs"])
    ind = np.asarray(inputs["ind"]).astype(np.int64)
    p1, p2, p3, pk = f(inputs["p1"]), f(inputs["p2"]), f(inputs["p3"]), f(inputs["pk"])

    xo = np.pad(x_in, ((0, 0), (0, 0), (0, 0), (RF - T, 0)))
    inp_t = np.ascontiguousarray(xo.transpose(0, 1, 3, 2))
    te = p1[ind]
    adp = np.einsum("bi,ijk->bjk", te, pk).astype(np.float32)

    start_w, start_b = f(inputs["start_w"]), f(inputs["start_b"])
    starta_w, starta_b = f(inputs["starta_w"]), f(inputs["starta_b"])
    fc1_w, fc2_w = f(inputs["fc1_w"]), f(inputs["fc2_w"])
    skip_w, skip_b = f(inputs["skip_w"]), f(inputs["skip_b"])
    gconv_w, gconv_b = f(inputs["gconv_w"]), f(inputs["gconv_b"])
    bn_g, bn_b = f(inputs["bn_g"]), f(inputs["bn_b"])
    bna_g, bna_b = f(inputs["bna_g"]), f(inputs["bna_b"])
    end1_w, end1_b = f(inputs["end1_w"]), f(inputs["end1_b"])
    end2_w, end2_b = f(inputs["end2_w"]), f(inputs["end2_b"])

    e8, e5 = np.eye(8, dtype=np.float32), np.eye(5, dtype=np.float32)
    e13 = np.eye(RF, dtype=np.float32)
    kr = lambda e, w: np.kron(e, np.ascontiguousarray(w.T)).astype(np.float32)

    wstart0 = np.stack([np.kron(e13[:, :8], w[:, 0][None, :])
                        for w in (start_w, starta_w)]).astype(np.float32)
    wstart1 = np.stack([np.kron(e13[:, 8:], w[:, 0][None, :])
                        for w in (start_w, starta_w)]).astype(np.float32)
    wgc0 = np.stack([np.stack([kr(e8, gconv_w[i][:, s * 16:(s + 1) * 16])
                               for s in range(3)]) for i in range(L)])
    wgc1 = np.stack([np.stack([kr(e5, gconv_w[i][:, s * 16:(s + 1) * 16])
                               for s in range(3)]) for i in range(L)])
    wskip0 = np.stack([kr(e8, skip_w[i]) for i in range(L)])
    wskip1 = np.stack([kr(e5, skip_w[i]) for i in range(L)])

    # end1 columns: ref skip rows are o*13+l within the (L-1-i)-th block;
    # ours are l*8+o
    we1 = np.zeros((L, SKR, 64), dtype=np.float32)
    ll, oo = np.meshgrid(np.arange(RF), np.arange(SC), indexing="ij")
    src_col = oo.ravel() * RF + ll.ravel()
    for i in range(L):
        we1[i] = end1_w[:, (L - 1 - i) * SKR + src_col].T

    t8 = lambda v: np.tile(v, 8)
    vecs = np.zeros((128, NV_COLS), dtype=np.float32)
    ci = 0
    vecs[:, ci] = t8(start_b); ci += 1
    vecs[:80, ci] = np.tile(start_b, 5); ci += 1
    vecs[:, ci] = t8(starta_b); ci += 1
    vecs[:80, ci] = np.tile(starta_b, 5); ci += 1
    for i in range(L):
        vecs[:SKR, ci] = np.tile(skip_b[i], RF); ci += 1
    bns = (bn_g / np.sqrt(1.0 + BN_EPS)).astype(np.float32)
    bnas = (bna_g / np.sqrt(1.0 + BN_EPS)).astype(np.float32)
    av = np.ones(16, dtype=np.float32)
    bv = np.zeros(16, dtype=np.float32)
    for i in range(L):
        bnb_adj = bn_b[i] + bns[i] * (gconv_b[i] + bv)
        vecs[:, ci] = t8(bns[i]); ci += 1
        vecs[:, ci] = t8(bnb_adj); ci += 1
        vecs[:, ci] = t8(av); ci += 1
        vecs[:80, ci] = np.tile(bns[i], 5); ci += 1
        vecs[:80, ci] = np.tile(bnb_adj, 5); ci += 1
        vecs[:80, ci] = np.tile(av, 5); ci += 1
        av = 2.0 * bnas[i] * av
        bv = 2.0 * bnas[i] * bv + bna_b[i]
    # rebuild per-layer diag(av) for the PE-side xa accumulation
    avs = [np.ones(16, dtype=np.float32)]
    for i in range(L):
        avs.append(2.0 * bnas[i] * avs[-1])
    wav0 = np.stack([np.diag(np.tile(avs[i], 8)) for i in range(L)])
    wav1 = np.stack([np.diag(np.tile(avs[i], 5)) for i in range(L)])
    wav0 = wav0.astype(ml_dtypes.bfloat16)
    wav1 = wav1.astype(ml_dtypes.bfloat16)
    vecs[:64, ci] = end1_b; ci += 1
    vecs[:12, ci] = end2_b; ci += 1
    assert ci == NV_COLS

    shared = {
        "p2T": np.ascontiguousarray(p2.T),
        "p3sT": np.ascontiguousarray(p3[:DIMS, :DIMS].T),
        "wstart0": wstart0, "wstart1": wstart1,
        "wfc1_0": kr(e8, fc1_w), "wfc1_1": kr(e5, fc1_w),
        "wfc2_0": bf(kr(e8, fc2_w)), "wfc2_1": bf(kr(e5, fc2_w)),
        "wskip0": bf(wskip0), "wskip1": bf(wskip1),
        "wgc0": bf(wgc0), "wgc1": bf(wgc1),
        "we1": bf(we1), "we2": np.ascontiguousarray(end2_w.T),
        "idenb": np.eye(128, dtype=ml_dtypes.bfloat16),
        "idenr": np.eye(128, dtype=np.float32),
        "wav0": wav0, "wav1": wav1,
        "vecs": vecs,
    }
    in_maps = []
    for c in range(NCORES):
        bs = slice(c * BPC, (c + 1) * BPC)
        m = dict(shared)
        m["inp"] = np.ascontiguousarray(inp_t[bs])
        m["adp"] = np.ascontiguousarray(adp[bs])
        in_maps.append(m)
    return in_maps


def _get_nc():
    global _CACHED
    if _CACHED is None:
        _CACHED = _build_nc()
    return _CACHED


def run(inputs, trace=False):
    nc = _get_nc()
    in_maps = _prep_host(inputs)
    res = run_bass_kernel_spmd(nc, in_maps, core_ids=list(range(NCORES)),
                               trace=trace)
    out = np.stack([res.results[c]["outp"] for c in range(NCORES)])
    out = out.reshape(B, 12, N, 1).astype(np.float32)
    return out, res


def kernel(**inputs):
    out, _ = run(inputs)
    return out


# revision 20
# speedup vs baseline: 1.2036x; 1.2036x over previous
"""DMSTGCN forward on 8 Trainium2 NeuronCores (Bass/Tile).

Sharding: data-parallel over batch B=16 -> 2 batches per core; parameters
replicated. The dynamic adjacency (1024x1024 per batch) is built and kept in
SBUF (bf16); 1x1 convs run as block-diagonal (W (x) I) matmuls in an l-major
"[(time,chan), node]" layout, graph hops in "[node, (time,chan)]" layout with
PE transposes between the two. Trunk math is float32r (TF32-like), graph-hop
operands bf16. The two batches are emitted layer-interleaved, all heavy ops
are sliced per 512 nodes, and PSUM tiles are single-bank so the scheduler can
overlap the two batch streams.
"""
import numpy as np
import ml_dtypes

import concourse.bacc as bacc
import concourse.mybir as mybir
from concourse.tile import TileContext
from concourse.bass_utils import run_bass_kernel_spmd

F32 = mybir.dt.float32
F32R = mybir.dt.float32r
BF16 = mybir.dt.bfloat16
AF = mybir.ActivationFunctionType
ALU = mybir.AluOpType

B, N, T, RF = 16, 2, 1024, 12  # placeholder, fixed below
B, N, T, RF = 16, 1024, 12, 13
RC, SC, DIMS, L = 16, 8, 32, 8
BN_EPS = 1e-5
NCORES = 8
BPC = B // NCORES          # batches per core
CL = RC * RF               # 208 rows in T-layout
SKR = SC * RF              # 104 skip rows
CH = ((0, 128), (128, 80))  # l-major T-layout row chunks
NV_COLS = 4 + L + L * 2 * 3 + 2

_CACHED = None


def _build_nc():
    nc = bacc.Bacc("TRN2", target_bir_lowering=False)

    d = {}
    def din(name, shape, dt=F32R):
        d[name] = nc.dram_tensor(name, list(shape), dt, kind="ExternalInput")

    din("inp", (BPC, 2, RF, N))
    din("adp", (BPC, DIMS, DIMS))
    din("p2T", (DIMS, N))
    din("p3sT", (DIMS, DIMS))
    din("wstart0", (2, RF, 128))
    din("wstart1", (2, RF, 80))
    din("wfc1_0", (128, 128)); din("wfc1_1", (80, 80))
    din("wfc2_0", (128, 128), BF16); din("wfc2_1", (80, 80), BF16)
    din("wskip0", (L, 128, 64), BF16)
    din("wskip1", (L, 80, 40), BF16)
    din("wgc0", (L, 3, 128, 128), BF16)
    din("wgc1", (L, 3, 80, 80), BF16)
    din("we1", (L, SKR, 64), BF16)
    din("we2", (64, 12))
    din("idenb", (128, 128), BF16)
    din("idenr", (128, 128))
    din("wav0", (L, 128, 128), BF16)
    din("wav1", (L, 80, 80), BF16)
    din("vecs", (128, NV_COLS), F32)
    outp = nc.dram_tensor("outp", [BPC, 12, N], F32, kind="ExternalOutput")

    with TileContext(nc) as tc, \
         tc.tile_pool(name="wp", bufs=1) as wp, \
         tc.tile_pool(name="ap", bufs=1) as ap, \
         tc.tile_pool(name="pp", bufs=1, space="PSUM") as pp:

        def wtile(name, src_ap, shape, dt=F32R, eng=None):
            t = wp.tile(shape, dt, tag=name, name=name)
            (eng or nc.sync).dma_start(out=t[:], in_=src_ap)
            return t

        # phase0-critical loads go first on the SP queue; bulk weights on
        # gpsimd so PE can start within ~2us.
        p2T = wtile("p2T", d["p2T"][:], (DIMS, N), eng=nc.sync)
        p3sT = wtile("p3sT", d["p3sT"][:], (DIMS, DIMS), eng=nc.sync)
        adps = [wtile(f"adp{b}", d["adp"][b], (DIMS, DIMS), eng=nc.sync)
                for b in range(BPC)]

        inps = []
        for b in range(BPC):
            t0 = ap.tile((RF, N), F32R, tag="in0", name=f"in0_{b}")[:]
            t1 = ap.tile((RF, N), F32R, tag="in1", name=f"in1_{b}")[:]
            nc.sync.dma_start(out=t0, in_=d["inp"][b, 0])
            nc.sync.dma_start(out=t1, in_=d["inp"][b, 1])
            inps.append((t0, t1))

        idenb = wtile("idenb", d["idenb"][:], (128, 128), BF16)
        idenr = wtile("idenr", d["idenr"][:], (128, 128))
        wav = [[wtile(f"wav{i}_{c}", d[f"wav{c}"][i],
                      (CH[c][1], CH[c][1]), BF16) for c in range(2)]
               for i in range(L)]
        vecs = wtile("vecs", d["vecs"][:], (128, NV_COLS), F32)
        wstart = [[wtile(f"wst{s}_{c}", d[f"wstart{c}"][s],
                         (RF, CH[c][1])) for c in range(2)] for s in range(2)]
        wfc1 = [wtile(f"wfc1_{c}", d[f"wfc1_{c}"][:],
                      (CH[c][1], CH[c][1])) for c in range(2)]
        wfc2 = [wtile(f"wfc2_{c}", d[f"wfc2_{c}"][:],
                      (CH[c][1], CH[c][1]), BF16) for c in range(2)]
        wskip = [[wtile(f"wsk{i}_{c}", d[f"wskip{c}"][i],
                        (CH[c][1], (64, 40)[c]), BF16) for c in range(2)]
                 for i in range(L)]
        we1 = [wtile(f"we1_{i}", d["we1"][i], (SKR, 64), BF16) for i in range(L)]
        we2 = wtile("we2", d["we2"][:], (64, 12))

        vc = {}
        ci = 0
        for nm in ("sb0", "sb1", "sab0", "sab1"):
            vc[nm] = ci; ci += 1
        for i in range(L):
            vc[f"skb{i}"] = ci; ci += 1
        for i in range(L):
            for c in range(2):
                for nm in ("bns", "bnb", "av"):
                    vc[f"{nm}{i}_{c}"] = ci; ci += 1
        vc["e1b"] = ci; ci += 1
        vc["e2b"] = ci; ci += 1
        assert ci == NV_COLS

        def vcol(nm, rows=128):
            return vecs[:rows, vc[nm]:vc[nm] + 1]

        NS = (slice(0, 512), slice(512, 1024))

        st = [dict() for _ in range(BPC)]

        # ---------------- adjacency (both batches interleaved) ----------
        def phase0_pair():
            BS = range(BPC)
            # L-stack rows: [u; -srcT], R-stack rows: [srcT; u] so that
            # D = x1^T - x1 is ONE K=64 matmul per (v, ns).
            Lst = [ap.tile((64, N), F32R, tag=f"Lst{b}", name=f"Lst{b}")
                   for b in BS]
            Rst = [ap.tile((64, N), F32R, tag=f"Rst{b}", name=f"Rst{b}")
                   for b in BS]
            for nsi, ns in enumerate(NS):
                pss = []
                for b in BS:
                    ps = pp.tile((DIMS, 512), F32, tag="pwork", bufs=3,
                                 name=f"srcTps{b}_{nsi}")
                    nc.tensor.matmul(ps[:], adps[b][:], p2T[:, ns],
                                     start=True, stop=True)
                    pss.append(ps)
                for b in BS:
                    nc.scalar.activation(Rst[b][0:32, ns], pss[b][:], AF.Copy)
                    nc.vector.tensor_scalar(Lst[b][32:64, ns], pss[b][:],
                                            -1.0, None, ALU.mult)
            for nsi, ns in enumerate(NS):
                pss = []
                for b in BS:
                    ps = pp.tile((DIMS, 512), F32, tag="pwork", bufs=3,
                                 name=f"ups{b}_{nsi}")
                    nc.tensor.matmul(ps[:], p3sT[:], Rst[b][0:32, ns],
                                     start=True, stop=True)
                    pss.append(ps)
                for b in BS:
                    nc.scalar.activation(Lst[b][0:32, ns], pss[b][:], AF.Copy)
                    nc.vector.tensor_copy(Rst[b][32:64, ns], pss[b][:])
            st[0]["LR"] = (Lst, Rst)

        def phase0_D():
            BS = range(BPC)
            Lst, Rst = st[0]["LR"]
            ATs = [[ap.tile((128, N), BF16, tag=f"AT{b}_{v}", name=f"AT{b}_{v}")
                    for v in range(8)] for b in BS]
            Dts = [[ap.tile((128, N), BF16, tag=f"Dt{b}",
                            name=f"Dt{b}_{v}") for v in range(8)] for b in BS]
            for v in range(8):
                cs = slice(v * 128, (v + 1) * 128)
                for nsi, ns in enumerate(NS):
                    dpss = []
                    for b in BS:
                        dps = pp.tile((128, 512), F32, tag="pwork", bufs=3,
                                      name=f"dps{b}_{v}_{nsi}")
                        nc.tensor.matmul(dps[:], Lst[b][:, cs], Rst[b][:, ns],
                                         start=True, stop=True)
                        dpss.append(dps)
                    # relu(tanh(D)) == tanh(max(D, 0))
                    for b in BS:
                        if b % 2 == 0:
                            nc.vector.tensor_scalar(Dts[b][v][:, ns], dpss[b][:],
                                                    0.0, None, ALU.max)
                        else:
                            nc.scalar.activation(Dts[b][v][:, ns], dpss[b][:],
                                                 AF.Relu)
                    for b in BS:
                        nc.scalar.activation(ATs[b][v][:, ns], Dts[b][v][:, ns],
                                             AF.Tanh)
            for b in BS:
                st[b]["AT"] = ATs[b]

        # ---------------- start convs ----------------
        def start(b):
            in0, in1 = inps[b]
            xt, xa = [None, None], [None, None]
            for c in range(2):
                rows = CH[c][1]
                xt[c] = ap.tile((rows, N), F32R, tag=f"XT{b}_{c}", bufs=2,
                                name=f"XT{b}_{c}_init")
                xa[c] = ap.tile((rows, N), BF16, tag=f"XA{b}_{c}",
                                name=f"XA{b}_{c}")
                for nsi, ns in enumerate(NS):
                    ps = pp.tile((rows, 512), F32, tag="pwork", bufs=3,
                                 name=f"stp{b}_{c}_{nsi}")
                    nc.tensor.matmul(ps[:], wstart[0][c][:], in0[:, ns],
                                     start=True, stop=True)
                    nc.scalar.activation(xt[c][:, ns], ps[:], AF.Identity,
                                         bias=vcol(f"sb{c}", rows))
                    psa = pp.tile((rows, 512), F32, tag="pwork", bufs=3,
                                  name=f"stpa{b}_{c}_{nsi}")
                    nc.tensor.matmul(psa[:], wstart[1][c][:], in1[:, ns],
                                     start=True, stop=True)
                    nc.scalar.activation(xa[c][:, ns], psa[:], AF.Identity,
                                         bias=vcol(f"sab{c}", rows))
            st[b]["xt"], st[b]["xa"] = xt, xa
            st[b]["end"] = ap.tile((64, N), F32, tag=f"END{b}", name=f"END{b}")

        # ---------------- one layer, both batches stage-interleaved ----------
        def layer_pair(i):
            BS = range(BPC)
            xt = [st[b]["xt"] for b in BS]
            xa = [st[b]["xa"] for b in BS]
            AT = [st[b]["AT"] for b in BS]

            gcw = [[[ap.tile((CH[c][1], CH[c][1]), BF16, tag=f"gcw{b}_{c}_{s}",
                             bufs=2, name=f"gcw{b}_{i}_{c}_{s}")
                     for c in range(2)] for s in range(3)] for b in BS]
            for b in BS:
                for s in range(3):
                    for c in range(2):
                        nc.sync.dma_start(out=gcw[b][s][c][:],
                                          in_=d[f"wgc{c}"][i, s])

            # -- attention + sigmoid
            xn = [[None, None] for b in BS]
            r1 = [[None, None] for b in BS]
            sg = [[None, None] for b in BS]
            for b in BS:
                for c in range(2):
                    rows = CH[c][1]
                    r1[b][c] = ap.tile((rows, N), BF16, tag=f"R1{b}_{c}",
                                       name=f"R1{b}_{i}_{c}")
                    sg[b][c] = ap.tile((rows, N), F32, tag=f"tmp{b}_{c}",
                                       name=f"sg{b}_{i}_{c}")
                    xn[b][c] = ap.tile((rows, N), BF16, tag=f"XN{b}_{c}",
                                       name=f"XN{b}_{i}_{c}")
            for c in range(2):
                rows = CH[c][1]
                for nsi, ns in enumerate(NS):
                    m1s = []
                    for b in BS:
                        m1 = pp.tile((rows, 512), F32, tag="pwork", bufs=3,
                                     name=f"m1_{b}_{i}_{c}_{nsi}")
                        nc.tensor.matmul(m1[:], wfc1[c][:], xt[b][c][:, ns],
                                         start=True, stop=True)
                        m1s.append(m1)
                    for b in BS:
                        if b % 2 == 0:
                            nc.scalar.activation(r1[b][c][:, ns], m1s[b][:],
                                                 AF.Relu)
                        else:
                            nc.vector.tensor_scalar(r1[b][c][:, ns], m1s[b][:],
                                                    0.0, None, ALU.max)
                    aps = []
                    for b in BS:
                        a_ps = pp.tile((rows, 512), F32, tag="pwork", bufs=3,
                                       name=f"aps{b}_{i}_{c}_{nsi}")
                        nc.tensor.matmul(a_ps[:], wfc2[c][:], r1[b][c][:, ns],
                                         start=True, stop=True)
                        aps.append(a_ps)
                    for b in BS:
                        nc.vector.scalar_tensor_tensor(
                            sg[b][c][:, ns], aps[b][:], 2.0,
                            xt[b][c][:, ns].bitcast(F32), ALU.mult, ALU.add)
                    for b in BS:
                        nc.scalar.activation(xn[b][c][:, ns], sg[b][c][:, ns],
                                             AF.Sigmoid)

            # -- skip conv -> relu -> end1 matmul -> SBUF accumulator
            rsk = [ap.tile((SKR, N), BF16, tag=f"rsk{b}", name=f"rsk{b}_{i}")
                   for b in BS]
            for nsi, ns in enumerate(NS):
                sks = []
                for b in BS:
                    sk_ps = pp.tile((SKR, 512), F32, tag="pwork", bufs=3,
                                    name=f"skp{b}_{i}_{nsi}")
                    nc.tensor.matmul(sk_ps[:64], wskip[i][0][:],
                                     xn[b][0][:, ns], start=True, stop=True)
                    nc.tensor.matmul(sk_ps[64:], wskip[i][1][:],
                                     xn[b][1][:, ns], start=True, stop=True)
                    sks.append(sk_ps)
                for b in BS:
                    if b % 2 == 0:
                        nc.vector.tensor_scalar(rsk[b][:, ns], sks[b][:],
                                                vcol(f"skb{i}", SKR), 0.0,
                                                ALU.add, ALU.max)
                    else:
                        nc.scalar.activation(rsk[b][:, ns], sks[b][:], AF.Relu,
                                             bias=vcol(f"skb{i}", SKR))
                for b in BS:
                    e_ps = pp.tile((64, 512), F32, tag="pwork", bufs=3,
                                   name=f"eps{b}_{i}_{nsi}")
                    nc.tensor.matmul(e_ps[:], we1[i][:], rsk[b][:, ns],
                                     start=True, stop=True)
                    if i == 0:
                        nc.vector.tensor_copy(st[b]["end"][:, ns], e_ps[:])
                    else:
                        nc.vector.scalar_tensor_tensor(
                            st[b]["end"][:, ns], e_ps[:], 0.0,
                            st[b]["end"][:, ns], ALU.bypass, ALU.add)

            # -- V-layout of xn via PE transposes
            xv = [[None] * 8 for b in BS]
            for v in range(8):
                cs = slice(v * 128, (v + 1) * 128)
                for b in BS:
                    tp = pp.tile((128, CL), BF16, tag="ptr", bufs=3,
                                 name=f"tpx{b}_{i}_{v}")
                    for c in range(2):
                        o, rows = CH[c]
                        nc.tensor.transpose(tp[:, o:o + rows],
                                            xn[b][c][:, cs],
                                            idenb[:rows, :rows])
                    xv[b][v] = ap.tile((128, CL), BF16, tag=f"XV{b}_{v}",
                                       name=f"XV{b}_{i}_{v}")
                    nc.vector.tensor_copy(xv[b][v][:], tp[:])

            def hop(rv, nm):
                """A-hop (V-orientation, w-pairs) + transpose back, both b."""
                hvp = [[None] * 4 for b in BS]
                for p in range(4):
                    for b in BS:
                        h_ps = pp.tile((128, 2 * CL), F32, tag="ptr", bufs=3,
                                       name=f"hp{nm}{b}_{i}_{p}")
                        for half in range(2):
                            w = 2 * p + half
                            ws = slice(w * 128, (w + 1) * 128)
                            dst = h_ps[:, half * CL:(half + 1) * CL]
                            for k in range(8):
                                nc.tensor.matmul(dst, AT[b][k][:, ws], rv(b, k),
                                                 start=(k == 0), stop=(k == 7))
                        hvp[b][p] = ap.tile((128, 2 * CL), BF16,
                                            tag=f"{nm}V{b}_{p}",
                                            name=f"{nm}V{b}_{i}_{p}")
                        if (b + p) % 2 == 0:
                            nc.vector.tensor_copy(hvp[b][p][:], h_ps[:])
                        else:
                            nc.scalar.activation(hvp[b][p][:], h_ps[:], AF.Copy)

                ht = [[ap.tile((CH[c][1], N), BF16, tag=f"{nm}T{b}_{c}",
                               name=f"{nm}T{b}_{i}_{c}") for c in range(2)]
                      for b in BS]
                for b in BS:
                    tpb = [pp.tile((CH[c][1], N), BF16, tag=f"ptb{c}",
                                   bufs=1, name=f"tpb{nm}{b}_{i}_{c}")
                           for c in range(2)]
                    for w in range(8):
                        src = hvp[b][w // 2][:, (w % 2) * CL:(w % 2) * CL + CL]
                        for c in range(2):
                            o, rows = CH[c]
                            nc.tensor.transpose(
                                tpb[c][:, w * 128:(w + 1) * 128],
                                src[:, o:o + rows], idenb[:, :])
                        if w % 4 == 3:
                            half = slice((w - 3) * 128, (w + 1) * 128)
                            for c in range(2):
                                if (b + c) % 2 == 0:
                                    nc.scalar.activation(ht[b][c][:, half],
                                                         tpb[c][:, half],
                                                         AF.Copy)
                                else:
                                    nc.vector.tensor_copy(ht[b][c][:, half],
                                                          tpb[c][:, half])
                return hvp, ht

            h1vp, h1t = hop(lambda b, k: xv[b][k][:], "H1")
            _, h2t = hop(
                lambda b, k: h1vp[b][k // 2][:, (k % 2) * CL:(k % 2) * CL + CL],
                "H2")

            # -- gconv (block-diag over l); residual x and av*xa are
            # accumulated in PSUM via identity/diagonal matmuls, then a
            # single affine eviction applies the batchnorm.
            for c in range(2):
                rows = CH[c][1]
                nxt = [ap.tile((rows, N), F32R, tag=f"XT{b}_{c}", bufs=2,
                               name=f"XT{b}_{i}_{c}") for b in BS]
                for nsi, ns in enumerate(NS):
                    gps = []
                    for b in BS:
                        g_ps = pp.tile((rows, 512), F32, tag="pwork", bufs=3,
                                       name=f"gp{b}_{i}_{c}_{nsi}")
                        srcs = (xn[b], h1t[b], h2t[b])
                        for s in range(3):
                            nc.tensor.matmul(g_ps[:], gcw[b][s][c][:],
                                             srcs[s][c][:, ns],
                                             start=(s == 0), stop=False)
                        nc.tensor.matmul(g_ps[:], idenr[:rows, :rows],
                                         xt[b][c][:, ns],
                                         start=False, stop=False)
                        nc.tensor.matmul(g_ps[:], wav[i][c][:],
                                         xa[b][c][:, ns],
                                         start=False, stop=True)
                        gps.append(g_ps)
                    for b in BS:
                        if b % 2 == 0:
                            nc.scalar.activation(nxt[b][:, ns], gps[b][:],
                                                 AF.Identity,
                                                 bias=vcol(f"bnb{i}_{c}", rows),
                                                 scale=vcol(f"bns{i}_{c}", rows))
                        else:
                            nc.vector.tensor_scalar(
                                nxt[b][:, ns], gps[b][:],
                                vcol(f"bns{i}_{c}", rows),
                                vcol(f"bnb{i}_{c}", rows), ALU.mult, ALU.add)
                for b in BS:
                    xt[b][c] = nxt[b]

        # ---------------- end convs ----------------
        def tail(b):
            o1 = ap.tile((64, N), F32R, tag="o1", name=f"o1_{b}")
            ob = ap.tile((12, N), F32, tag="ob", name=f"ob{b}")
            for nsi, ns in enumerate(NS):
                nc.scalar.activation(o1[:, ns], st[b]["end"][:, ns], AF.Relu,
                                     bias=vcol("e1b", 64))
                o2_ps = pp.tile((12, 512), F32, tag="pwork", bufs=3,
                                name=f"o2p{b}_{nsi}")
                nc.tensor.matmul(o2_ps[:], we2[:], o1[:, ns],
                                 start=True, stop=True)
                nc.scalar.activation(ob[:, ns], o2_ps[:], AF.Identity,
                                     bias=vcol("e2b", 12))
            nc.sync.dma_start(out=outp[b], in_=ob[:])

        phase0_pair()
        for b in range(BPC):
            start(b)
        phase0_D()
        for i in range(L):
            layer_pair(i)
        for b in range(BPC):
            tail(b)

    nc.finalize()
    return nc


# ----------------------------------------------------------------------------
# host-side preprocessing
# ----------------------------------------------------------------------------

def _prep_host(inputs):
    f = lambda x: np.asarray(x, dtype=np.float32)
    bf = lambda x: np.ascontiguousarray(x).astype(ml_dtypes.bfloat16)
    x_in = f(inputs["inputs"])
    ind = np.asarray(inputs["ind"]).astype(np.int64)
    p1, p2, p3, pk = f(inputs["p1"]), f(inputs["p2"]), f(inputs["p3"]), f(inputs["pk"])

    xo = np.pad(x_in, ((0, 0), (0, 0), (0, 0), (RF - T, 0)))
    inp_t = np.ascontiguousarray(xo.transpose(0, 1, 3, 2))
    te = p1[ind]
    adp = np.einsum("bi,ijk->bjk", te, pk).astype(np.float32)

    start_w, start_b = f(inputs["start_w"]), f(inputs["start_b"])
    starta_w, starta_b = f(inputs["starta_w"]), f(inputs["starta_b"])
    fc1_w, fc2_w = f(inputs["fc1_w"]), f(inputs["fc2_w"])
    skip_w, skip_b = f(inputs["skip_w"]), f(inputs["skip_b"])
    gconv_w, gconv_b = f(inputs["gconv_w"]), f(inputs["gconv_b"])
    bn_g, bn_b = f(inputs["bn_g"]), f(inputs["bn_b"])
    bna_g, bna_b = f(inputs["bna_g"]), f(inputs["bna_b"])
    end1_w, end1_b = f(inputs["end1_w"]), f(inputs["end1_b"])
    end2_w, end2_b = f(inputs["end2_w"]), f(inputs["end2_b"])

    e8, e5 = np.eye(8, dtype=np.float32), np.eye(5, dtype=np.float32)
    e13 = np.eye(RF, dtype=np.float32)
    kr = lambda e, w: np.kron(e, np.ascontiguousarray(w.T)).astype(np.float32)

    wstart0 = np.stack([np.kron(e13[:, :8], w[:, 0][None, :])
                        for w in (start_w, starta_w)]).astype(np.float32)
    wstart1 = np.stack([np.kron(e13[:, 8:], w[:, 0][None, :])
                        for w in (start_w, starta_w)]).astype(np.float32)
    wgc0 = np.stack([np.stack([kr(e8, gconv_w[i][:, s * 16:(s + 1) * 16])
                               for s in range(3)]) for i in range(L)])
    wgc1 = np.stack([np.stack([kr(e5, gconv_w[i][:, s * 16:(s + 1) * 16])
                               for s in range(3)]) for i in range(L)])
    wskip0 = np.stack([kr(e8, skip_w[i]) for i in range(L)])
    wskip1 = np.stack([kr(e5, skip_w[i]) for i in range(L)])

    # end1 columns: ref skip rows are o*13+l within the (L-1-i)-th block;
    # ours are l*8+o
    we1 = np.zeros((L, SKR, 64), dtype=np.float32)
    ll, oo = np.meshgrid(np.arange(RF), np.arange(SC), indexing="ij")
    src_col = oo.ravel() * RF + ll.ravel()
    for i in range(L):
        we1[i] = end1_w[:, (L - 1 - i) * SKR + src_col].T

    t8 = lambda v: np.tile(v, 8)
    vecs = np.zeros((128, NV_COLS), dtype=np.float32)
    ci = 0
    vecs[:, ci] = t8(start_b); ci += 1
    vecs[:80, ci] = np.tile(start_b, 5); ci += 1
    vecs[:, ci] = t8(starta_b); ci += 1
    vecs[:80, ci] = np.tile(starta_b, 5); ci += 1
    for i in range(L):
        vecs[:SKR, ci] = np.tile(skip_b[i], RF); ci += 1
    bns = (bn_g / np.sqrt(1.0 + BN_EPS)).astype(np.float32)
    bnas = (bna_g / np.sqrt(1.0 + BN_EPS)).astype(np.float32)
    av = np.ones(16, dtype=np.float32)
    bv = np.zeros(16, dtype=np.float32)
    for i in range(L):
        bnb_adj = bn_b[i] + bns[i] * (gconv_b[i] + bv)
        vecs[:, ci] = t8(bns[i]); ci += 1
        vecs[:, ci] = t8(bnb_adj); ci += 1
        vecs[:, ci] = t8(av); ci += 1
        vecs[:80, ci] = np.tile(bns[i], 5); ci += 1
        vecs[:80, ci] = np.tile(bnb_adj, 5); ci += 1
        vecs[:80, ci] = np.tile(av, 5); ci += 1
        av = 2.0 * bnas[i] * av
        bv = 2.0 * bnas[i] * bv + bna_b[i]
    # rebuild per-layer diag(av) for the PE-side xa accumulation
    avs = [np.ones(16, dtype=np.float32)]
    for i in range(L):
        avs.append(2.0 * bnas[i] * avs[-1])
    wav0 = np.stack([np.diag(np.tile(avs[i], 8)) for i in range(L)])
    wav1 = np.stack([np.diag(np.tile(avs[i], 5)) for i in range(L)])
    wav0 = wav0.astype(ml_dtypes.bfloat16)
    wav1 = wav1.astype(ml_dtypes.bfloat16)
    vecs[:64, ci] = end1_b; ci += 1
    vecs[:12, ci] = end2_b; ci += 1
    assert ci == NV_COLS

    shared = {
        "p2T": np.ascontiguousarray(p2.T),
        "p3sT": np.ascontiguousarray(p3[:DIMS, :DIMS].T),
        "wstart0": wstart0, "wstart1": wstart1,
        "wfc1_0": kr(e8, fc1_w), "wfc1_1": kr(e5, fc1_w),
        "wfc2_0": bf(kr(e8, fc2_w)), "wfc2_1": bf(kr(e5, fc2_w)),
        "wskip0": bf(wskip0), "wskip1": bf(wskip1),
        "wgc0": bf(wgc0), "wgc1": bf(wgc1),
        "we1": bf(we1), "we2": np.ascontiguousarray(end2_w.T),
        "idenb": np.eye(128, dtype=ml_dtypes.bfloat16),
        "idenr": np.eye(128, dtype=np.float32),
        "wav0": wav0, "wav1": wav1,
        "vecs": vecs,
    }
    in_maps = []
    for c in range(NCORES):
        bs = slice(c * BPC, (c + 1) * BPC)
        m = dict(shared)
        m["inp"] = np.ascontiguousarray(inp_t[bs])
        m["adp"] = np.ascontiguousarray(adp[bs])
        in_maps.append(m)
    return in_maps


def _get_nc():
    global _CACHED
    if _CACHED is None:
        _CACHED = _build_nc()
    return _CACHED


def run(inputs, trace=False):
    nc = _get_nc()
    in_maps = _prep_host(inputs)
    res = run_bass_kernel_spmd(nc, in_maps, core_ids=list(range(NCORES)),
                               trace=trace)
    out = np.stack([res.results[c]["outp"] for c in range(NCORES)])
    out = out.reshape(B, 12, N, 1).astype(np.float32)
    return out, res


def kernel(**inputs):
    out, _ = run(inputs)
    return out


# revision 25
# speedup vs baseline: 3247.9746x; 2698.5257x over previous
"""DMSTGCN forward on 8 Trainium2 NeuronCores (Bass/Tile).

Sharding: data-parallel over batch B=16 -> 2 batches per core; parameters
replicated. The dynamic adjacency (1024x1024 per batch) is built and kept in
SBUF (bf16); 1x1 convs run as block-diagonal (W (x) I) matmuls in an l-major
"[(time,chan), node]" layout, graph hops in "[node, (time,chan)]" layout with
PE transposes between the two. Trunk math is float32r (TF32-like), graph-hop
operands bf16. The two batches are emitted layer-interleaved, all heavy ops
are sliced per 512 nodes, and PSUM tiles are single-bank so the scheduler can
overlap the two batch streams.
"""
import numpy as np
import ml_dtypes

import concourse.bacc as bacc
import concourse.mybir as mybir
from concourse.tile import TileContext
from concourse.bass_utils import run_bass_kernel_spmd

F32 = mybir.dt.float32
F32R = mybir.dt.float32r
BF16 = mybir.dt.bfloat16
AF = mybir.ActivationFunctionType
ALU = mybir.AluOpType

B, N, T, RF = 16, 2, 1024, 12  # placeholder, fixed below
B, N, T, RF = 16, 1024, 12, 13
RC, SC, DIMS, L = 16, 8, 32, 8
BN_EPS = 1e-5
NCORES = 8
BPC = B // NCORES          # batches per core
CL = RC * RF               # 208 rows in T-layout
SKR = SC * RF              # 104 skip rows
CH = ((0, 128), (128, 80))  # l-major T-layout row chunks
NV_COLS = 4 + L + L * 2 * 3 + 2

_CACHED = None


def _build_nc():
    nc = bacc.Bacc("TRN2", target_bir_lowering=False)

    d = {}
    def din(name, shape, dt=F32R):
        d[name] = nc.dram_tensor(name, list(shape), dt, kind="ExternalInput")

    din("inp", (BPC, 2, RF, N))
    din("adp", (BPC, DIMS, DIMS))
    din("p2T", (DIMS, N))
    din("p3sT", (DIMS, DIMS))
    din("wstart0", (2, RF, 128))
    din("wstart1", (2, RF, 80))
    din("wfc1_0", (128, 128)); din("wfc1_1", (80, 80))
    din("wfc2_0", (128, 128), BF16); din("wfc2_1", (80, 80), BF16)
    din("wskip0", (L, 128, 64), BF16)
    din("wskip1", (L, 80, 40), BF16)
    din("wgc0", (L, 3, 128, 128), BF16)
    din("wgc1", (L, 3, 80, 80), BF16)
    din("we1", (L, SKR, 64), BF16)
    din("we2", (64, 12))
    din("idenb", (128, 128), BF16)
    din("idenr", (128, 128))
    din("wav0", (L, 128, 128), BF16)
    din("wav1", (L, 80, 80), BF16)
    din("vecs", (128, NV_COLS), F32)
    outp = nc.dram_tensor("outp", [BPC, 12, N], F32, kind="ExternalOutput")

    with TileContext(nc) as tc, \
         tc.tile_pool(name="wp", bufs=1) as wp, \
         tc.tile_pool(name="ap", bufs=1) as ap, \
         tc.tile_pool(name="pp", bufs=1, space="PSUM") as pp:

        def wtile(name, src_ap, shape, dt=F32R, eng=None):
            t = wp.tile(shape, dt, tag=name, name=name)
            (eng or nc.sync).dma_start(out=t[:], in_=src_ap)
            return t

        # phase0-critical loads go first on the SP queue; bulk weights on
        # gpsimd so PE can start within ~2us.
        p2T = wtile("p2T", d["p2T"][:], (DIMS, N), eng=nc.sync)
        p3sT = wtile("p3sT", d["p3sT"][:], (DIMS, DIMS), eng=nc.sync)
        adps = [wtile(f"adp{b}", d["adp"][b], (DIMS, DIMS), eng=nc.sync)
                for b in range(BPC)]

        inps = []
        for b in range(BPC):
            t0 = ap.tile((RF, N), F32R, tag="in0", name=f"in0_{b}")[:]
            t1 = ap.tile((RF, N), F32R, tag="in1", name=f"in1_{b}")[:]
            nc.sync.dma_start(out=t0, in_=d["inp"][b, 0])
            nc.sync.dma_start(out=t1, in_=d["inp"][b, 1])
            inps.append((t0, t1))

        idenb = wtile("idenb", d["idenb"][:], (128, 128), BF16)
        idenr = wtile("idenr", d["idenr"][:], (128, 128))
        wav = [[wtile(f"wav{i}_{c}", d[f"wav{c}"][i],
                      (CH[c][1], CH[c][1]), BF16) for c in range(2)]
               for i in range(L)]
        vecs = wtile("vecs", d["vecs"][:], (128, NV_COLS), F32)
        wstart = [[wtile(f"wst{s}_{c}", d[f"wstart{c}"][s],
                         (RF, CH[c][1])) for c in range(2)] for s in range(2)]
        wfc1 = [wtile(f"wfc1_{c}", d[f"wfc1_{c}"][:],
                      (CH[c][1], CH[c][1])) for c in range(2)]
        wfc2 = [wtile(f"wfc2_{c}", d[f"wfc2_{c}"][:],
                      (CH[c][1], CH[c][1]), BF16) for c in range(2)]
        wskip = [[wtile(f"wsk{i}_{c}", d[f"wskip{c}"][i],
                        (CH[c][1], (64, 40)[c]), BF16) for c in range(2)]
                 for i in range(L)]
        we1 = [wtile(f"we1_{i}", d["we1"][i], (SKR, 64), BF16) for i in range(L)]
        we2 = wtile("we2", d["we2"][:], (64, 12))

        vc = {}
        ci = 0
        for nm in ("sb0", "sb1", "sab0", "sab1"):
            vc[nm] = ci; ci += 1
        for i in range(L):
            vc[f"skb{i}"] = ci; ci += 1
        for i in range(L):
            for c in range(2):
                for nm in ("bns", "bnb", "av"):
                    vc[f"{nm}{i}_{c}"] = ci; ci += 1
        vc["e1b"] = ci; ci += 1
        vc["e2b"] = ci; ci += 1
        assert ci == NV_COLS

        def vcol(nm, rows=128):
            return vecs[:rows, vc[nm]:vc[nm] + 1]

        NS = (slice(0, 512), slice(512, 1024))

        st = [dict() for _ in range(BPC)]

        # ---------------- adjacency (both batches interleaved) ----------
        def phase0_pair():
            BS = range(BPC)
            # L-stack rows: [u; -srcT], R-stack rows: [srcT; u] so that
            # D = x1^T - x1 is ONE K=64 matmul per (v, ns).
            Lst = [ap.tile((64, N), F32R, tag=f"Lst{b}", name=f"Lst{b}")
                   for b in BS]
            Rst = [ap.tile((64, N), F32R, tag=f"Rst{b}", name=f"Rst{b}")
                   for b in BS]
            for nsi, ns in enumerate(NS):
                pss = []
                for b in BS:
                    ps = pp.tile((DIMS, 512), F32, tag="pwork", bufs=3,
                                 name=f"srcTps{b}_{nsi}")
                    nc.tensor.matmul(ps[:], adps[b][:], p2T[:, ns],
                                     start=True, stop=True)
                    pss.append(ps)
                for b in BS:
                    nc.scalar.activation(Rst[b][0:32, ns], pss[b][:], AF.Copy)
                    nc.vector.tensor_scalar(Lst[b][32:64, ns], pss[b][:],
                                            -1.0, None, ALU.mult)
            for nsi, ns in enumerate(NS):
                pss = []
                for b in BS:
                    ps = pp.tile((DIMS, 512), F32, tag="pwork", bufs=3,
                                 name=f"ups{b}_{nsi}")
                    nc.tensor.matmul(ps[:], p3sT[:], Rst[b][0:32, ns],
                                     start=True, stop=True)
                    pss.append(ps)
                for b in BS:
                    nc.scalar.activation(Lst[b][0:32, ns], pss[b][:], AF.Copy)
                    nc.vector.tensor_copy(Rst[b][32:64, ns], pss[b][:])
            st[0]["LR"] = (Lst, Rst)

        def phase0_D(fillers=()):
            BS = range(BPC)
            fillers = list(fillers)
            Lst, Rst = st[0]["LR"]
            ATs = [[ap.tile((128, N), BF16, tag=f"AT{b}_{v}", name=f"AT{b}_{v}")
                    for v in range(8)] for b in BS]
            Dts = [[ap.tile((128, N), BF16, tag=f"Dt{b}",
                            name=f"Dt{b}_{v}") for v in range(8)] for b in BS]
            for v in range(8):
                cs = slice(v * 128, (v + 1) * 128)
                for nsi, ns in enumerate(NS):
                    dpss = []
                    for b in BS:
                        dps = pp.tile((128, 512), F32, tag="pwork", bufs=3,
                                      name=f"dps{b}_{v}_{nsi}")
                        nc.tensor.matmul(dps[:], Lst[b][:, cs], Rst[b][:, ns],
                                         start=True, stop=True)
                        dpss.append(dps)
                    # relu(tanh(D)) == tanh(max(D, 0)); relu on DVE so the
                    # tanh-bound ACT queue stays short
                    for b in BS:
                        nc.vector.tensor_scalar(Dts[b][v][:, ns], dpss[b][:],
                                                0.0, None, ALU.max)
                    for b in BS:
                        nc.scalar.activation(ATs[b][v][:, ns], Dts[b][v][:, ns],
                                             AF.Tanh)
                if v % 2 == 1 and fillers:
                    fillers.pop(0)()
            while fillers:
                fillers.pop(0)()
            for b in BS:
                st[b]["AT"] = ATs[b]

        # ---------------- start convs (emitted as fillers in phase0_D) ----
        def start(b):
            in0, in1 = inps[b]
            xt, xa = [None, None], [None, None]
            fillers = []
            for c in range(2):
                rows = CH[c][1]
                xt[c] = ap.tile((rows, N), F32R, tag=f"XT{b}_{c}", bufs=2,
                                name=f"XT{b}_{c}_init")
                xa[c] = ap.tile((rows, N), BF16, tag=f"XA{b}_{c}",
                                name=f"XA{b}_{c}")
                def mk(c, xtt, xat):
                    rows = CH[c][1]
                    def emit():
                        for nsi, ns in enumerate(NS):
                            ps = pp.tile((rows, 512), F32, tag="pwork", bufs=3,
                                         name=f"stp{b}_{c}_{nsi}")
                            nc.tensor.matmul(ps[:], wstart[0][c][:],
                                             in0[:, ns], start=True, stop=True)
                            nc.scalar.activation(xtt[:, ns], ps[:], AF.Identity,
                                                 bias=vcol(f"sb{c}", rows))
                            psa = pp.tile((rows, 512), F32, tag="pwork",
                                          bufs=3, name=f"stpa{b}_{c}_{nsi}")
                            nc.tensor.matmul(psa[:], wstart[1][c][:],
                                             in1[:, ns], start=True, stop=True)
                            nc.scalar.activation(xat[:, ns], psa[:],
                                                 AF.Identity,
                                                 bias=vcol(f"sab{c}", rows))
                    return emit
                fillers.append(mk(c, xt[c], xa[c]))
            st[b]["xt"], st[b]["xa"] = xt, xa
            st[b]["end"] = ap.tile((64, N), F32, tag=f"END{b}", name=f"END{b}")
            return fillers

        # ---------------- one layer, both batches stage-interleaved ----------
        def layer_pair(i):
            BS = range(BPC)
            xt = [st[b]["xt"] for b in BS]
            xa = [st[b]["xa"] for b in BS]
            AT = [st[b]["AT"] for b in BS]

            gcw = [[[ap.tile((CH[c][1], CH[c][1]), BF16, tag=f"gcw{b}_{c}_{s}",
                             bufs=2, name=f"gcw{b}_{i}_{c}_{s}")
                     for c in range(2)] for s in range(3)] for b in BS]
            for b in BS:
                for s in range(3):
                    for c in range(2):
                        nc.sync.dma_start(out=gcw[b][s][c][:],
                                          in_=d[f"wgc{c}"][i, s])

            # -- attention + sigmoid
            xn = [[None, None] for b in BS]
            r1 = [[None, None] for b in BS]
            sg = [[None, None] for b in BS]
            for b in BS:
                for c in range(2):
                    rows = CH[c][1]
                    r1[b][c] = ap.tile((rows, N), BF16, tag=f"R1{b}_{c}",
                                       name=f"R1{b}_{i}_{c}")
                    sg[b][c] = ap.tile((rows, N), F32, tag=f"tmp{b}_{c}",
                                       name=f"sg{b}_{i}_{c}")
                    xn[b][c] = ap.tile((rows, N), BF16, tag=f"XN{b}_{c}",
                                       name=f"XN{b}_{i}_{c}")
            groups = [(c, nsi) for c in range(2) for nsi in range(2)]
            m1s, apss = {}, {}
            for c, nsi in groups:
                rows, ns = CH[c][1], NS[nsi]
                for b in BS:
                    m1 = pp.tile((rows, 512), F32, tag="pwork", bufs=3,
                                 name=f"m1_{b}_{i}_{c}_{nsi}")
                    nc.tensor.matmul(m1[:], wfc1[c][:], xt[b][c][:, ns],
                                     start=True, stop=True)
                    m1s[b, c, nsi] = m1
                for b in BS:
                    if b % 2 == 0:
                        nc.scalar.activation(r1[b][c][:, ns], m1s[b, c, nsi][:],
                                             AF.Relu)
                    else:
                        nc.vector.tensor_scalar(r1[b][c][:, ns],
                                                m1s[b, c, nsi][:],
                                                0.0, None, ALU.max)
            for c, nsi in groups:
                rows, ns = CH[c][1], NS[nsi]
                for b in BS:
                    a_ps = pp.tile((rows, 512), F32, tag="pwork", bufs=3,
                                   name=f"aps{b}_{i}_{c}_{nsi}")
                    nc.tensor.matmul(a_ps[:], wfc2[c][:], r1[b][c][:, ns],
                                     start=True, stop=True)
                    apss[b, c, nsi] = a_ps
                for b in BS:
                    nc.vector.scalar_tensor_tensor(
                        sg[b][c][:, ns], apss[b, c, nsi][:], 2.0,
                        xt[b][c][:, ns].bitcast(F32), ALU.mult, ALU.add)
                for b in BS:
                    nc.scalar.activation(xn[b][c][:, ns], sg[b][c][:, ns],
                                         AF.Sigmoid)

            # -- skip conv -> relu -> end1 matmul -> SBUF accumulator
            rsk = [ap.tile((SKR, N), BF16, tag=f"rsk{b}", name=f"rsk{b}_{i}")
                   for b in BS]
            sks = {}
            for nsi, ns in enumerate(NS):
                for b in BS:
                    sk_ps = pp.tile((SKR, 512), F32, tag="pwork", bufs=3,
                                    name=f"skp{b}_{i}_{nsi}")
                    nc.tensor.matmul(sk_ps[:64], wskip[i][0][:],
                                     xn[b][0][:, ns], start=True, stop=True)
                    nc.tensor.matmul(sk_ps[64:], wskip[i][1][:],
                                     xn[b][1][:, ns], start=True, stop=True)
                    sks[b, nsi] = sk_ps
                for b in BS:
                    if b % 2 == 0:
                        nc.vector.tensor_scalar(rsk[b][:, ns], sks[b, nsi][:],
                                                vcol(f"skb{i}", SKR), 0.0,
                                                ALU.add, ALU.max)
                    else:
                        nc.scalar.activation(rsk[b][:, ns], sks[b, nsi][:],
                                             AF.Relu, bias=vcol(f"skb{i}", SKR))
            for nsi, ns in enumerate(NS):
                for b in BS:
                    e_ps = pp.tile((64, 512), F32, tag="pwork", bufs=3,
                                   name=f"eps{b}_{i}_{nsi}")
                    nc.tensor.matmul(e_ps[:], we1[i][:], rsk[b][:, ns],
                                     start=True, stop=True)
                    if i == 0:
                        nc.vector.tensor_copy(st[b]["end"][:, ns], e_ps[:])
                    else:
                        nc.vector.scalar_tensor_tensor(
                            st[b]["end"][:, ns], e_ps[:], 0.0,
                            st[b]["end"][:, ns], ALU.bypass, ALU.add)

            # -- V-layout of xn via PE transposes
            xv = [[None] * 8 for b in BS]
            for v in range(8):
                cs = slice(v * 128, (v + 1) * 128)
                for b in BS:
                    tp = pp.tile((128, CL), BF16, tag="ptr", bufs=3,
                                 name=f"tpx{b}_{i}_{v}")
                    for c in range(2):
                        o, rows = CH[c]
                        nc.tensor.transpose(tp[:, o:o + rows],
                                            xn[b][c][:, cs],
                                            idenb[:rows, :rows])
                    xv[b][v] = ap.tile((128, CL), BF16, tag=f"XV{b}_{v}",
                                       name=f"XV{b}_{i}_{v}")
                    nc.vector.tensor_copy(xv[b][v][:], tp[:])

            def hop(rv, nm):
                """A-hop (V-orientation, w-pairs) + transpose back, both b."""
                hvp = [[None] * 4 for b in BS]
                for p in range(4):
                    for b in BS:
                        h_ps = pp.tile((128, 2 * CL), F32, tag="ptr", bufs=3,
                                       name=f"hp{nm}{b}_{i}_{p}")
                        for half in range(2):
                            w = 2 * p + half
                            ws = slice(w * 128, (w + 1) * 128)
                            dst = h_ps[:, half * CL:(half + 1) * CL]
                            for k in range(8):
                                nc.tensor.matmul(dst, AT[b][k][:, ws], rv(b, k),
                                                 start=(k == 0), stop=(k == 7))
                        hvp[b][p] = ap.tile((128, 2 * CL), BF16,
                                            tag=f"{nm}V{b}_{p}",
                                            name=f"{nm}V{b}_{i}_{p}")
                        if (b + p) % 2 == 0:
                            nc.vector.tensor_copy(hvp[b][p][:], h_ps[:])
                        else:
                            nc.scalar.activation(hvp[b][p][:], h_ps[:], AF.Copy)

                ht = [[ap.tile((CH[c][1], N), BF16, tag=f"{nm}T{b}_{c}",
                               name=f"{nm}T{b}_{i}_{c}") for c in range(2)]
                      for b in BS]
                for b in BS:
                    tpb = [pp.tile((CH[c][1], N), BF16, tag=f"ptb{c}",
                                   bufs=1, name=f"tpb{nm}{b}_{i}_{c}")
                           for c in range(2)]
                    for w in range(8):
                        src = hvp[b][w // 2][:, (w % 2) * CL:(w % 2) * CL + CL]
                        for c in range(2):
                            o, rows = CH[c]
                            nc.tensor.transpose(
                                tpb[c][:, w * 128:(w + 1) * 128],
                                src[:, o:o + rows], idenb[:, :])
                        if w % 4 == 3:
                            half = slice((w - 3) * 128, (w + 1) * 128)
                            for c in range(2):
                                if (b + c) % 2 == 0:
                                    nc.scalar.activation(ht[b][c][:, half],
                                                         tpb[c][:, half],
                                                         AF.Copy)
                                else:
                                    nc.vector.tensor_copy(ht[b][c][:, half],
                                                          tpb[c][:, half])
                return hvp, ht

            h1vp, h1t = hop(lambda b, k: xv[b][k][:], "H1")
            _, h2t = hop(
                lambda b, k: h1vp[b][k // 2][:, (k % 2) * CL:(k % 2) * CL + CL],
                "H2")

            # -- gconv (block-diag over l); residual x and av*xa are
            # accumulated in PSUM via identity/diagonal matmuls, then a
            # single affine eviction applies the batchnorm.
            for c in range(2):
                rows = CH[c][1]
                nxt = [ap.tile((rows, N), F32R, tag=f"XT{b}_{c}", bufs=2,
                               name=f"XT{b}_{i}_{c}") for b in BS]
                for nsi, ns in enumerate(NS):
                    gps = []
                    for b in BS:
                        g_ps = pp.tile((rows, 512), F32, tag="pwork", bufs=3,
                                       name=f"gp{b}_{i}_{c}_{nsi}")
                        srcs = (xn[b], h1t[b], h2t[b])
                        for s in range(3):
                            nc.tensor.matmul(g_ps[:], gcw[b][s][c][:],
                                             srcs[s][c][:, ns],
                                             start=(s == 0), stop=False)
                        nc.tensor.matmul(g_ps[:], idenr[:rows, :rows],
                                         xt[b][c][:, ns],
                                         start=False, stop=False)
                        nc.tensor.matmul(g_ps[:], wav[i][c][:],
                                         xa[b][c][:, ns],
                                         start=False, stop=True)
                        gps.append(g_ps)
                    for b in BS:
                        if b % 2 == 0:
                            nc.scalar.activation(nxt[b][:, ns], gps[b][:],
                                                 AF.Identity,
                                                 bias=vcol(f"bnb{i}_{c}", rows),
                                                 scale=vcol(f"bns{i}_{c}", rows))
                        else:
                            nc.vector.tensor_scalar(
                                nxt[b][:, ns], gps[b][:],
                                vcol(f"bns{i}_{c}", rows),
                                vcol(f"bnb{i}_{c}", rows), ALU.mult, ALU.add)
                for b in BS:
                    xt[b][c] = nxt[b]

        # ---------------- end convs ----------------
        def tail(b):
            o1 = ap.tile((64, N), F32R, tag="o1", name=f"o1_{b}")
            ob = ap.tile((12, N), F32, tag="ob", name=f"ob{b}")
            for nsi, ns in enumerate(NS):
                nc.scalar.activation(o1[:, ns], st[b]["end"][:, ns], AF.Relu,
                                     bias=vcol("e1b", 64))
                o2_ps = pp.tile((12, 512), F32, tag="pwork", bufs=3,
                                name=f"o2p{b}_{nsi}")
                nc.tensor.matmul(o2_ps[:], we2[:], o1[:, ns],
                                 start=True, stop=True)
                nc.scalar.activation(ob[:, ns], o2_ps[:], AF.Identity,
                                     bias=vcol("e2b", 12))
            nc.sync.dma_start(out=outp[b], in_=ob[:])

        phase0_pair()
        fillers = []
        for b in range(BPC):
            fillers.extend(start(b))
        phase0_D(fillers)
        for i in range(L):
            layer_pair(i)
        for b in range(BPC):
            tail(b)

    nc.finalize()
    return nc


# ----------------------------------------------------------------------------
# host-side preprocessing
# ----------------------------------------------------------------------------

def _prep_host(inputs):
    f = lambda x: np.asarray(x, dtype=np.float32)
    bf = lambda x: np.ascontiguousarray(x).astype(ml_dtypes.bfloat16)
    x_in = f(inputs["inputs"])
    ind = np.asarray(inputs["ind"]).astype(np.int64)
    p1, p2, p3, pk = f(inputs["p1"]), f(inputs["p2"]), f(inputs["p3"]), f(inputs["pk"])

    xo = np.pad(x_in, ((0, 0), (0, 0), (0, 0), (RF - T, 0)))
    inp_t = np.ascontiguousarray(xo.transpose(0, 1, 3, 2))
    te = p1[ind]
    adp = np.einsum("bi,ijk->bjk", te, pk).astype(np.float32)

    start_w, start_b = f(inputs["start_w"]), f(inputs["start_b"])
    starta_w, starta_b = f(inputs["starta_w"]), f(inputs["starta_b"])
    fc1_w, fc2_w = f(inputs["fc1_w"]), f(inputs["fc2_w"])
    skip_w, skip_b = f(inputs["skip_w"]), f(inputs["skip_b"])
    gconv_w, gconv_b = f(inputs["gconv_w"]), f(inputs["gconv_b"])
    bn_g, bn_b = f(inputs["bn_g"]), f(inputs["bn_b"])
    bna_g, bna_b = f(inputs["bna_g"]), f(inputs["bna_b"])
    end1_w, end1_b = f(inputs["end1_w"]), f(inputs["end1_b"])
    end2_w, end2_b = f(inputs["end2_w"]), f(inputs["end2_b"])

    e8, e5 = np.eye(8, dtype=np.float32), np.eye(5, dtype=np.float32)
    e13 = np.eye(RF, dtype=np.float32)
    kr = lambda e, w: np.kron(e, np.ascontiguousarray(w.T)).astype(np.float32)

    wstart0 = np.stack([np.kron(e13[:, :8], w[:, 0][None, :])
                        for w in (start_w, starta_w)]).astype(np.float32)
    wstart1 = np.stack([np.kron(e13[:, 8:], w[:, 0][None, :])
                        for w in (start_w, starta_w)]).astype(np.float32)
    wgc0 = np.stack([np.stack([kr(e8, gconv_w[i][:, s * 16:(s + 1) * 16])
                               for s in range(3)]) for i in range(L)])
    wgc1 = np.stack([np.stack([kr(e5, gconv_w[i][:, s * 16:(s + 1) * 16])
                               for s in range(3)]) for i in range(L)])
    wskip0 = np.stack([kr(e8, skip_w[i]) for i in range(L)])
    wskip1 = np.stack([kr(e5, skip_w[i]) for i in range(L)])

    # end1 columns: ref skip rows are o*13+l within the (L-1-i)-th block;
    # ours are l*8+o
    we1 = np.zeros((L, SKR, 64), dtype=np.float32)
    ll, oo = np.meshgrid(np.arange(RF), np.arange(SC), indexing="ij")
    src_col = oo.ravel() * RF + ll.ravel()
    for i in range(L):
        we1[i] = end1_w[:, (L - 1 - i) * SKR + src_col].T

    t8 = lambda v: np.tile(v, 8)
    vecs = np.zeros((128, NV_COLS), dtype=np.float32)
    ci = 0
    vecs[:, ci] = t8(start_b); ci += 1
    vecs[:80, ci] = np.tile(start_b, 5); ci += 1
    vecs[:, ci] = t8(starta_b); ci += 1
    vecs[:80, ci] = np.tile(starta_b, 5); ci += 1
    for i in range(L):
        vecs[:SKR, ci] = np.tile(skip_b[i], RF); ci += 1
    bns = (bn_g / np.sqrt(1.0 + BN_EPS)).astype(np.float32)
    bnas = (bna_g / np.sqrt(1.0 + BN_EPS)).astype(np.float32)
    av = np.ones(16, dtype=np.float32)
    bv = np.zeros(16, dtype=np.float32)
    for i in range(L):
        bnb_adj = bn_b[i] + bns[i] * (gconv_b[i] + bv)
        vecs[:, ci] = t8(bns[i]); ci += 1
        vecs[:, ci] = t8(bnb_adj); ci += 1
        vecs[:, ci] = t8(av); ci += 1
        vecs[:80, ci] = np.tile(bns[i], 5); ci += 1
        vecs[:80, ci] = np.tile(bnb_adj, 5); ci += 1
        vecs[:80, ci] = np.tile(av, 5); ci += 1
        av = 2.0 * bnas[i] * av
        bv = 2.0 * bnas[i] * bv + bna_b[i]
    # rebuild per-layer diag(av) for the PE-side xa accumulation
    avs = [np.ones(16, dtype=np.float32)]
    for i in range(L):
        avs.append(2.0 * bnas[i] * avs[-1])
    wav0 = np.stack([np.diag(np.tile(avs[i], 8)) for i in range(L)])
    wav1 = np.stack([np.diag(np.tile(avs[i], 5)) for i in range(L)])
    wav0 = wav0.astype(ml_dtypes.bfloat16)
    wav1 = wav1.astype(ml_dtypes.bfloat16)
    vecs[:64, ci] = end1_b; ci += 1
    vecs[:12, ci] = end2_b; ci += 1
    assert ci == NV_COLS

    shared = {
        "p2T": np.ascontiguousarray(p2.T),
        "p3sT": np.ascontiguousarray(p3[:DIMS, :DIMS].T),
        "wstart0": wstart0, "wstart1": wstart1,
        "wfc1_0": kr(e8, fc1_w), "wfc1_1": kr(e5, fc1_w),
        "wfc2_0": bf(kr(e8, fc2_w)), "wfc2_1": bf(kr(e5, fc2_w)),
        "wskip0": bf(wskip0), "wskip1": bf(wskip1),
        "wgc0": bf(wgc0), "wgc1": bf(wgc1),
        "we1": bf(we1), "we2": np.ascontiguousarray(end2_w.T),
        "idenb": np.eye(128, dtype=ml_dtypes.bfloat16),
        "idenr": np.eye(128, dtype=np.float32),
        "wav0": wav0, "wav1": wav1,
        "vecs": vecs,
    }
    in_maps = []
    for c in range(NCORES):
        bs = slice(c * BPC, (c + 1) * BPC)
        m = dict(shared)
        m["inp"] = np.ascontiguousarray(inp_t[bs])
        m["adp"] = np.ascontiguousarray(adp[bs])
        in_maps.append(m)
    return in_maps


def _get_nc():
    global _CACHED
    if _CACHED is None:
        _CACHED = _build_nc()
    return _CACHED


def run(inputs, trace=False):
    nc = _get_nc()
    in_maps = _prep_host(inputs)
    res = run_bass_kernel_spmd(nc, in_maps, core_ids=list(range(NCORES)),
                               trace=trace)
    out = np.stack([res.results[c]["outp"] for c in range(NCORES)])
    out = out.reshape(B, 12, N, 1).astype(np.float32)
    return out, res


def kernel(**inputs):
    out, _ = run(inputs)
    return out


# revision 28
# speedup vs baseline: 3317.6050x; 1.0214x over previous
"""DMSTGCN forward on 8 Trainium2 NeuronCores (Bass/Tile).

Sharding: data-parallel over batch B=16 -> 2 batches per core; parameters
replicated. The dynamic adjacency (1024x1024 per batch) is built and kept in
SBUF (bf16); 1x1 convs run as block-diagonal (W (x) I) matmuls in an l-major
"[(time,chan), node]" layout, graph hops in "[node, (time,chan)]" layout with
PE transposes between the two. Trunk math is float32r (TF32-like), graph-hop
operands bf16. The two batches are emitted layer-interleaved, all heavy ops
are sliced per 512 nodes, and PSUM tiles are single-bank so the scheduler can
overlap the two batch streams.
"""
import numpy as np
import ml_dtypes

import concourse.bacc as bacc
import concourse.mybir as mybir
from concourse.tile import TileContext
from concourse.bass_utils import run_bass_kernel_spmd

F32 = mybir.dt.float32
F32R = mybir.dt.float32r
BF16 = mybir.dt.bfloat16
AF = mybir.ActivationFunctionType
ALU = mybir.AluOpType

B, N, T, RF = 16, 2, 1024, 12  # placeholder, fixed below
B, N, T, RF = 16, 1024, 12, 13
RC, SC, DIMS, L = 16, 8, 32, 8
BN_EPS = 1e-5
NCORES = 8
BPC = B // NCORES          # batches per core
CL = RC * RF               # 208 rows in T-layout
SKR = SC * RF              # 104 skip rows
CH = ((0, 128), (128, 80))  # l-major T-layout row chunks
NV_COLS = 4 + L + L * 2 * 3 + 2

_CACHED = None


def _build_nc():
    nc = bacc.Bacc("TRN2", target_bir_lowering=False)

    d = {}
    def din(name, shape, dt=F32R):
        d[name] = nc.dram_tensor(name, list(shape), dt, kind="ExternalInput")

    din("inp", (BPC, 2, RF, N))
    din("adp", (BPC, DIMS, DIMS))
    din("p2T", (DIMS, N))
    din("p3sT", (DIMS, DIMS))
    din("wstart0", (2, RF, 128))
    din("wstart1", (2, RF, 80))
    din("wfc1_0", (128, 128)); din("wfc1_1", (80, 80))
    din("wfc2_0", (128, 128), BF16); din("wfc2_1", (80, 80), BF16)
    din("wskip0", (L, 128, 64), BF16)
    din("wskip1", (L, 80, 40), BF16)
    din("wgc0", (L, 3, 128, 128), BF16)
    din("wgc1", (L, 3, 80, 80), BF16)
    din("we1", (L, SKR, 64), BF16)
    din("we2", (64, 12))
    din("idenb", (128, 128), BF16)
    din("idenr", (128, 128))
    din("idenh", (128, 128))
    din("wav0", (L, 128, 128), BF16)
    din("wav1", (L, 80, 80), BF16)
    din("vecs", (128, NV_COLS), F32)
    outp = nc.dram_tensor("outp", [BPC, 12, N], F32, kind="ExternalOutput")

    with TileContext(nc) as tc, \
         tc.tile_pool(name="wp", bufs=1) as wp, \
         tc.tile_pool(name="ap", bufs=1) as ap, \
         tc.tile_pool(name="pp", bufs=1, space="PSUM") as pp:

        def wtile(name, src_ap, shape, dt=F32R, eng=None):
            t = wp.tile(shape, dt, tag=name, name=name)
            (eng or nc.sync).dma_start(out=t[:], in_=src_ap)
            return t

        # phase0-critical loads go first on the SP queue; bulk weights on
        # gpsimd so PE can start within ~2us.
        p2T = wtile("p2T", d["p2T"][:], (DIMS, N), eng=nc.sync)
        p3sT = wtile("p3sT", d["p3sT"][:], (DIMS, DIMS), eng=nc.sync)
        adps = [wtile(f"adp{b}", d["adp"][b], (DIMS, DIMS), eng=nc.sync)
                for b in range(BPC)]

        inps = []
        for b in range(BPC):
            t0 = ap.tile((RF, N), F32R, tag="in0", name=f"in0_{b}")[:]
            t1 = ap.tile((RF, N), F32R, tag="in1", name=f"in1_{b}")[:]
            nc.sync.dma_start(out=t0, in_=d["inp"][b, 0])
            nc.sync.dma_start(out=t1, in_=d["inp"][b, 1])
            inps.append((t0, t1))

        idenb = wtile("idenb", d["idenb"][:], (128, 128), BF16)
        idenr = wtile("idenr", d["idenr"][:], (128, 128))
        idenh = wtile("idenh", d["idenh"][:], (128, 128))
        wav = [[wtile(f"wav{i}_{c}", d[f"wav{c}"][i],
                      (CH[c][1], CH[c][1]), BF16) for c in range(2)]
               for i in range(L)]
        vecs = wtile("vecs", d["vecs"][:], (128, NV_COLS), F32)
        wstart = [[wtile(f"wst{s}_{c}", d[f"wstart{c}"][s],
                         (RF, CH[c][1])) for c in range(2)] for s in range(2)]
        wfc1 = [wtile(f"wfc1_{c}", d[f"wfc1_{c}"][:],
                      (CH[c][1], CH[c][1])) for c in range(2)]
        wfc2 = [wtile(f"wfc2_{c}", d[f"wfc2_{c}"][:],
                      (CH[c][1], CH[c][1]), BF16) for c in range(2)]
        wskip = [[wtile(f"wsk{i}_{c}", d[f"wskip{c}"][i],
                        (CH[c][1], (64, 40)[c]), BF16) for c in range(2)]
                 for i in range(L)]
        we1 = [wtile(f"we1_{i}", d["we1"][i], (SKR, 64), BF16) for i in range(L)]
        we2 = wtile("we2", d["we2"][:], (64, 12))

        vc = {}
        ci = 0
        for nm in ("sb0", "sb1", "sab0", "sab1"):
            vc[nm] = ci; ci += 1
        for i in range(L):
            vc[f"skb{i}"] = ci; ci += 1
        for i in range(L):
            for c in range(2):
                for nm in ("bns", "bnb", "av"):
                    vc[f"{nm}{i}_{c}"] = ci; ci += 1
        vc["e1b"] = ci; ci += 1
        vc["e2b"] = ci; ci += 1
        assert ci == NV_COLS

        def vcol(nm, rows=128):
            return vecs[:rows, vc[nm]:vc[nm] + 1]

        NS = (slice(0, 512), slice(512, 1024))

        st = [dict() for _ in range(BPC)]

        # ---------------- adjacency (both batches interleaved) ----------
        def phase0_pair():
            BS = range(BPC)
            # L-stack rows: [u; -srcT], R-stack rows: [srcT; u] so that
            # D = x1^T - x1 is ONE K=64 matmul per (v, ns).
            Lst = [ap.tile((64, N), F32R, tag=f"Lst{b}", name=f"Lst{b}")
                   for b in BS]
            Rst = [ap.tile((64, N), F32R, tag=f"Rst{b}", name=f"Rst{b}")
                   for b in BS]
            for nsi, ns in enumerate(NS):
                pss = []
                for b in BS:
                    ps = pp.tile((DIMS, 512), F32, tag="pwork", bufs=3,
                                 name=f"srcTps{b}_{nsi}")
                    nc.tensor.matmul(ps[:], adps[b][:], p2T[:, ns],
                                     start=True, stop=True)
                    pss.append(ps)
                for b in BS:
                    nc.scalar.activation(Rst[b][0:32, ns], pss[b][:], AF.Copy)
                    nc.vector.tensor_scalar(Lst[b][32:64, ns], pss[b][:],
                                            -1.0, None, ALU.mult)
            for nsi, ns in enumerate(NS):
                pss = []
                for b in BS:
                    ps = pp.tile((DIMS, 512), F32, tag="pwork", bufs=3,
                                 name=f"ups{b}_{nsi}")
                    nc.tensor.matmul(ps[:], p3sT[:], Rst[b][0:32, ns],
                                     start=True, stop=True)
                    pss.append(ps)
                for b in BS:
                    nc.scalar.activation(Lst[b][0:32, ns], pss[b][:], AF.Copy)
                    nc.vector.tensor_copy(Rst[b][32:64, ns], pss[b][:])
            st[0]["LR"] = (Lst, Rst)

        def phase0_D(fillers=()):
            BS = range(BPC)
            fillers = list(fillers)
            Lst, Rst = st[0]["LR"]
            ATs = [[ap.tile((128, N), BF16, tag=f"AT{b}_{v}", name=f"AT{b}_{v}")
                    for v in range(8)] for b in BS]
            Dts = [[ap.tile((128, N), BF16, tag=f"Dt{b}",
                            name=f"Dt{b}_{v}") for v in range(8)] for b in BS]
            for v in range(8):
                cs = slice(v * 128, (v + 1) * 128)
                for nsi, ns in enumerate(NS):
                    dpss = []
                    for b in BS:
                        dps = pp.tile((128, 512), F32, tag="pwork", bufs=3,
                                      name=f"dps{b}_{v}_{nsi}")
                        nc.tensor.matmul(dps[:], Lst[b][:, cs], Rst[b][:, ns],
                                         start=True, stop=True)
                        dpss.append(dps)
                    # relu(tanh(D)) == tanh(max(D, 0)); relu on DVE so the
                    # tanh-bound ACT queue stays short
                    for b in BS:
                        nc.vector.tensor_scalar(Dts[b][v][:, ns], dpss[b][:],
                                                0.0, None, ALU.max)
                    for b in BS:
                        nc.scalar.activation(ATs[b][v][:, ns], Dts[b][v][:, ns],
                                             AF.Tanh)
                if v % 2 == 1 and fillers:
                    fillers.pop(0)()
            while fillers:
                fillers.pop(0)()
            for b in BS:
                st[b]["AT"] = ATs[b]

        # ---------------- start convs (emitted as fillers in phase0_D) ----
        def start(b):
            in0, in1 = inps[b]
            xt, xa = [None, None], [None, None]
            fillers = []
            for c in range(2):
                rows = CH[c][1]
                xt[c] = ap.tile((rows, N), F32R, tag=f"XT{b}_{c}", bufs=2,
                                name=f"XT{b}_{c}_init")
                xa[c] = ap.tile((rows, N), BF16, tag=f"XA{b}_{c}",
                                name=f"XA{b}_{c}")
                def mk(c, xtt, xat):
                    rows = CH[c][1]
                    def emit():
                        for nsi, ns in enumerate(NS):
                            ps = pp.tile((rows, 512), F32, tag="pwork", bufs=3,
                                         name=f"stp{b}_{c}_{nsi}")
                            nc.tensor.matmul(ps[:], wstart[0][c][:],
                                             in0[:, ns], start=True, stop=True)
                            nc.scalar.activation(xtt[:, ns], ps[:], AF.Identity,
                                                 bias=vcol(f"sb{c}", rows))
                            psa = pp.tile((rows, 512), F32, tag="pwork",
                                          bufs=3, name=f"stpa{b}_{c}_{nsi}")
                            nc.tensor.matmul(psa[:], wstart[1][c][:],
                                             in1[:, ns], start=True, stop=True)
                            nc.scalar.activation(xat[:, ns], psa[:],
                                                 AF.Identity,
                                                 bias=vcol(f"sab{c}", rows))
                    return emit
                fillers.append(mk(c, xt[c], xa[c]))
            st[b]["xt"], st[b]["xa"] = xt, xa
            st[b]["end"] = ap.tile((64, N), F32, tag=f"END{b}", name=f"END{b}")
            return fillers

        # ---------------- one layer, both batches stage-interleaved ----------
        def layer_pair(i):
            BS = range(BPC)
            xt = [st[b]["xt"] for b in BS]
            xa = [st[b]["xa"] for b in BS]
            AT = [st[b]["AT"] for b in BS]

            gcw = [[[ap.tile((CH[c][1], CH[c][1]), BF16, tag=f"gcw{b}_{c}_{s}",
                             bufs=2, name=f"gcw{b}_{i}_{c}_{s}")
                     for c in range(2)] for s in range(3)] for b in BS]
            for b in BS:
                for s in range(3):
                    for c in range(2):
                        nc.sync.dma_start(out=gcw[b][s][c][:],
                                          in_=d[f"wgc{c}"][i, s])

            # -- attention + sigmoid
            xn = [[None, None] for b in BS]
            r1 = [[None, None] for b in BS]
            for b in BS:
                for c in range(2):
                    rows = CH[c][1]
                    r1[b][c] = ap.tile((rows, N), BF16, tag=f"R1{b}_{c}",
                                       name=f"R1{b}_{i}_{c}")
                    xn[b][c] = ap.tile((rows, N), BF16, tag=f"XN{b}_{c}",
                                       name=f"XN{b}_{i}_{c}")
            groups = [(c, nsi) for c in range(2) for nsi in range(2)]
            m1s, apss = {}, {}
            for c, nsi in groups:
                rows, ns = CH[c][1], NS[nsi]
                for b in BS:
                    m1 = pp.tile((rows, 512), F32, tag="pwork", bufs=3,
                                 name=f"m1_{b}_{i}_{c}_{nsi}")
                    nc.tensor.matmul(m1[:], wfc1[c][:], xt[b][c][:, ns],
                                     start=True, stop=True)
                    m1s[b, c, nsi] = m1
                for b in BS:
                    if b % 2 == 0:
                        nc.scalar.activation(r1[b][c][:, ns], m1s[b, c, nsi][:],
                                             AF.Relu)
                    else:
                        nc.vector.tensor_scalar(r1[b][c][:, ns],
                                                m1s[b, c, nsi][:],
                                                0.0, None, ALU.max)
            for c, nsi in groups:
                rows, ns = CH[c][1], NS[nsi]
                for b in BS:
                    a_ps = pp.tile((rows, 512), F32, tag="pwork", bufs=3,
                                   name=f"aps{b}_{i}_{c}_{nsi}")
                    nc.tensor.matmul(a_ps[:], wfc2[c][:], r1[b][c][:, ns],
                                     start=True, stop=False)
                    nc.tensor.matmul(a_ps[:], idenh[:rows, :rows],
                                     xt[b][c][:, ns], start=False, stop=True)
                    apss[b, c, nsi] = a_ps
                for b in BS:
                    # xn = sigmoid(2*(a + x/2)) straight from PSUM
                    nc.scalar.activation(xn[b][c][:, ns], apss[b, c, nsi][:],
                                         AF.Sigmoid, scale=2.0)

            # -- skip conv -> relu -> end1 matmul -> SBUF accumulator
            rsk = [ap.tile((SKR, N), BF16, tag=f"rsk{b}", name=f"rsk{b}_{i}")
                   for b in BS]
            sks = {}
            for nsi, ns in enumerate(NS):
                for b in BS:
                    sk_ps = pp.tile((SKR, 512), F32, tag="pwork", bufs=3,
                                    name=f"skp{b}_{i}_{nsi}")
                    nc.tensor.matmul(sk_ps[:64], wskip[i][0][:],
                                     xn[b][0][:, ns], start=True, stop=True)
                    nc.tensor.matmul(sk_ps[64:], wskip[i][1][:],
                                     xn[b][1][:, ns], start=True, stop=True)
                    sks[b, nsi] = sk_ps
                for b in BS:
                    if b % 2 == 0:
                        nc.vector.tensor_scalar(rsk[b][:, ns], sks[b, nsi][:],
                                                vcol(f"skb{i}", SKR), 0.0,
                                                ALU.add, ALU.max)
                    else:
                        nc.scalar.activation(rsk[b][:, ns], sks[b, nsi][:],
                                             AF.Relu, bias=vcol(f"skb{i}", SKR))
            for nsi, ns in enumerate(NS):
                for b in BS:
                    e_ps = pp.tile((64, 512), F32, tag="pwork", bufs=3,
                                   name=f"eps{b}_{i}_{nsi}")
                    nc.tensor.matmul(e_ps[:], we1[i][:], rsk[b][:, ns],
                                     start=True, stop=True)
                    if i == 0:
                        nc.vector.tensor_copy(st[b]["end"][:, ns], e_ps[:])
                    else:
                        nc.vector.scalar_tensor_tensor(
                            st[b]["end"][:, ns], e_ps[:], 0.0,
                            st[b]["end"][:, ns], ALU.bypass, ALU.add)

            # -- V-layout of xn via PE transposes
            xv = [[None] * 8 for b in BS]
            for v in range(8):
                cs = slice(v * 128, (v + 1) * 128)
                for b in BS:
                    tp = pp.tile((128, CL), BF16, tag="ptr", bufs=3,
                                 name=f"tpx{b}_{i}_{v}")
                    for c in range(2):
                        o, rows = CH[c]
                        nc.tensor.transpose(tp[:, o:o + rows],
                                            xn[b][c][:, cs],
                                            idenb[:rows, :rows])
                    xv[b][v] = ap.tile((128, CL), BF16, tag=f"XV{b}_{v}",
                                       name=f"XV{b}_{i}_{v}")
                    nc.vector.tensor_copy(xv[b][v][:], tp[:])

            def hop(rv, nm):
                """A-hop (V-orientation, w-pairs) + transpose back, both b."""
                hvp = [[None] * 4 for b in BS]
                for p in range(4):
                    for b in BS:
                        h_ps = pp.tile((128, 2 * CL), F32, tag="ptr", bufs=3,
                                       name=f"hp{nm}{b}_{i}_{p}")
                        for half in range(2):
                            w = 2 * p + half
                            ws = slice(w * 128, (w + 1) * 128)
                            dst = h_ps[:, half * CL:(half + 1) * CL]
                            for k in range(8):
                                nc.tensor.matmul(dst, AT[b][k][:, ws], rv(b, k),
                                                 start=(k == 0), stop=(k == 7))
                        hvp[b][p] = ap.tile((128, 2 * CL), BF16,
                                            tag=f"{nm}V{b}_{p}",
                                            name=f"{nm}V{b}_{i}_{p}")
                        if (b + p) % 2 == 0:
                            nc.vector.tensor_copy(hvp[b][p][:], h_ps[:])
                        else:
                            nc.scalar.activation(hvp[b][p][:], h_ps[:], AF.Copy)

                ht = [[ap.tile((CH[c][1], N), BF16, tag=f"{nm}T{b}_{c}",
                               name=f"{nm}T{b}_{i}_{c}") for c in range(2)]
                      for b in BS]
                for b in BS:
                    tpb = [pp.tile((CH[c][1], N), BF16, tag=f"ptb{c}",
                                   bufs=1, name=f"tpb{nm}{b}_{i}_{c}")
                           for c in range(2)]
                    for w in range(8):
                        src = hvp[b][w // 2][:, (w % 2) * CL:(w % 2) * CL + CL]
                        for c in range(2):
                            o, rows = CH[c]
                            nc.tensor.transpose(
                                tpb[c][:, w * 128:(w + 1) * 128],
                                src[:, o:o + rows], idenb[:, :])
                        if w % 4 == 3:
                            half = slice((w - 3) * 128, (w + 1) * 128)
                            for c in range(2):
                                if (b + c) % 2 == 0:
                                    nc.scalar.activation(ht[b][c][:, half],
                                                         tpb[c][:, half],
                                                         AF.Copy)
                                else:
                                    nc.vector.tensor_copy(ht[b][c][:, half],
                                                          tpb[c][:, half])
                return hvp, ht

            h1vp, h1t = hop(lambda b, k: xv[b][k][:], "H1")
            _, h2t = hop(
                lambda b, k: h1vp[b][k // 2][:, (k % 2) * CL:(k % 2) * CL + CL],
                "H2")

            # -- gconv (block-diag over l); residual x and av*xa are
            # accumulated in PSUM via identity/diagonal matmuls, then a
            # single affine eviction applies the batchnorm.
            for c in range(2):
                rows = CH[c][1]
                nxt = [ap.tile((rows, N), F32R, tag=f"XT{b}_{c}", bufs=2,
                               name=f"XT{b}_{i}_{c}") for b in BS]
                for nsi, ns in enumerate(NS):
                    gps = []
                    for b in BS:
                        g_ps = pp.tile((rows, 512), F32, tag="pwork", bufs=3,
                                       name=f"gp{b}_{i}_{c}_{nsi}")
                        srcs = (xn[b], h1t[b], h2t[b])
                        for s in range(3):
                            nc.tensor.matmul(g_ps[:], gcw[b][s][c][:],
                                             srcs[s][c][:, ns],
                                             start=(s == 0), stop=False)
                        nc.tensor.matmul(g_ps[:], idenr[:rows, :rows],
                                         xt[b][c][:, ns],
                                         start=False, stop=False)
                        nc.tensor.matmul(g_ps[:], wav[i][c][:],
                                         xa[b][c][:, ns],
                                         start=False, stop=True)
                        gps.append(g_ps)
                    for b in BS:
                        if b % 2 == 0:
                            nc.scalar.activation(nxt[b][:, ns], gps[b][:],
                                                 AF.Identity,
                                                 bias=vcol(f"bnb{i}_{c}", rows),
                                                 scale=vcol(f"bns{i}_{c}", rows))
                        else:
                            nc.vector.tensor_scalar(
                                nxt[b][:, ns], gps[b][:],
                                vcol(f"bns{i}_{c}", rows),
                                vcol(f"bnb{i}_{c}", rows), ALU.mult, ALU.add)
                for b in BS:
                    xt[b][c] = nxt[b]

        # ---------------- end convs ----------------
        def tail(b):
            o1 = ap.tile((64, N), F32R, tag="o1", name=f"o1_{b}")
            ob = ap.tile((12, N), F32, tag="ob", name=f"ob{b}")
            for nsi, ns in enumerate(NS):
                nc.scalar.activation(o1[:, ns], st[b]["end"][:, ns], AF.Relu,
                                     bias=vcol("e1b", 64))
                o2_ps = pp.tile((12, 512), F32, tag="pwork", bufs=3,
                                name=f"o2p{b}_{nsi}")
                nc.tensor.matmul(o2_ps[:], we2[:], o1[:, ns],
                                 start=True, stop=True)
                nc.scalar.activation(ob[:, ns], o2_ps[:], AF.Identity,
                                     bias=vcol("e2b", 12))
            nc.sync.dma_start(out=outp[b], in_=ob[:])

        phase0_pair()
        fillers = []
        for b in range(BPC):
            fillers.extend(start(b))
        phase0_D(fillers)
        for i in range(L):
            layer_pair(i)
        for b in range(BPC):
            tail(b)

    nc.finalize()
    return nc


# ----------------------------------------------------------------------------
# host-side preprocessing
# ----------------------------------------------------------------------------

def _prep_host(inputs):
    f = lambda x: np.asarray(x, dtype=np.float32)
    bf = lambda x: np.ascontiguousarray(x).astype(ml_dtypes.bfloat16)
    x_in = f(inputs["inputs"])
    ind = np.asarray(inputs["ind"]).astype(np.int64)
    p1, p2, p3, pk = f(inputs["p1"]), f(inputs["p2"]), f(inputs["p3"]), f(inputs["pk"])

    xo = np.pad(x_in, ((0, 0), (0, 0), (0, 0), (RF - T, 0)))
    inp_t = np.ascontiguousarray(xo.transpose(0, 1, 3, 2))
    te = p1[ind]
    adp = np.einsum("bi,ijk->bjk", te, pk).astype(np.float32)

    start_w, start_b = f(inputs["start_w"]), f(inputs["start_b"])
    starta_w, starta_b = f(inputs["starta_w"]), f(inputs["starta_b"])
    fc1_w, fc2_w = f(inputs["fc1_w"]), f(inputs["fc2_w"])
    skip_w, skip_b = f(inputs["skip_w"]), f(inputs["skip_b"])
    gconv_w, gconv_b = f(inputs["gconv_w"]), f(inputs["gconv_b"])
    bn_g, bn_b = f(inputs["bn_g"]), f(inputs["bn_b"])
    bna_g, bna_b = f(inputs["bna_g"]), f(inputs["bna_b"])
    end1_w, end1_b = f(inputs["end1_w"]), f(inputs["end1_b"])
    end2_w, end2_b = f(inputs["end2_w"]), f(inputs["end2_b"])

    e8, e5 = np.eye(8, dtype=np.float32), np.eye(5, dtype=np.float32)
    e13 = np.eye(RF, dtype=np.float32)
    kr = lambda e, w: np.kron(e, np.ascontiguousarray(w.T)).astype(np.float32)

    wstart0 = np.stack([np.kron(e13[:, :8], w[:, 0][None, :])
                        for w in (start_w, starta_w)]).astype(np.float32)
    wstart1 = np.stack([np.kron(e13[:, 8:], w[:, 0][None, :])
                        for w in (start_w, starta_w)]).astype(np.float32)
    wgc0 = np.stack([np.stack([kr(e8, gconv_w[i][:, s * 16:(s + 1) * 16])
                               for s in range(3)]) for i in range(L)])
    wgc1 = np.stack([np.stack([kr(e5, gconv_w[i][:, s * 16:(s + 1) * 16])
                               for s in range(3)]) for i in range(L)])
    wskip0 = np.stack([kr(e8, skip_w[i]) for i in range(L)])
    wskip1 = np.stack([kr(e5, skip_w[i]) for i in range(L)])

    # end1 columns: ref skip rows are o*13+l within the (L-1-i)-th block;
    # ours are l*8+o
    we1 = np.zeros((L, SKR, 64), dtype=np.float32)
    ll, oo = np.meshgrid(np.arange(RF), np.arange(SC), indexing="ij")
    src_col = oo.ravel() * RF + ll.ravel()
    for i in range(L):
        we1[i] = end1_w[:, (L - 1 - i) * SKR + src_col].T

    t8 = lambda v: np.tile(v, 8)
    vecs = np.zeros((128, NV_COLS), dtype=np.float32)
    ci = 0
    vecs[:, ci] = t8(start_b); ci += 1
    vecs[:80, ci] = np.tile(start_b, 5); ci += 1
    vecs[:, ci] = t8(starta_b); ci += 1
    vecs[:80, ci] = np.tile(starta_b, 5); ci += 1
    for i in range(L):
        vecs[:SKR, ci] = np.tile(skip_b[i], RF); ci += 1
    bns = (bn_g / np.sqrt(1.0 + BN_EPS)).astype(np.float32)
    bnas = (bna_g / np.sqrt(1.0 + BN_EPS)).astype(np.float32)
    av = np.ones(16, dtype=np.float32)
    bv = np.zeros(16, dtype=np.float32)
    for i in range(L):
        bnb_adj = bn_b[i] + bns[i] * (gconv_b[i] + bv)
        vecs[:, ci] = t8(bns[i]); ci += 1
        vecs[:, ci] = t8(bnb_adj); ci += 1
        vecs[:, ci] = t8(av); ci += 1
        vecs[:80, ci] = np.tile(bns[i], 5); ci += 1
        vecs[:80, ci] = np.tile(bnb_adj, 5); ci += 1
        vecs[:80, ci] = np.tile(av, 5); ci += 1
        av = 2.0 * bnas[i] * av
        bv = 2.0 * bnas[i] * bv + bna_b[i]
    # rebuild per-layer diag(av) for the PE-side xa accumulation
    avs = [np.ones(16, dtype=np.float32)]
    for i in range(L):
        avs.append(2.0 * bnas[i] * avs[-1])
    wav0 = np.stack([np.diag(np.tile(avs[i], 8)) for i in range(L)])
    wav1 = np.stack([np.diag(np.tile(avs[i], 5)) for i in range(L)])
    wav0 = wav0.astype(ml_dtypes.bfloat16)
    wav1 = wav1.astype(ml_dtypes.bfloat16)
    vecs[:64, ci] = end1_b; ci += 1
    vecs[:12, ci] = end2_b; ci += 1
    assert ci == NV_COLS

    shared = {
        "p2T": np.ascontiguousarray(p2.T),
        "p3sT": np.ascontiguousarray(p3[:DIMS, :DIMS].T),
        "wstart0": wstart0, "wstart1": wstart1,
        "wfc1_0": kr(e8, fc1_w), "wfc1_1": kr(e5, fc1_w),
        "wfc2_0": bf(kr(e8, fc2_w)), "wfc2_1": bf(kr(e5, fc2_w)),
        "wskip0": bf(wskip0), "wskip1": bf(wskip1),
        "wgc0": bf(wgc0), "wgc1": bf(wgc1),
        "we1": bf(we1), "we2": np.ascontiguousarray(end2_w.T),
        "idenb": np.eye(128, dtype=ml_dtypes.bfloat16),
        "idenr": np.eye(128, dtype=np.float32),
        "idenh": 0.5 * np.eye(128, dtype=np.float32),
        "wav0": wav0, "wav1": wav1,
        "vecs": vecs,
    }
    in_maps = []
    for c in range(NCORES):
        bs = slice(c * BPC, (c + 1) * BPC)
        m = dict(shared)
        m["inp"] = np.ascontiguousarray(inp_t[bs])
        m["adp"] = np.ascontiguousarray(adp[bs])
        in_maps.append(m)
    return in_maps


def _get_nc():
    global _CACHED
    if _CACHED is None:
        _CACHED = _build_nc()
    return _CACHED


def run(inputs, trace=False):
    nc = _get_nc()
    in_maps = _prep_host(inputs)
    res = run_bass_kernel_spmd(nc, in_maps, core_ids=list(range(NCORES)),
                               trace=trace)
    out = np.stack([res.results[c]["outp"] for c in range(NCORES)])
    out = out.reshape(B, 12, N, 1).astype(np.float32)
    return out, res


def kernel(**inputs):
    out, _ = run(inputs)
    return out


# revision 32
# speedup vs baseline: 3474.8241x; 1.0474x over previous
"""DMSTGCN forward on 8 Trainium2 NeuronCores (Bass/Tile).

Sharding: data-parallel over batch B=16 -> 2 batches per core; parameters
replicated. The dynamic adjacency (1024x1024 per batch) is built and kept in
SBUF (bf16); 1x1 convs run as block-diagonal (W (x) I) matmuls in an l-major
"[(time,chan), node]" layout, graph hops in "[node, (time,chan)]" layout with
PE transposes between the two. Trunk math is float32r (TF32-like), graph-hop
operands bf16. The two batches are emitted layer-interleaved, all heavy ops
are sliced per 512 nodes, and PSUM tiles are single-bank so the scheduler can
overlap the two batch streams.
"""
import numpy as np
import ml_dtypes

import concourse.bacc as bacc
import concourse.mybir as mybir
from concourse.tile import TileContext
from concourse.bass_utils import run_bass_kernel_spmd

F32 = mybir.dt.float32
F32R = mybir.dt.float32r
BF16 = mybir.dt.bfloat16
AF = mybir.ActivationFunctionType
ALU = mybir.AluOpType

B, N, T, RF = 16, 2, 1024, 12  # placeholder, fixed below
B, N, T, RF = 16, 1024, 12, 13
RC, SC, DIMS, L = 16, 8, 32, 8
BN_EPS = 1e-5
NCORES = 8
BPC = B // NCORES          # batches per core
CL = RC * RF               # 208 rows in T-layout
SKR = SC * RF              # 104 skip rows
CH = ((0, 128), (128, 80))  # l-major T-layout row chunks
NV_COLS = 4 + L + L * 2 * 3 + 2

_CACHED = None


def _build_nc():
    nc = bacc.Bacc("TRN2", target_bir_lowering=False)

    d = {}
    def din(name, shape, dt=F32R):
        d[name] = nc.dram_tensor(name, list(shape), dt, kind="ExternalInput")

    din("inp", (BPC, 2, RF, N))
    din("adp", (BPC, DIMS, DIMS))
    din("p2T", (DIMS, N))
    din("p3sT", (DIMS, DIMS))
    din("wstart0", (2, RF, 128))
    din("wstart1", (2, RF, 80))
    din("wfc1_0", (128, 128)); din("wfc1_1", (80, 80))
    din("wfc2_0", (128, 128), BF16); din("wfc2_1", (80, 80), BF16)
    din("wskip0", (L, 128, 64), BF16)
    din("wskip1", (L, 80, 40), BF16)
    din("wgc0", (L, 3, 128, 128), BF16)
    din("wgc1", (L, 3, 80, 80), BF16)
    din("we1", (L, SKR, 64), BF16)
    din("we2", (64, 12))
    din("idenb", (128, 128), BF16)
    din("idenr", (128, 128))
    din("idenh", (128, 128))
    din("wav0", (L, 128, 128), BF16)
    din("wav1", (L, 80, 80), BF16)
    din("vecs", (128, NV_COLS), F32)
    outp = nc.dram_tensor("outp", [BPC, 12, N], F32, kind="ExternalOutput")

    with TileContext(nc) as tc, \
         tc.tile_pool(name="wp", bufs=1) as wp, \
         tc.tile_pool(name="ap", bufs=1) as ap, \
         tc.tile_pool(name="pp", bufs=1, space="PSUM") as pp:

        def wtile(name, src_ap, shape, dt=F32R, eng=None):
            t = wp.tile(shape, dt, tag=name, name=name)
            (eng or nc.sync).dma_start(out=t[:], in_=src_ap)
            return t

        # phase0-critical loads go first on the SP queue; bulk weights on
        # gpsimd so PE can start within ~2us.
        p2T = wtile("p2T", d["p2T"][:], (DIMS, N), eng=nc.sync)
        p3sT = wtile("p3sT", d["p3sT"][:], (DIMS, DIMS), eng=nc.sync)
        adps = [wtile(f"adp{b}", d["adp"][b], (DIMS, DIMS), eng=nc.sync)
                for b in range(BPC)]

        inps = []
        for b in range(BPC):
            t0 = ap.tile((RF, N), F32R, tag="in0", name=f"in0_{b}")[:]
            t1 = ap.tile((RF, N), F32R, tag="in1", name=f"in1_{b}")[:]
            nc.sync.dma_start(out=t0, in_=d["inp"][b, 0])
            nc.sync.dma_start(out=t1, in_=d["inp"][b, 1])
            inps.append((t0, t1))

        idenb = wtile("idenb", d["idenb"][:], (128, 128), BF16)
        idenr = wtile("idenr", d["idenr"][:], (128, 128))
        idenh = wtile("idenh", d["idenh"][:], (128, 128))
        wav = [[wtile(f"wav{i}_{c}", d[f"wav{c}"][i],
                      (CH[c][1], CH[c][1]), BF16) for c in range(2)]
               for i in range(L)]
        vecs = wtile("vecs", d["vecs"][:], (128, NV_COLS), F32)
        wstart = [[wtile(f"wst{s}_{c}", d[f"wstart{c}"][s],
                         (RF, CH[c][1])) for c in range(2)] for s in range(2)]
        wfc1 = [wtile(f"wfc1_{c}", d[f"wfc1_{c}"][:],
                      (CH[c][1], CH[c][1])) for c in range(2)]
        wfc2 = [wtile(f"wfc2_{c}", d[f"wfc2_{c}"][:],
                      (CH[c][1], CH[c][1]), BF16) for c in range(2)]
        wskip = [[wtile(f"wsk{i}_{c}", d[f"wskip{c}"][i],
                        (CH[c][1], (64, 40)[c]), BF16) for c in range(2)]
                 for i in range(L)]
        we1 = [wtile(f"we1_{i}", d["we1"][i], (SKR, 64), BF16) for i in range(L)]
        we2 = wtile("we2", d["we2"][:], (64, 12))

        vc = {}
        ci = 0
        for nm in ("sb0", "sb1", "sab0", "sab1"):
            vc[nm] = ci; ci += 1
        for i in range(L):
            vc[f"skb{i}"] = ci; ci += 1
        for i in range(L):
            for c in range(2):
                for nm in ("bns", "bnb", "av"):
                    vc[f"{nm}{i}_{c}"] = ci; ci += 1
        vc["e1b"] = ci; ci += 1
        vc["e2b"] = ci; ci += 1
        assert ci == NV_COLS

        def vcol(nm, rows=128):
            return vecs[:rows, vc[nm]:vc[nm] + 1]

        NS = (slice(0, 512), slice(512, 1024))

        st = [dict() for _ in range(BPC)]

        # ---------------- adjacency (both batches interleaved) ----------
        def phase0_pair():
            BS = range(BPC)
            # L-stack rows: [u; -srcT], R-stack rows: [srcT; u] so that
            # D = x1^T - x1 is ONE K=64 matmul per (v, ns).
            Lst = [ap.tile((64, N), F32R, tag=f"Lst{b}", name=f"Lst{b}")
                   for b in BS]
            Rst = [ap.tile((64, N), F32R, tag=f"Rst{b}", name=f"Rst{b}")
                   for b in BS]
            for nsi, ns in enumerate(NS):
                pss = []
                for b in BS:
                    ps = pp.tile((DIMS, 512), F32, tag="pwork", bufs=3,
                                 name=f"srcTps{b}_{nsi}")
                    nc.tensor.matmul(ps[:], adps[b][:], p2T[:, ns],
                                     start=True, stop=True)
                    pss.append(ps)
                for b in BS:
                    nc.scalar.activation(Rst[b][0:32, ns], pss[b][:], AF.Copy)
                    nc.vector.tensor_scalar(Lst[b][32:64, ns], pss[b][:],
                                            -1.0, None, ALU.mult)
            for nsi, ns in enumerate(NS):
                pss = []
                for b in BS:
                    ps = pp.tile((DIMS, 512), F32, tag="pwork", bufs=3,
                                 name=f"ups{b}_{nsi}")
                    nc.tensor.matmul(ps[:], p3sT[:], Rst[b][0:32, ns],
                                     start=True, stop=True)
                    pss.append(ps)
                for b in BS:
                    nc.scalar.activation(Lst[b][0:32, ns], pss[b][:], AF.Copy)
                    nc.vector.tensor_copy(Rst[b][32:64, ns], pss[b][:])
            st[0]["LR"] = (Lst, Rst)

        def phase0_D(fillers=()):
            BS = range(BPC)
            fillers = list(fillers)
            Lst, Rst = st[0]["LR"]
            ATs = [[ap.tile((128, N), BF16, tag=f"AT{b}_{v}", name=f"AT{b}_{v}")
                    for v in range(8)] for b in BS]
            Dts = [[ap.tile((128, N), BF16, tag=f"Dt{b}",
                            name=f"Dt{b}_{v}") for v in range(8)] for b in BS]
            for v in range(8):
                cs = slice(v * 128, (v + 1) * 128)
                for nsi, ns in enumerate(NS):
                    dpss = []
                    for b in BS:
                        dps = pp.tile((128, 512), F32, tag="pwork", bufs=3,
                                      name=f"dps{b}_{v}_{nsi}")
                        nc.tensor.matmul(dps[:], Lst[b][:, cs], Rst[b][:, ns],
                                         start=True, stop=True)
                        dpss.append(dps)
                    # relu(tanh(D)) == tanh(max(D, 0)); relu on DVE so the
                    # tanh-bound ACT queue stays short
                    for b in BS:
                        nc.vector.tensor_scalar(Dts[b][v][:, ns], dpss[b][:],
                                                0.0, None, ALU.max)
                    for b in BS:
                        nc.scalar.activation(ATs[b][v][:, ns], Dts[b][v][:, ns],
                                             AF.Tanh)
                if v % 2 == 1 and fillers:
                    fillers.pop(0)()
            while fillers:
                fillers.pop(0)()
            for b in BS:
                st[b]["AT"] = ATs[b]

        # ---------------- start convs (emitted as fillers in phase0_D) ----
        def start(b):
            in0, in1 = inps[b]
            xt, xa = [None, None], [None, None]
            fillers = []
            for c in range(2):
                rows = CH[c][1]
                xt[c] = ap.tile((rows, N), F32R, tag=f"XT{b}_{c}", bufs=2,
                                name=f"XT{b}_{c}_init")
                xa[c] = ap.tile((rows, N), BF16, tag=f"XA{b}_{c}",
                                name=f"XA{b}_{c}")
                def mk(c, xtt, xat):
                    rows = CH[c][1]
                    def emit():
                        for nsi, ns in enumerate(NS):
                            ps = pp.tile((rows, 512), F32, tag="pwork", bufs=3,
                                         name=f"stp{b}_{c}_{nsi}")
                            nc.tensor.matmul(ps[:], wstart[0][c][:],
                                             in0[:, ns], start=True, stop=True)
                            nc.scalar.activation(xtt[:, ns], ps[:], AF.Identity,
                                                 bias=vcol(f"sb{c}", rows))
                            psa = pp.tile((rows, 512), F32, tag="pwork",
                                          bufs=3, name=f"stpa{b}_{c}_{nsi}")
                            nc.tensor.matmul(psa[:], wstart[1][c][:],
                                             in1[:, ns], start=True, stop=True)
                            nc.scalar.activation(xat[:, ns], psa[:],
                                                 AF.Identity,
                                                 bias=vcol(f"sab{c}", rows))
                    return emit
                fillers.append(mk(c, xt[c], xa[c]))
            st[b]["xt"], st[b]["xa"] = xt, xa
            st[b]["end"] = ap.tile((64, N), F32, tag=f"END{b}", name=f"END{b}")
            return fillers

        # ---------------- one layer, both batches stage-interleaved ----------
        def layer_pair(i):
            BS = range(BPC)
            xt = [st[b]["xt"] for b in BS]
            xa = [st[b]["xa"] for b in BS]
            AT = [st[b]["AT"] for b in BS]

            gcw = [[[ap.tile((CH[c][1], CH[c][1]), BF16, tag=f"gcw{b}_{c}_{s}",
                             bufs=2, name=f"gcw{b}_{i}_{c}_{s}")
                     for c in range(2)] for s in range(3)] for b in BS]
            for b in BS:
                for s in range(3):
                    for c in range(2):
                        nc.sync.dma_start(out=gcw[b][s][c][:],
                                          in_=d[f"wgc{c}"][i, s])

            # -- attention + sigmoid
            xn = [[None, None] for b in BS]
            r1 = [[None, None] for b in BS]
            for b in BS:
                for c in range(2):
                    rows = CH[c][1]
                    r1[b][c] = ap.tile((rows, N), BF16, tag=f"R1{b}_{c}",
                                       name=f"R1{b}_{i}_{c}")
                    xn[b][c] = ap.tile((rows, N), BF16, tag=f"XN{b}_{c}",
                                       name=f"XN{b}_{i}_{c}")
            groups = [(c, nsi) for c in range(2) for nsi in range(2)]
            m1s, apss = {}, {}
            for c, nsi in groups:
                rows, ns = CH[c][1], NS[nsi]
                for b in BS:
                    m1 = pp.tile((rows, 512), F32, tag="pwork", bufs=3,
                                 name=f"m1_{b}_{i}_{c}_{nsi}")
                    nc.tensor.matmul(m1[:], wfc1[c][:], xt[b][c][:, ns],
                                     start=True, stop=True)
                    m1s[b, c, nsi] = m1
                for b in BS:
                    if b % 2 == 0:
                        nc.scalar.activation(r1[b][c][:, ns], m1s[b, c, nsi][:],
                                             AF.Relu)
                    else:
                        nc.vector.tensor_scalar(r1[b][c][:, ns],
                                                m1s[b, c, nsi][:],
                                                0.0, None, ALU.max)
            for c, nsi in groups:
                rows, ns = CH[c][1], NS[nsi]
                for b in BS:
                    a_ps = pp.tile((rows, 512), F32, tag="pwork", bufs=3,
                                   name=f"aps{b}_{i}_{c}_{nsi}")
                    nc.tensor.matmul(a_ps[:], wfc2[c][:], r1[b][c][:, ns],
                                     start=True, stop=False)
                    nc.tensor.matmul(a_ps[:], idenh[:rows, :rows],
                                     xt[b][c][:, ns], start=False, stop=True)
                    apss[b, c, nsi] = a_ps
                for b in BS:
                    # xn = sigmoid(2*(a + x/2)) straight from PSUM
                    nc.scalar.activation(xn[b][c][:, ns], apss[b, c, nsi][:],
                                         AF.Sigmoid, scale=2.0)

            # -- V-layout of xn via PE transposes
            xv = [[None] * 8 for b in BS]
            for v in range(8):
                cs = slice(v * 128, (v + 1) * 128)
                for b in BS:
                    tp = pp.tile((128, CL), BF16, tag="ptr", bufs=3,
                                 name=f"tpx{b}_{i}_{v}")
                    for c in range(2):
                        o, rows = CH[c]
                        nc.tensor.transpose(tp[:, o:o + rows],
                                            xn[b][c][:, cs],
                                            idenb[:rows, :rows])
                    xv[b][v] = ap.tile((128, CL), BF16, tag=f"XV{b}_{v}",
                                       name=f"XV{b}_{i}_{v}")
                    nc.vector.tensor_copy(xv[b][v][:], tp[:])

            def hop(rv, nm):
                """A-hop (V-orientation, w-pairs) + transpose back, both b."""
                hvp = [[None] * 4 for b in BS]
                for p in range(4):
                    for b in BS:
                        h_ps = pp.tile((128, 2 * CL), F32, tag="ptr", bufs=3,
                                       name=f"hp{nm}{b}_{i}_{p}")
                        for half in range(2):
                            w = 2 * p + half
                            ws = slice(w * 128, (w + 1) * 128)
                            dst = h_ps[:, half * CL:(half + 1) * CL]
                            for k in range(8):
                                nc.tensor.matmul(dst, AT[b][k][:, ws], rv(b, k),
                                                 start=(k == 0), stop=(k == 7))
                        hvp[b][p] = ap.tile((128, 2 * CL), BF16,
                                            tag=f"{nm}V{b}_{p}",
                                            name=f"{nm}V{b}_{i}_{p}")
                        if (b + p) % 2 == 0:
                            nc.vector.tensor_copy(hvp[b][p][:], h_ps[:])
                        else:
                            nc.scalar.activation(hvp[b][p][:], h_ps[:], AF.Copy)

                ht = [[ap.tile((CH[c][1], N), BF16, tag=f"{nm}T{b}_{c}",
                               name=f"{nm}T{b}_{i}_{c}") for c in range(2)]
                      for b in BS]
                for b in BS:
                    tpb = [pp.tile((CH[c][1], N), BF16, tag=f"ptb{c}",
                                   bufs=1, name=f"tpb{nm}{b}_{i}_{c}")
                           for c in range(2)]
                    for w in range(8):
                        src = hvp[b][w // 2][:, (w % 2) * CL:(w % 2) * CL + CL]
                        for c in range(2):
                            o, rows = CH[c]
                            nc.tensor.transpose(
                                tpb[c][:, w * 128:(w + 1) * 128],
                                src[:, o:o + rows], idenb[:, :])
                        if w % 4 == 3:
                            half = slice((w - 3) * 128, (w + 1) * 128)
                            for c in range(2):
                                if (b + c) % 2 == 0:
                                    nc.scalar.activation(ht[b][c][:, half],
                                                         tpb[c][:, half],
                                                         AF.Copy)
                                else:
                                    nc.vector.tensor_copy(ht[b][c][:, half],
                                                          tpb[c][:, half])
                return hvp, ht

            h1vp, h1t = hop(lambda b, k: xv[b][k][:], "H1")
            # -- skip conv -> relu -> end1 matmul -> SBUF accumulator
            rsk = [ap.tile((SKR, N), BF16, tag=f"rsk{b}", name=f"rsk{b}_{i}")
                   for b in BS]
            sks = {}
            for nsi, ns in enumerate(NS):
                for b in BS:
                    sk_ps = pp.tile((SKR, 512), F32, tag="pwork", bufs=3,
                                    name=f"skp{b}_{i}_{nsi}")
                    nc.tensor.matmul(sk_ps[:64], wskip[i][0][:],
                                     xn[b][0][:, ns], start=True, stop=True)
                    nc.tensor.matmul(sk_ps[64:], wskip[i][1][:],
                                     xn[b][1][:, ns], start=True, stop=True)
                    sks[b, nsi] = sk_ps
                for b in BS:
                    if b % 2 == 0:
                        nc.vector.tensor_scalar(rsk[b][:, ns], sks[b, nsi][:],
                                                vcol(f"skb{i}", SKR), 0.0,
                                                ALU.add, ALU.max)
                    else:
                        nc.scalar.activation(rsk[b][:, ns], sks[b, nsi][:],
                                             AF.Relu, bias=vcol(f"skb{i}", SKR))
            for nsi, ns in enumerate(NS):
                for b in BS:
                    e_ps = pp.tile((64, 512), F32, tag="pwork", bufs=3,
                                   name=f"eps{b}_{i}_{nsi}")
                    nc.tensor.matmul(e_ps[:], we1[i][:], rsk[b][:, ns],
                                     start=True, stop=True)
                    if i == 0:
                        nc.vector.tensor_copy(st[b]["end"][:, ns], e_ps[:])
                    else:
                        nc.vector.scalar_tensor_tensor(
                            st[b]["end"][:, ns], e_ps[:], 0.0,
                            st[b]["end"][:, ns], ALU.bypass, ALU.add)

            _, h2t = hop(
                lambda b, k: h1vp[b][k // 2][:, (k % 2) * CL:(k % 2) * CL + CL],
                "H2")

            # -- gconv (block-diag over l); residual x and av*xa are
            # accumulated in PSUM via identity/diagonal matmuls, then a
            # single affine eviction applies the batchnorm.
            for c in range(2):
                rows = CH[c][1]
                nxt = [ap.tile((rows, N), F32R, tag=f"XT{b}_{c}", bufs=2,
                               name=f"XT{b}_{i}_{c}") for b in BS]
                for nsi, ns in enumerate(NS):
                    gps = []
                    for b in BS:
                        g_ps = pp.tile((rows, 512), F32, tag="pwork", bufs=3,
                                       name=f"gp{b}_{i}_{c}_{nsi}")
                        srcs = (xn[b], h1t[b], h2t[b])
                        for s in range(3):
                            nc.tensor.matmul(g_ps[:], gcw[b][s][c][:],
                                             srcs[s][c][:, ns],
                                             start=(s == 0), stop=False)
                        nc.tensor.matmul(g_ps[:], idenr[:rows, :rows],
                                         xt[b][c][:, ns],
                                         start=False, stop=False)
                        nc.tensor.matmul(g_ps[:], wav[i][c][:],
                                         xa[b][c][:, ns],
                                         start=False, stop=True)
                        gps.append(g_ps)
                    for b in BS:
                        if b % 2 == 0:
                            nc.scalar.activation(nxt[b][:, ns], gps[b][:],
                                                 AF.Identity,
                                                 bias=vcol(f"bnb{i}_{c}", rows),
                                                 scale=vcol(f"bns{i}_{c}", rows))
                        else:
                            nc.vector.tensor_scalar(
                                nxt[b][:, ns], gps[b][:],
                                vcol(f"bns{i}_{c}", rows),
                                vcol(f"bnb{i}_{c}", rows), ALU.mult, ALU.add)
                for b in BS:
                    xt[b][c] = nxt[b]

        # ---------------- end convs ----------------
        def tail(b):
            o1 = ap.tile((64, N), F32R, tag="o1", name=f"o1_{b}")
            ob = ap.tile((12, N), F32, tag="ob", name=f"ob{b}")
            for nsi, ns in enumerate(NS):
                nc.scalar.activation(o1[:, ns], st[b]["end"][:, ns], AF.Relu,
                                     bias=vcol("e1b", 64))
                o2_ps = pp.tile((12, 512), F32, tag="pwork", bufs=3,
                                name=f"o2p{b}_{nsi}")
                nc.tensor.matmul(o2_ps[:], we2[:], o1[:, ns],
                                 start=True, stop=True)
                nc.scalar.activation(ob[:, ns], o2_ps[:], AF.Identity,
                                     bias=vcol("e2b", 12))
            nc.sync.dma_start(out=outp[b], in_=ob[:])

        phase0_pair()
        fillers = []
        for b in range(BPC):
            fillers.extend(start(b))
        phase0_D(fillers)
        for i in range(L):
            layer_pair(i)
        for b in range(BPC):
            tail(b)

    nc.finalize()
    return nc


# ----------------------------------------------------------------------------
# host-side preprocessing
# ----------------------------------------------------------------------------

def _prep_host(inputs):
    f = lambda x: np.asarray(x, dtype=np.float32)
    bf = lambda x: np.ascontiguousarray(x).astype(ml_dtypes.bfloat16)
    x_in = f(inputs["inputs"])
    ind = np.asarray(inputs["ind"]).astype(np.int64)
    p1, p2, p3, pk = f(inputs["p1"]), f(inputs["p2"]), f(inputs["p3"]), f(inputs["pk"])

    xo = np.pad(x_in, ((0, 0), (0, 0), (0, 0), (RF - T, 0)))
    inp_t = np.ascontiguousarray(xo.transpose(0, 1, 3, 2))
    te = p1[ind]
    adp = np.einsum("bi,ijk->bjk", te, pk).astype(np.float32)

    start_w, start_b = f(inputs["start_w"]), f(inputs["start_b"])
    starta_w, starta_b = f(inputs["starta_w"]), f(inputs["starta_b"])
    fc1_w, fc2_w = f(inputs["fc1_w"]), f(inputs["fc2_w"])
    skip_w, skip_b = f(inputs["skip_w"]), f(inputs["skip_b"])
    gconv_w, gconv_b = f(inputs["gconv_w"]), f(inputs["gconv_b"])
    bn_g, bn_b = f(inputs["bn_g"]), f(inputs["bn_b"])
    bna_g, bna_b = f(inputs["bna_g"]), f(inputs["bna_b"])
    end1_w, end1_b = f(inputs["end1_w"]), f(inputs["end1_b"])
    end2_w, end2_b = f(inputs["end2_w"]), f(inputs["end2_b"])

    e8, e5 = np.eye(8, dtype=np.float32), np.eye(5, dtype=np.float32)
    e13 = np.eye(RF, dtype=np.float32)
    kr = lambda e, w: np.kron(e, np.ascontiguousarray(w.T)).astype(np.float32)

    wstart0 = np.stack([np.kron(e13[:, :8], w[:, 0][None, :])
                        for w in (start_w, starta_w)]).astype(np.float32)
    wstart1 = np.stack([np.kron(e13[:, 8:], w[:, 0][None, :])
                        for w in (start_w, starta_w)]).astype(np.float32)
    wgc0 = np.stack([np.stack([kr(e8, gconv_w[i][:, s * 16:(s + 1) * 16])
                               for s in range(3)]) for i in range(L)])
    wgc1 = np.stack([np.stack([kr(e5, gconv_w[i][:, s * 16:(s + 1) * 16])
                               for s in range(3)]) for i in range(L)])
    wskip0 = np.stack([kr(e8, skip_w[i]) for i in range(L)])
    wskip1 = np.stack([kr(e5, skip_w[i]) for i in range(L)])

    # end1 columns: ref skip rows are o*13+l within the (L-1-i)-th block;
    # ours are l*8+o
    we1 = np.zeros((L, SKR, 64), dtype=np.float32)
    ll, oo = np.meshgrid(np.arange(RF), np.arange(SC), indexing="ij")
    src_col = oo.ravel() * RF + ll.ravel()
    for i in range(L):
        we1[i] = end1_w[:, (L - 1 - i) * SKR + src_col].T

    t8 = lambda v: np.tile(v, 8)
    vecs = np.zeros((128, NV_COLS), dtype=np.float32)
    ci = 0
    vecs[:, ci] = t8(start_b); ci += 1
    vecs[:80, ci] = np.tile(start_b, 5); ci += 1
    vecs[:, ci] = t8(starta_b); ci += 1
    vecs[:80, ci] = np.tile(starta_b, 5); ci += 1
    for i in range(L):
        vecs[:SKR, ci] = np.tile(skip_b[i], RF); ci += 1
    bns = (bn_g / np.sqrt(1.0 + BN_EPS)).astype(np.float32)
    bnas = (bna_g / np.sqrt(1.0 + BN_EPS)).astype(np.float32)
    av = np.ones(16, dtype=np.float32)
    bv = np.zeros(16, dtype=np.float32)
    for i in range(L):
        bnb_adj = bn_b[i] + bns[i] * (gconv_b[i] + bv)
        vecs[:, ci] = t8(bns[i]); ci += 1
        vecs[:, ci] = t8(bnb_adj); ci += 1
        vecs[:, ci] = t8(av); ci += 1
        vecs[:80, ci] = np.tile(bns[i], 5); ci += 1
        vecs[:80, ci] = np.tile(bnb_adj, 5); ci += 1
        vecs[:80, ci] = np.tile(av, 5); ci += 1
        av = 2.0 * bnas[i] * av
        bv = 2.0 * bnas[i] * bv + bna_b[i]
    # rebuild per-layer diag(av) for the PE-side xa accumulation
    avs = [np.ones(16, dtype=np.float32)]
    for i in range(L):
        avs.append(2.0 * bnas[i] * avs[-1])
    wav0 = np.stack([np.diag(np.tile(avs[i], 8)) for i in range(L)])
    wav1 = np.stack([np.diag(np.tile(avs[i], 5)) for i in range(L)])
    wav0 = wav0.astype(ml_dtypes.bfloat16)
    wav1 = wav1.astype(ml_dtypes.bfloat16)
    vecs[:64, ci] = end1_b; ci += 1
    vecs[:12, ci] = end2_b; ci += 1
    assert ci == NV_COLS

    shared = {
        "p2T": np.ascontiguousarray(p2.T),
        "p3sT": np.ascontiguousarray(p3[:DIMS, :DIMS].T),
        "wstart0": wstart0, "wstart1": wstart1,
        "wfc1_0": kr(e8, fc1_w), "wfc1_1": kr(e5, fc1_w),
        "wfc2_0": bf(kr(e8, fc2_w)), "wfc2_1": bf(kr(e5, fc2_w)),
        "wskip0": bf(wskip0), "wskip1": bf(wskip1),
        "wgc0": bf(wgc0), "wgc1": bf(wgc1),
        "we1": bf(we1), "we2": np.ascontiguousarray(end2_w.T),
        "idenb": np.eye(128, dtype=ml_dtypes.bfloat16),
        "idenr": np.eye(128, dtype=np.float32),
        "idenh": 0.5 * np.eye(128, dtype=np.float32),
        "wav0": wav0, "wav1": wav1,
        "vecs": vecs,
    }
    in_maps = []
    for c in range(NCORES):
        bs = slice(c * BPC, (c + 1) * BPC)
        m = dict(shared)
        m["inp"] = np.ascontiguousarray(inp_t[bs])
        m["adp"] = np.ascontiguousarray(adp[bs])
        in_maps.append(m)
    return in_maps


def _get_nc():
    global _CACHED
    if _CACHED is None:
        _CACHED = _build_nc()
    return _CACHED


def run(inputs, trace=False):
    nc = _get_nc()
    in_maps = _prep_host(inputs)
    res = run_bass_kernel_spmd(nc, in_maps, core_ids=list(range(NCORES)),
                               trace=trace)
    out = np.stack([res.results[c]["outp"] for c in range(NCORES)])
    out = out.reshape(B, 12, N, 1).astype(np.float32)
    return out, res


def kernel(**inputs):
    out, _ = run(inputs)
    return out


# revision 34
# speedup vs baseline: 3573.7283x; 1.0285x over previous
"""DMSTGCN forward on 8 Trainium2 NeuronCores (Bass/Tile).

Sharding: data-parallel over batch B=16 -> 2 batches per core; parameters
replicated. The dynamic adjacency (1024x1024 per batch) is built and kept in
SBUF (bf16); 1x1 convs run as block-diagonal (W (x) I) matmuls in an l-major
"[(time,chan), node]" layout, graph hops in "[node, (time,chan)]" layout with
PE transposes between the two. Trunk math is float32r (TF32-like), graph-hop
operands bf16. The two batches are emitted layer-interleaved, all heavy ops
are sliced per 512 nodes, and PSUM tiles are single-bank so the scheduler can
overlap the two batch streams.
"""
import numpy as np
import ml_dtypes

import concourse.bacc as bacc
import concourse.mybir as mybir
from concourse.tile import TileContext
from concourse.bass_utils import run_bass_kernel_spmd

F32 = mybir.dt.float32
F32R = mybir.dt.float32r
BF16 = mybir.dt.bfloat16
AF = mybir.ActivationFunctionType
ALU = mybir.AluOpType

B, N, T, RF = 16, 2, 1024, 12  # placeholder, fixed below
B, N, T, RF = 16, 1024, 12, 13
RC, SC, DIMS, L = 16, 8, 32, 8
BN_EPS = 1e-5
NCORES = 8
BPC = B // NCORES          # batches per core
CL = RC * RF               # 208 rows in T-layout
SKR = SC * RF              # 104 skip rows
CH = ((0, 128), (128, 80))  # l-major T-layout row chunks
NV_COLS = 4 + L + L * 2 * 3 + 2

_CACHED = None


def _build_nc():
    nc = bacc.Bacc("TRN2", target_bir_lowering=False)

    d = {}
    def din(name, shape, dt=F32R):
        d[name] = nc.dram_tensor(name, list(shape), dt, kind="ExternalInput")

    din("inp", (BPC, 2, RF, N))
    din("adp", (BPC, DIMS, DIMS))
    din("p2T", (DIMS, N))
    din("p3sT", (DIMS, DIMS))
    din("wstart0", (2, RF, 128))
    din("wstart1", (2, RF, 80))
    din("wfc1_0", (128, 128)); din("wfc1_1", (80, 80))
    din("wfc2_0", (128, 128), BF16); din("wfc2_1", (80, 80), BF16)
    din("wskip0", (L, 128, 64), BF16)
    din("wskip1", (L, 80, 40), BF16)
    din("wgc0", (L, 3, 128, 128), BF16)
    din("wgc1", (L, 3, 80, 80), BF16)
    din("we1", (L, SKR, 64), BF16)
    din("we2", (64, 12))
    din("idenb", (128, 128), BF16)
    din("idenr", (128, 128))
    din("idenh", (128, 128))
    din("wav0", (L, 128, 128), BF16)
    din("wav1", (L, 80, 80), BF16)
    din("vecs", (128, NV_COLS), F32)
    outp = nc.dram_tensor("outp", [BPC, 12, N], F32, kind="ExternalOutput")

    with TileContext(nc) as tc, \
         tc.tile_pool(name="wp", bufs=1) as wp, \
         tc.tile_pool(name="ap", bufs=1) as ap, \
         tc.tile_pool(name="pp", bufs=1, space="PSUM") as pp:

        def wtile(name, src_ap, shape, dt=F32R, eng=None):
            t = wp.tile(shape, dt, tag=name, name=name)
            (eng or nc.sync).dma_start(out=t[:], in_=src_ap)
            return t

        # phase0-critical loads go first on the SP queue; bulk weights on
        # gpsimd so PE can start within ~2us.
        p2T = wtile("p2T", d["p2T"][:], (DIMS, N), eng=nc.sync)
        p3sT = wtile("p3sT", d["p3sT"][:], (DIMS, DIMS), eng=nc.sync)
        adps = [wtile(f"adp{b}", d["adp"][b], (DIMS, DIMS), eng=nc.sync)
                for b in range(BPC)]

        inps = []
        for b in range(BPC):
            t0 = ap.tile((RF, N), F32R, tag="in0", name=f"in0_{b}")[:]
            t1 = ap.tile((RF, N), F32R, tag="in1", name=f"in1_{b}")[:]
            nc.sync.dma_start(out=t0, in_=d["inp"][b, 0])
            nc.sync.dma_start(out=t1, in_=d["inp"][b, 1])
            inps.append((t0, t1))

        idenb = wtile("idenb", d["idenb"][:], (128, 128), BF16)
        idenr = wtile("idenr", d["idenr"][:], (128, 128))
        idenh = wtile("idenh", d["idenh"][:], (128, 128))
        wav = [[wtile(f"wav{i}_{c}", d[f"wav{c}"][i],
                      (CH[c][1], CH[c][1]), BF16) for c in range(2)]
               for i in range(L)]
        vecs = wtile("vecs", d["vecs"][:], (128, NV_COLS), F32)
        wstart = [[wtile(f"wst{s}_{c}", d[f"wstart{c}"][s],
                         (RF, CH[c][1])) for c in range(2)] for s in range(2)]
        wfc1 = [wtile(f"wfc1_{c}", d[f"wfc1_{c}"][:],
                      (CH[c][1], CH[c][1])) for c in range(2)]
        wfc2 = [wtile(f"wfc2_{c}", d[f"wfc2_{c}"][:],
                      (CH[c][1], CH[c][1]), BF16) for c in range(2)]
        wskip = [[wtile(f"wsk{i}_{c}", d[f"wskip{c}"][i],
                        (CH[c][1], (64, 40)[c]), BF16) for c in range(2)]
                 for i in range(L)]
        we1 = [wtile(f"we1_{i}", d["we1"][i], (SKR, 64), BF16) for i in range(L)]
        we2 = wtile("we2", d["we2"][:], (64, 12))

        vc = {}
        ci = 0
        for nm in ("sb0", "sb1", "sab0", "sab1"):
            vc[nm] = ci; ci += 1
        for i in range(L):
            vc[f"skb{i}"] = ci; ci += 1
        for i in range(L):
            for c in range(2):
                for nm in ("bns", "bnb", "av"):
                    vc[f"{nm}{i}_{c}"] = ci; ci += 1
        vc["e1b"] = ci; ci += 1
        vc["e2b"] = ci; ci += 1
        assert ci == NV_COLS

        def vcol(nm, rows=128):
            return vecs[:rows, vc[nm]:vc[nm] + 1]

        NS = (slice(0, 512), slice(512, 1024))

        st = [dict() for _ in range(BPC)]

        # ---------------- adjacency (both batches interleaved) ----------
        def phase0_pair():
            BS = range(BPC)
            # L-stack rows: [u; -srcT], R-stack rows: [srcT; u] so that
            # D = x1^T - x1 is ONE K=64 matmul per (v, ns).
            Lst = [ap.tile((64, N), F32R, tag=f"Lst{b}", name=f"Lst{b}")
                   for b in BS]
            Rst = [ap.tile((64, N), F32R, tag=f"Rst{b}", name=f"Rst{b}")
                   for b in BS]
            for nsi, ns in enumerate(NS):
                pss = []
                for b in BS:
                    ps = pp.tile((DIMS, 512), F32, tag="pwork", bufs=3,
                                 name=f"srcTps{b}_{nsi}")
                    nc.tensor.matmul(ps[:], adps[b][:], p2T[:, ns],
                                     start=True, stop=True)
                    pss.append(ps)
                for b in BS:
                    nc.scalar.activation(Rst[b][0:32, ns], pss[b][:], AF.Copy)
                    nc.vector.tensor_scalar(Lst[b][32:64, ns], pss[b][:],
                                            -1.0, None, ALU.mult)
            for nsi, ns in enumerate(NS):
                pss = []
                for b in BS:
                    ps = pp.tile((DIMS, 512), F32, tag="pwork", bufs=3,
                                 name=f"ups{b}_{nsi}")
                    nc.tensor.matmul(ps[:], p3sT[:], Rst[b][0:32, ns],
                                     start=True, stop=True)
                    pss.append(ps)
                for b in BS:
                    nc.scalar.activation(Lst[b][0:32, ns], pss[b][:], AF.Copy)
                    nc.vector.tensor_copy(Rst[b][32:64, ns], pss[b][:])
            st[0]["LR"] = (Lst, Rst)

        def phase0_D(fillers=()):
            BS = range(BPC)
            fillers = list(fillers)
            Lst, Rst = st[0]["LR"]
            ATs = [[ap.tile((128, N), BF16, tag=f"AT{b}_{v}", name=f"AT{b}_{v}")
                    for v in range(8)] for b in BS]
            Dts = [[ap.tile((128, N), BF16, tag=f"Dt{b}",
                            name=f"Dt{b}_{v}") for v in range(8)] for b in BS]
            for v in range(8):
                cs = slice(v * 128, (v + 1) * 128)
                for nsi, ns in enumerate(NS):
                    dpss = []
                    for b in BS:
                        dps = pp.tile((128, 512), F32, tag="pwork", bufs=3,
                                      name=f"dps{b}_{v}_{nsi}")
                        nc.tensor.matmul(dps[:], Lst[b][:, cs], Rst[b][:, ns],
                                         start=True, stop=True)
                        dpss.append(dps)
                    # relu(tanh(D)) == tanh(max(D, 0)); relu on DVE so the
                    # tanh-bound ACT queue stays short
                    for b in BS:
                        nc.vector.tensor_scalar(Dts[b][v][:, ns], dpss[b][:],
                                                0.0, None, ALU.max)
                    for b in BS:
                        nc.scalar.activation(ATs[b][v][:, ns], Dts[b][v][:, ns],
                                             AF.Tanh)
                if v % 2 == 1 and fillers:
                    fillers.pop(0)()
            while fillers:
                fillers.pop(0)()
            for b in BS:
                st[b]["AT"] = ATs[b]

        # ---------------- start convs (emitted as fillers in phase0_D) ----
        def start(b):
            in0, in1 = inps[b]
            xt, xa = [None, None], [None, None]
            fillers = []
            for c in range(2):
                rows = CH[c][1]
                xt[c] = ap.tile((rows, N), F32R, tag=f"XT{b}_{c}", bufs=2,
                                name=f"XT{b}_{c}_init")
                xa[c] = ap.tile((rows, N), BF16, tag=f"XA{b}_{c}",
                                name=f"XA{b}_{c}")
                def mk(c, xtt, xat):
                    rows = CH[c][1]
                    def emit():
                        for nsi, ns in enumerate(NS):
                            ps = pp.tile((rows, 512), F32, tag="pwork", bufs=3,
                                         name=f"stp{b}_{c}_{nsi}")
                            nc.tensor.matmul(ps[:], wstart[0][c][:],
                                             in0[:, ns], start=True, stop=True)
                            nc.scalar.activation(xtt[:, ns], ps[:], AF.Identity,
                                                 bias=vcol(f"sb{c}", rows))
                            psa = pp.tile((rows, 512), F32, tag="pwork",
                                          bufs=3, name=f"stpa{b}_{c}_{nsi}")
                            nc.tensor.matmul(psa[:], wstart[1][c][:],
                                             in1[:, ns], start=True, stop=True)
                            nc.scalar.activation(xat[:, ns], psa[:],
                                                 AF.Identity,
                                                 bias=vcol(f"sab{c}", rows))
                    return emit
                fillers.append(mk(c, xt[c], xa[c]))
            st[b]["xt"], st[b]["xa"] = xt, xa
            st[b]["end"] = ap.tile((64, N), F32, tag=f"END{b}", name=f"END{b}")
            return fillers

        # ---------------- one layer, both batches stage-interleaved ----------
        def layer_pair(i):
            BS = range(BPC)
            xt = [st[b]["xt"] for b in BS]
            xa = [st[b]["xa"] for b in BS]
            AT = [st[b]["AT"] for b in BS]

            gcw = [[[ap.tile((CH[c][1], CH[c][1]), BF16, tag=f"gcw{b}_{c}_{s}",
                             bufs=2, name=f"gcw{b}_{i}_{c}_{s}")
                     for c in range(2)] for s in range(3)] for b in BS]
            for b in BS:
                for s in range(3):
                    for c in range(2):
                        nc.sync.dma_start(out=gcw[b][s][c][:],
                                          in_=d[f"wgc{c}"][i, s])

            # -- attention + sigmoid
            xn = [[None, None] for b in BS]
            r1 = [[None, None] for b in BS]
            for b in BS:
                for c in range(2):
                    rows = CH[c][1]
                    r1[b][c] = ap.tile((rows, N), BF16, tag=f"R1{b}_{c}",
                                       name=f"R1{b}_{i}_{c}")
                    xn[b][c] = ap.tile((rows, N), BF16, tag=f"XN{b}_{c}",
                                       name=f"XN{b}_{i}_{c}")
            groups = [(c, nsi) for c in range(2) for nsi in range(2)]
            m1s, apss = {}, {}
            for c, nsi in groups:
                rows, ns = CH[c][1], NS[nsi]
                for b in BS:
                    m1 = pp.tile((rows, 512), F32, tag="pwork", bufs=3,
                                 name=f"m1_{b}_{i}_{c}_{nsi}")
                    nc.tensor.matmul(m1[:], wfc1[c][:], xt[b][c][:, ns],
                                     start=True, stop=True)
                    m1s[b, c, nsi] = m1
                for b in BS:
                    if b % 2 == 0:
                        nc.scalar.activation(r1[b][c][:, ns], m1s[b, c, nsi][:],
                                             AF.Relu)
                    else:
                        nc.vector.tensor_scalar(r1[b][c][:, ns],
                                                m1s[b, c, nsi][:],
                                                0.0, None, ALU.max)
            for c, nsi in groups:
                rows, ns = CH[c][1], NS[nsi]
                for b in BS:
                    a_ps = pp.tile((rows, 512), F32, tag="pwork", bufs=3,
                                   name=f"aps{b}_{i}_{c}_{nsi}")
                    nc.tensor.matmul(a_ps[:], wfc2[c][:], r1[b][c][:, ns],
                                     start=True, stop=False)
                    nc.tensor.matmul(a_ps[:], idenh[:rows, :rows],
                                     xt[b][c][:, ns], start=False, stop=True)
                    apss[b, c, nsi] = a_ps
                for b in BS:
                    # xn = sigmoid(2*(a + x/2)) straight from PSUM
                    nc.scalar.activation(xn[b][c][:, ns], apss[b, c, nsi][:],
                                         AF.Sigmoid, scale=2.0)

            # -- V-layout of xn via PE transposes
            xv = [[None] * 8 for b in BS]
            for v in range(8):
                cs = slice(v * 128, (v + 1) * 128)
                for b in BS:
                    tp = pp.tile((128, CL), BF16, tag="ptr", bufs=3,
                                 name=f"tpx{b}_{i}_{v}")
                    for c in range(2):
                        o, rows = CH[c]
                        nc.tensor.transpose(tp[:, o:o + rows],
                                            xn[b][c][:, cs],
                                            idenb[:rows, :rows])
                    xv[b][v] = ap.tile((128, CL), BF16, tag=f"XV{b}_{v}",
                                       name=f"XV{b}_{i}_{v}")
                    nc.vector.tensor_copy(xv[b][v][:], tp[:])

            def hop(rv, nm):
                """A-hop (V-orientation, w-pairs) + transpose back, both b."""
                hvp = [[None] * 4 for b in BS]
                for p in range(4):
                    for b in BS:
                        h_ps = pp.tile((128, 2 * CL), F32, tag="ptr", bufs=3,
                                       name=f"hp{nm}{b}_{i}_{p}")
                        for half in range(2):
                            w = 2 * p + half
                            ws = slice(w * 128, (w + 1) * 128)
                            dst = h_ps[:, half * CL:(half + 1) * CL]
                            for k in range(8):
                                nc.tensor.matmul(dst, AT[b][k][:, ws], rv(b, k),
                                                 start=(k == 0), stop=(k == 7))
                        hvp[b][p] = ap.tile((128, 2 * CL), BF16,
                                            tag=f"{nm}V{b}_{p}",
                                            name=f"{nm}V{b}_{i}_{p}")
                        if (b + p) % 2 == 0:
                            nc.vector.tensor_copy(hvp[b][p][:], h_ps[:])
                        else:
                            nc.scalar.activation(hvp[b][p][:], h_ps[:], AF.Copy)

                ht = [[ap.tile((CH[c][1], N), BF16, tag=f"{nm}T{b}_{c}",
                               name=f"{nm}T{b}_{i}_{c}") for c in range(2)]
                      for b in BS]
                for b in BS:
                    tpb = [pp.tile((CH[c][1], N), BF16, tag=f"ptb{c}",
                                   bufs=1, name=f"tpb{nm}{b}_{i}_{c}")
                           for c in range(2)]
                    for w in range(8):
                        src = hvp[b][w // 2][:, (w % 2) * CL:(w % 2) * CL + CL]
                        for c in range(2):
                            o, rows = CH[c]
                            nc.tensor.transpose(
                                tpb[c][:, w * 128:(w + 1) * 128],
                                src[:, o:o + rows], idenb[:, :])
                        if w % 4 == 3:
                            half = slice((w - 3) * 128, (w + 1) * 128)
                            for c in range(2):
                                if (b + c) % 2 == 0:
                                    nc.scalar.activation(ht[b][c][:, half],
                                                         tpb[c][:, half],
                                                         AF.Copy)
                                else:
                                    nc.vector.tensor_copy(ht[b][c][:, half],
                                                          tpb[c][:, half])
                return hvp, ht

            h1vp, h1t = hop(lambda b, k: xv[b][k][:], "H1")
            # -- skip conv -> relu -> end1 matmul -> SBUF accumulator
            rsk = [ap.tile((SKR, N), BF16, tag=f"rsk{b}", name=f"rsk{b}_{i}")
                   for b in BS]
            sks = {}
            for nsi, ns in enumerate(NS):
                for b in BS:
                    sk_ps = pp.tile((SKR, 512), F32, tag="pwork", bufs=3,
                                    name=f"skp{b}_{i}_{nsi}")
                    nc.tensor.matmul(sk_ps[:64], wskip[i][0][:],
                                     xn[b][0][:, ns], start=True, stop=True)
                    nc.tensor.matmul(sk_ps[64:], wskip[i][1][:],
                                     xn[b][1][:, ns], start=True, stop=True)
                    sks[b, nsi] = sk_ps
                for b in BS:
                    if b % 2 == 0:
                        nc.vector.tensor_scalar(rsk[b][:, ns], sks[b, nsi][:],
                                                vcol(f"skb{i}", SKR), 0.0,
                                                ALU.add, ALU.max)
                    else:
                        nc.scalar.activation(rsk[b][:, ns], sks[b, nsi][:],
                                             AF.Relu, bias=vcol(f"skb{i}", SKR))
            for nsi, ns in enumerate(NS):
                for b in BS:
                    e_ps = pp.tile((64, 512), F32, tag="pwork", bufs=3,
                                   name=f"eps{b}_{i}_{nsi}")
                    nc.tensor.matmul(e_ps[:], we1[i][:], rsk[b][:, ns],
                                     start=True, stop=True)
                    if i == 0:
                        nc.vector.tensor_copy(st[b]["end"][:, ns], e_ps[:])
                    else:
                        nc.vector.scalar_tensor_tensor(
                            st[b]["end"][:, ns], e_ps[:], 0.0,
                            st[b]["end"][:, ns], ALU.bypass, ALU.add)

            _, h2t = hop(
                lambda b, k: h1vp[b][k // 2][:, (k % 2) * CL:(k % 2) * CL + CL],
                "H2")

            # -- gconv (block-diag over l); av*xa accumulated in PSUM via a
            # diagonal matmul; bn affine applied on eviction; the residual
            # bns*x is added by the otherwise-idle Pool engine.
            for c in range(2):
                rows = CH[c][1]
                nxs = [ap.tile((rows, N), F32, tag=f"tmp{b}_{c}",
                               name=f"nxs{b}_{i}_{c}") for b in BS]
                nxt = [ap.tile((rows, N), F32R, tag=f"XT{b}_{c}", bufs=2,
                               name=f"XT{b}_{i}_{c}") for b in BS]
                # Pool precomputes pre = bns*x + bnb early (depends only on
                # layer-start x); the PSUM eviction is one DVE op.
                for nsi, ns in enumerate(NS):
                    for b in BS:
                        nc.gpsimd.tensor_scalar(
                            nxs[b][:, ns], xt[b][c][:, ns].bitcast(F32),
                            vcol(f"bns{i}_{c}", rows),
                            vcol(f"bnb{i}_{c}", rows), ALU.mult, ALU.add)
                for nsi, ns in enumerate(NS):
                    gps = []
                    for b in BS:
                        g_ps = pp.tile((rows, 512), F32, tag="pwork", bufs=3,
                                       name=f"gp{b}_{i}_{c}_{nsi}")
                        srcs = (xn[b], h1t[b], h2t[b])
                        for s in range(3):
                            nc.tensor.matmul(g_ps[:], gcw[b][s][c][:],
                                             srcs[s][c][:, ns],
                                             start=(s == 0), stop=False)
                        nc.tensor.matmul(g_ps[:], wav[i][c][:],
                                         xa[b][c][:, ns],
                                         start=False, stop=True)
                        gps.append(g_ps)
                    for b in BS:
                        nc.vector.scalar_tensor_tensor(
                            nxt[b][:, ns], gps[b][:],
                            vcol(f"bns{i}_{c}", rows), nxs[b][:, ns],
                            ALU.mult, ALU.add)
                for b in BS:
                    xt[b][c] = nxt[b]

        # ---------------- end convs ----------------
        def tail(b):
            o1 = ap.tile((64, N), F32R, tag="o1", name=f"o1_{b}")
            ob = ap.tile((12, N), F32, tag="ob", name=f"ob{b}")
            for nsi, ns in enumerate(NS):
                nc.scalar.activation(o1[:, ns], st[b]["end"][:, ns], AF.Relu,
                                     bias=vcol("e1b", 64))
                o2_ps = pp.tile((12, 512), F32, tag="pwork", bufs=3,
                                name=f"o2p{b}_{nsi}")
                nc.tensor.matmul(o2_ps[:], we2[:], o1[:, ns],
                                 start=True, stop=True)
                nc.scalar.activation(ob[:, ns], o2_ps[:], AF.Identity,
                                     bias=vcol("e2b", 12))
            nc.sync.dma_start(out=outp[b], in_=ob[:])

        phase0_pair()
        fillers = []
        for b in range(BPC):
            fillers.extend(start(b))
        phase0_D(fillers)
        for i in range(L):
            layer_pair(i)
        for b in range(BPC):
            tail(b)

    nc.finalize()
    return nc


# ----------------------------------------------------------------------------
# host-side preprocessing
# ----------------------------------------------------------------------------

def _prep_host(inputs):
    f = lambda x: np.asarray(x, dtype=np.float32)
    bf = lambda x: np.ascontiguousarray(x).astype(ml_dtypes.bfloat16)
    x_in = f(inputs["inputs"])
    ind = np.asarray(inputs["ind"]).astype(np.int64)
    p1, p2, p3, pk = f(inputs["p1"]), f(inputs["p2"]), f(inputs["p3"]), f(inputs["pk"])

    xo = np.pad(x_in, ((0, 0), (0, 0), (0, 0), (RF - T, 0)))
    inp_t = np.ascontiguousarray(xo.transpose(0, 1, 3, 2))
    te = p1[ind]
    adp = np.einsum("bi,ijk->bjk", te, pk).astype(np.float32)

    start_w, start_b = f(inputs["start_w"]), f(inputs["start_b"])
    starta_w, starta_b = f(inputs["starta_w"]), f(inputs["starta_b"])
    fc1_w, fc2_w = f(inputs["fc1_w"]), f(inputs["fc2_w"])
    skip_w, skip_b = f(inputs["skip_w"]), f(inputs["skip_b"])
    gconv_w, gconv_b = f(inputs["gconv_w"]), f(inputs["gconv_b"])
    bn_g, bn_b = f(inputs["bn_g"]), f(inputs["bn_b"])
    bna_g, bna_b = f(inputs["bna_g"]), f(inputs["bna_b"])
    end1_w, end1_b = f(inputs["end1_w"]), f(inputs["end1_b"])
    end2_w, end2_b = f(inputs["end2_w"]), f(inputs["end2_b"])

    e8, e5 = np.eye(8, dtype=np.float32), np.eye(5, dtype=np.float32)
    e13 = np.eye(RF, dtype=np.float32)
    kr = lambda e, w: np.kron(e, np.ascontiguousarray(w.T)).astype(np.float32)

    wstart0 = np.stack([np.kron(e13[:, :8], w[:, 0][None, :])
                        for w in (start_w, starta_w)]).astype(np.float32)
    wstart1 = np.stack([np.kron(e13[:, 8:], w[:, 0][None, :])
                        for w in (start_w, starta_w)]).astype(np.float32)
    wgc0 = np.stack([np.stack([kr(e8, gconv_w[i][:, s * 16:(s + 1) * 16])
                               for s in range(3)]) for i in range(L)])
    wgc1 = np.stack([np.stack([kr(e5, gconv_w[i][:, s * 16:(s + 1) * 16])
                               for s in range(3)]) for i in range(L)])
    wskip0 = np.stack([kr(e8, skip_w[i]) for i in range(L)])
    wskip1 = np.stack([kr(e5, skip_w[i]) for i in range(L)])

    # end1 columns: ref skip rows are o*13+l within the (L-1-i)-th block;
    # ours are l*8+o
    we1 = np.zeros((L, SKR, 64), dtype=np.float32)
    ll, oo = np.meshgrid(np.arange(RF), np.arange(SC), indexing="ij")
    src_col = oo.ravel() * RF + ll.ravel()
    for i in range(L):
        we1[i] = end1_w[:, (L - 1 - i) * SKR + src_col].T

    t8 = lambda v: np.tile(v, 8)
    vecs = np.zeros((128, NV_COLS), dtype=np.float32)
    ci = 0
    vecs[:, ci] = t8(start_b); ci += 1
    vecs[:80, ci] = np.tile(start_b, 5); ci += 1
    vecs[:, ci] = t8(starta_b); ci += 1
    vecs[:80, ci] = np.tile(starta_b, 5); ci += 1
    for i in range(L):
        vecs[:SKR, ci] = np.tile(skip_b[i], RF); ci += 1
    bns = (bn_g / np.sqrt(1.0 + BN_EPS)).astype(np.float32)
    bnas = (bna_g / np.sqrt(1.0 + BN_EPS)).astype(np.float32)
    av = np.ones(16, dtype=np.float32)
    bv = np.zeros(16, dtype=np.float32)
    for i in range(L):
        bnb_adj = bn_b[i] + bns[i] * (gconv_b[i] + bv)
        vecs[:, ci] = t8(bns[i]); ci += 1
        vecs[:, ci] = t8(bnb_adj); ci += 1
        vecs[:, ci] = t8(av); ci += 1
        vecs[:80, ci] = np.tile(bns[i], 5); ci += 1
        vecs[:80, ci] = np.tile(bnb_adj, 5); ci += 1
        vecs[:80, ci] = np.tile(av, 5); ci += 1
        av = 2.0 * bnas[i] * av
        bv = 2.0 * bnas[i] * bv + bna_b[i]
    # rebuild per-layer diag(av) for the PE-side xa accumulation
    avs = [np.ones(16, dtype=np.float32)]
    for i in range(L):
        avs.append(2.0 * bnas[i] * avs[-1])
    wav0 = np.stack([np.diag(np.tile(avs[i], 8)) for i in range(L)])
    wav1 = np.stack([np.diag(np.tile(avs[i], 5)) for i in range(L)])
    wav0 = wav0.astype(ml_dtypes.bfloat16)
    wav1 = wav1.astype(ml_dtypes.bfloat16)
    vecs[:64, ci] = end1_b; ci += 1
    vecs[:12, ci] = end2_b; ci += 1
    assert ci == NV_COLS

    shared = {
        "p2T": np.ascontiguousarray(p2.T),
        "p3sT": np.ascontiguousarray(p3[:DIMS, :DIMS].T),
        "wstart0": wstart0, "wstart1": wstart1,
        "wfc1_0": kr(e8, fc1_w), "wfc1_1": kr(e5, fc1_w),
        "wfc2_0": bf(kr(e8, fc2_w)), "wfc2_1": bf(kr(e5, fc2_w)),
        "wskip0": bf(wskip0), "wskip1": bf(wskip1),
        "wgc0": bf(wgc0), "wgc1": bf(wgc1),
        "we1": bf(we1), "we2": np.ascontiguousarray(end2_w.T),
        "idenb": np.eye(128, dtype=ml_dtypes.bfloat16),
        "idenr": np.eye(128, dtype=np.float32),
        "idenh": 0.5 * np.eye(128, dtype=np.float32),
        "wav0": wav0, "wav1": wav1,
        "vecs": vecs,
    }
    in_maps = []
    for c in range(NCORES):
        bs = slice(c * BPC, (c + 1) * BPC)
        m = dict(shared)
        m["inp"] = np.ascontiguousarray(inp_t[bs])
        m["adp"] = np.ascontiguousarray(adp[bs])
        in_maps.append(m)
    return in_maps


def _get_nc():
    global _CACHED
    if _CACHED is None:
        _CACHED = _build_nc()
    return _CACHED


def run(inputs, trace=False):
    nc = _get_nc()
    in_maps = _prep_host(inputs)
    res = run_bass_kernel_spmd(nc, in_maps, core_ids=list(range(NCORES)),
                               trace=trace)
    out = np.stack([res.results[c]["outp"] for c in range(NCORES)])
    out = out.reshape(B, 12, N, 1).astype(np.float32)
    return out, res


def kernel(**inputs):
    out, _ = run(inputs)
    return out


# revision 37
# speedup vs baseline: 3594.7229x; 1.0059x over previous
"""DMSTGCN forward on 8 Trainium2 NeuronCores (Bass/Tile).

Sharding: data-parallel over batch B=16 -> 2 batches per core; parameters
replicated. The dynamic adjacency (1024x1024 per batch) is built and kept in
SBUF (bf16); 1x1 convs run as block-diagonal (W (x) I) matmuls in an l-major
"[(time,chan), node]" layout, graph hops in "[node, (time,chan)]" layout with
PE transposes between the two. Trunk math is float32r (TF32-like), graph-hop
operands bf16. The two batches are emitted layer-interleaved, all heavy ops
are sliced per 512 nodes, and PSUM tiles are single-bank so the scheduler can
overlap the two batch streams.
"""
import numpy as np
import ml_dtypes

import concourse.bacc as bacc
import concourse.mybir as mybir
from concourse.tile import TileContext
from concourse.bass_utils import run_bass_kernel_spmd

F32 = mybir.dt.float32
F32R = mybir.dt.float32r
BF16 = mybir.dt.bfloat16
AF = mybir.ActivationFunctionType
ALU = mybir.AluOpType

B, N, T, RF = 16, 2, 1024, 12  # placeholder, fixed below
B, N, T, RF = 16, 1024, 12, 13
RC, SC, DIMS, L = 16, 8, 32, 8
BN_EPS = 1e-5
NCORES = 8
BPC = B // NCORES          # batches per core
CL = RC * RF               # 208 rows in T-layout
SKR = SC * RF              # 104 skip rows
CH = ((0, 128), (128, 80))  # l-major T-layout row chunks
NV_COLS = 4 + L + L * 2 * 3 + 2

_CACHED = None


def _build_nc():
    nc = bacc.Bacc("TRN2", target_bir_lowering=False)

    d = {}
    def din(name, shape, dt=F32R):
        d[name] = nc.dram_tensor(name, list(shape), dt, kind="ExternalInput")

    din("inp", (BPC, 2, RF, N))
    din("adp", (BPC, DIMS, DIMS))
    din("p2T", (DIMS, N))
    din("p3sT", (DIMS, DIMS))
    din("wstart0", (2, RF, 128))
    din("wstart1", (2, RF, 80))
    din("wfc1_0", (128, 128)); din("wfc1_1", (80, 80))
    din("wfc2_0", (128, 128), BF16); din("wfc2_1", (80, 80), BF16)
    din("wskip0", (L, 128, 64), BF16)
    din("wskip1", (L, 80, 40), BF16)
    din("wgc0", (L, 3, 128, 128), BF16)
    din("wgc1", (L, 3, 80, 80), BF16)
    din("we1", (L, SKR, 64), BF16)
    din("we2", (64, 12))
    din("idenb", (128, 128), BF16)
    din("idenr", (128, 128))
    din("idenh", (128, 128))
    din("wav0", (L, 128, 128), BF16)
    din("wav1", (L, 80, 80), BF16)
    din("vecs", (128, NV_COLS), F32)
    outp = nc.dram_tensor("outp", [BPC, 12, N], F32, kind="ExternalOutput")

    with TileContext(nc) as tc, \
         tc.tile_pool(name="wp", bufs=1) as wp, \
         tc.tile_pool(name="ap", bufs=1) as ap, \
         tc.tile_pool(name="pp", bufs=1, space="PSUM") as pp:

        def wtile(name, src_ap, shape, dt=F32R, eng=None):
            t = wp.tile(shape, dt, tag=name, name=name)
            (eng or nc.sync).dma_start(out=t[:], in_=src_ap)
            return t

        # phase0-critical loads go first on the SP queue; bulk weights on
        # gpsimd so PE can start within ~2us.
        p2T = wtile("p2T", d["p2T"][:], (DIMS, N), eng=nc.sync)
        p3sT = wtile("p3sT", d["p3sT"][:], (DIMS, DIMS), eng=nc.sync)
        adps = [wtile(f"adp{b}", d["adp"][b], (DIMS, DIMS), eng=nc.sync)
                for b in range(BPC)]

        inps = []
        for b in range(BPC):
            t0 = ap.tile((RF, N), F32R, tag="in0", name=f"in0_{b}")[:]
            t1 = ap.tile((RF, N), F32R, tag="in1", name=f"in1_{b}")[:]
            nc.sync.dma_start(out=t0, in_=d["inp"][b, 0])
            nc.sync.dma_start(out=t1, in_=d["inp"][b, 1])
            inps.append((t0, t1))

        idenb = wtile("idenb", d["idenb"][:], (128, 128), BF16)
        idenr = wtile("idenr", d["idenr"][:], (128, 128))
        idenh = wtile("idenh", d["idenh"][:], (128, 128))
        wav = [[wtile(f"wav{i}_{c}", d[f"wav{c}"][i],
                      (CH[c][1], CH[c][1]), BF16) for c in range(2)]
               for i in range(L)]
        vecs = wtile("vecs", d["vecs"][:], (128, NV_COLS), F32)
        wstart = [[wtile(f"wst{s}_{c}", d[f"wstart{c}"][s],
                         (RF, CH[c][1])) for c in range(2)] for s in range(2)]
        wfc1 = [wtile(f"wfc1_{c}", d[f"wfc1_{c}"][:],
                      (CH[c][1], CH[c][1])) for c in range(2)]
        wfc2 = [wtile(f"wfc2_{c}", d[f"wfc2_{c}"][:],
                      (CH[c][1], CH[c][1]), BF16) for c in range(2)]
        wskip = [[wtile(f"wsk{i}_{c}", d[f"wskip{c}"][i],
                        (CH[c][1], (64, 40)[c]), BF16) for c in range(2)]
                 for i in range(L)]
        we1 = [wtile(f"we1_{i}", d["we1"][i], (SKR, 64), BF16) for i in range(L)]
        we2 = wtile("we2", d["we2"][:], (64, 12))

        vc = {}
        ci = 0
        for nm in ("sb0", "sb1", "sab0", "sab1"):
            vc[nm] = ci; ci += 1
        for i in range(L):
            vc[f"skb{i}"] = ci; ci += 1
        for i in range(L):
            for c in range(2):
                for nm in ("bns", "bnb", "av"):
                    vc[f"{nm}{i}_{c}"] = ci; ci += 1
        vc["e1b"] = ci; ci += 1
        vc["e2b"] = ci; ci += 1
        assert ci == NV_COLS

        def vcol(nm, rows=128):
            return vecs[:rows, vc[nm]:vc[nm] + 1]

        NS = (slice(0, 512), slice(512, 1024))

        st = [dict() for _ in range(BPC)]

        # ---------------- adjacency (both batches interleaved) ----------
        def phase0_pair():
            BS = range(BPC)
            # L-stack rows: [u; -srcT], R-stack rows: [srcT; u] so that
            # D = x1^T - x1 is ONE K=64 matmul per (v, ns).
            Lst = [ap.tile((64, N), F32R, tag=f"Lst{b}", name=f"Lst{b}")
                   for b in BS]
            Rst = [ap.tile((64, N), F32R, tag=f"Rst{b}", name=f"Rst{b}")
                   for b in BS]
            for nsi, ns in enumerate(NS):
                pss = []
                for b in BS:
                    ps = pp.tile((DIMS, 512), F32, tag="pwork", bufs=3,
                                 name=f"srcTps{b}_{nsi}")
                    nc.tensor.matmul(ps[:], adps[b][:], p2T[:, ns],
                                     start=True, stop=True)
                    pss.append(ps)
                for b in BS:
                    nc.scalar.activation(Rst[b][0:32, ns], pss[b][:], AF.Copy)
                    nc.vector.tensor_scalar(Lst[b][32:64, ns], pss[b][:],
                                            -1.0, None, ALU.mult)
            for nsi, ns in enumerate(NS):
                pss = []
                for b in BS:
                    ps = pp.tile((DIMS, 512), F32, tag="pwork", bufs=3,
                                 name=f"ups{b}_{nsi}")
                    nc.tensor.matmul(ps[:], p3sT[:], Rst[b][0:32, ns],
                                     start=True, stop=True)
                    pss.append(ps)
                for b in BS:
                    nc.scalar.activation(Lst[b][0:32, ns], pss[b][:], AF.Copy)
                    nc.vector.tensor_copy(Rst[b][32:64, ns], pss[b][:])
            st[0]["LR"] = (Lst, Rst)

        def phase0_D(fillers=()):
            BS = range(BPC)
            fillers = list(fillers)
            Lst, Rst = st[0]["LR"]
            ATs = [[ap.tile((128, N), BF16, tag=f"AT{b}_{v}", name=f"AT{b}_{v}")
                    for v in range(8)] for b in BS]
            Dts = [[ap.tile((128, N), BF16, tag=f"Dt{b}",
                            name=f"Dt{b}_{v}") for v in range(8)] for b in BS]
            for v in range(8):
                cs = slice(v * 128, (v + 1) * 128)
                for nsi, ns in enumerate(NS):
                    dpss = []
                    for b in BS:
                        dps = pp.tile((128, 512), F32, tag="pwork", bufs=3,
                                      name=f"dps{b}_{v}_{nsi}")
                        nc.tensor.matmul(dps[:], Lst[b][:, cs], Rst[b][:, ns],
                                         start=True, stop=True)
                        dpss.append(dps)
                    # relu(tanh(D)): tanh straight from PSUM on ACT (frees
                    # the bank sooner), relu on DVE
                    for b in BS:
                        nc.scalar.activation(Dts[b][v][:, ns], dpss[b][:],
                                             AF.Tanh)
                    for b in BS:
                        nc.vector.tensor_scalar(ATs[b][v][:, ns],
                                                Dts[b][v][:, ns],
                                                0.0, None, ALU.max)
                if v % 2 == 1 and fillers:
                    fillers.pop(0)()
            while fillers:
                fillers.pop(0)()
            for b in BS:
                st[b]["AT"] = ATs[b]

        # ---------------- start convs (emitted as fillers in phase0_D) ----
        def start(b):
            in0, in1 = inps[b]
            xt, xa = [None, None], [None, None]
            fillers = []
            for c in range(2):
                rows = CH[c][1]
                xt[c] = ap.tile((rows, N), F32R, tag=f"XT{b}_{c}", bufs=2,
                                name=f"XT{b}_{c}_init")
                xa[c] = ap.tile((rows, N), BF16, tag=f"XA{b}_{c}",
                                name=f"XA{b}_{c}")
                def mk(c, xtt, xat):
                    rows = CH[c][1]
                    def emit():
                        for nsi, ns in enumerate(NS):
                            ps = pp.tile((rows, 512), F32, tag="pwork", bufs=3,
                                         name=f"stp{b}_{c}_{nsi}")
                            nc.tensor.matmul(ps[:], wstart[0][c][:],
                                             in0[:, ns], start=True, stop=True)
                            nc.scalar.activation(xtt[:, ns], ps[:], AF.Identity,
                                                 bias=vcol(f"sb{c}", rows))
                            psa = pp.tile((rows, 512), F32, tag="pwork",
                                          bufs=3, name=f"stpa{b}_{c}_{nsi}")
                            nc.tensor.matmul(psa[:], wstart[1][c][:],
                                             in1[:, ns], start=True, stop=True)
                            nc.scalar.activation(xat[:, ns], psa[:],
                                                 AF.Identity,
                                                 bias=vcol(f"sab{c}", rows))
                    return emit
                fillers.append(mk(c, xt[c], xa[c]))
            st[b]["xt"], st[b]["xa"] = xt, xa
            st[b]["end"] = ap.tile((64, N), F32, tag=f"END{b}", name=f"END{b}")
            return fillers

        # ---------------- one layer, both batches stage-interleaved ----------
        def layer_pair(i):
            BS = range(BPC)
            xt = [st[b]["xt"] for b in BS]
            xa = [st[b]["xa"] for b in BS]
            AT = [st[b]["AT"] for b in BS]

            gcw = [[[ap.tile((CH[c][1], CH[c][1]), BF16, tag=f"gcw{b}_{c}_{s}",
                             bufs=2, name=f"gcw{b}_{i}_{c}_{s}")
                     for c in range(2)] for s in range(3)] for b in BS]
            for b in BS:
                for s in range(3):
                    for c in range(2):
                        nc.sync.dma_start(out=gcw[b][s][c][:],
                                          in_=d[f"wgc{c}"][i, s])

            # -- attention + sigmoid
            xn = [[None, None] for b in BS]
            r1 = [[None, None] for b in BS]
            for b in BS:
                for c in range(2):
                    rows = CH[c][1]
                    r1[b][c] = ap.tile((rows, N), BF16, tag=f"R1{b}_{c}",
                                       name=f"R1{b}_{i}_{c}")
                    xn[b][c] = ap.tile((rows, N), BF16, tag=f"XN{b}_{c}",
                                       name=f"XN{b}_{i}_{c}")
            groups = [(c, nsi) for c in range(2) for nsi in range(2)]
            m1s, apss = {}, {}
            for c, nsi in groups:
                rows, ns = CH[c][1], NS[nsi]
                for b in BS:
                    m1 = pp.tile((rows, 512), F32, tag="pwork", bufs=3,
                                 name=f"m1_{b}_{i}_{c}_{nsi}")
                    nc.tensor.matmul(m1[:], wfc1[c][:], xt[b][c][:, ns],
                                     start=True, stop=True)
                    m1s[b, c, nsi] = m1
                for b in BS:
                    if b % 2 == 0:
                        nc.scalar.activation(r1[b][c][:, ns], m1s[b, c, nsi][:],
                                             AF.Relu)
                    else:
                        nc.vector.tensor_scalar(r1[b][c][:, ns],
                                                m1s[b, c, nsi][:],
                                                0.0, None, ALU.max)
            for c, nsi in groups:
                rows, ns = CH[c][1], NS[nsi]
                for b in BS:
                    a_ps = pp.tile((rows, 512), F32, tag="pwork", bufs=3,
                                   name=f"aps{b}_{i}_{c}_{nsi}")
                    nc.tensor.matmul(a_ps[:], wfc2[c][:], r1[b][c][:, ns],
                                     start=True, stop=False)
                    nc.tensor.matmul(a_ps[:], idenh[:rows, :rows],
                                     xt[b][c][:, ns], start=False, stop=True)
                    apss[b, c, nsi] = a_ps
                for b in BS:
                    # xn = sigmoid(2*(a + x/2)) straight from PSUM
                    nc.scalar.activation(xn[b][c][:, ns], apss[b, c, nsi][:],
                                         AF.Sigmoid, scale=2.0)

            # -- V-layout of xn via PE transposes
            xv = [[None] * 8 for b in BS]
            for v in range(8):
                cs = slice(v * 128, (v + 1) * 128)
                for b in BS:
                    tp = pp.tile((128, CL), BF16, tag="ptr", bufs=3,
                                 name=f"tpx{b}_{i}_{v}")
                    for c in range(2):
                        o, rows = CH[c]
                        nc.tensor.transpose(tp[:, o:o + rows],
                                            xn[b][c][:, cs],
                                            idenb[:rows, :rows])
                    xv[b][v] = ap.tile((128, CL), BF16, tag=f"XV{b}_{v}",
                                       name=f"XV{b}_{i}_{v}")
                    nc.vector.tensor_copy(xv[b][v][:], tp[:])

            def hop(rv, nm):
                """A-hop (V-orientation, w-pairs) + transpose back, both b."""
                hvp = [[None] * 4 for b in BS]
                for p in range(4):
                    for b in BS:
                        h_ps = pp.tile((128, 2 * CL), F32, tag="ptr", bufs=3,
                                       name=f"hp{nm}{b}_{i}_{p}")
                        for half in range(2):
                            w = 2 * p + half
                            ws = slice(w * 128, (w + 1) * 128)
                            dst = h_ps[:, half * CL:(half + 1) * CL]
                            for k in range(8):
                                nc.tensor.matmul(dst, AT[b][k][:, ws], rv(b, k),
                                                 start=(k == 0), stop=(k == 7))
                        hvp[b][p] = ap.tile((128, 2 * CL), BF16,
                                            tag=f"{nm}V{b}_{p}",
                                            name=f"{nm}V{b}_{i}_{p}")
                        if (b + p) % 2 == 0:
                            nc.vector.tensor_copy(hvp[b][p][:], h_ps[:])
                        else:
                            nc.scalar.activation(hvp[b][p][:], h_ps[:], AF.Copy)

                ht = [[ap.tile((CH[c][1], N), BF16, tag=f"{nm}T{b}_{c}",
                               name=f"{nm}T{b}_{i}_{c}") for c in range(2)]
                      for b in BS]
                for b in BS:
                    tpb = [pp.tile((CH[c][1], N), BF16, tag=f"ptb{c}",
                                   bufs=1, name=f"tpb{nm}{b}_{i}_{c}")
                           for c in range(2)]
                    for w in range(8):
                        src = hvp[b][w // 2][:, (w % 2) * CL:(w % 2) * CL + CL]
                        for c in range(2):
                            o, rows = CH[c]
                            nc.tensor.transpose(
                                tpb[c][:, w * 128:(w + 1) * 128],
                                src[:, o:o + rows], idenb[:, :])
                        if w % 4 == 3:
                            half = slice((w - 3) * 128, (w + 1) * 128)
                            for c in range(2):
                                if (b + c) % 2 == 0:
                                    nc.scalar.activation(ht[b][c][:, half],
                                                         tpb[c][:, half],
                                                         AF.Copy)
                                else:
                                    nc.vector.tensor_copy(ht[b][c][:, half],
                                                          tpb[c][:, half])
                return hvp, ht

            h1vp, h1t = hop(lambda b, k: xv[b][k][:], "H1")
            # -- skip conv -> relu -> end1 matmul -> SBUF accumulator
            rsk = [ap.tile((SKR, N), BF16, tag=f"rsk{b}", name=f"rsk{b}_{i}")
                   for b in BS]
            sks = {}
            for nsi, ns in enumerate(NS):
                for b in BS:
                    sk_ps = pp.tile((SKR, 512), F32, tag="pwork", bufs=3,
                                    name=f"skp{b}_{i}_{nsi}")
                    nc.tensor.matmul(sk_ps[:64], wskip[i][0][:],
                                     xn[b][0][:, ns], start=True, stop=True)
                    nc.tensor.matmul(sk_ps[64:], wskip[i][1][:],
                                     xn[b][1][:, ns], start=True, stop=True)
                    sks[b, nsi] = sk_ps
                for b in BS:
                    if b % 2 == 0:
                        nc.vector.tensor_scalar(rsk[b][:, ns], sks[b, nsi][:],
                                                vcol(f"skb{i}", SKR), 0.0,
                                                ALU.add, ALU.max)
                    else:
                        nc.scalar.activation(rsk[b][:, ns], sks[b, nsi][:],
                                             AF.Relu, bias=vcol(f"skb{i}", SKR))
            for nsi, ns in enumerate(NS):
                for b in BS:
                    e_ps = pp.tile((64, 512), F32, tag="pwork", bufs=3,
                                   name=f"eps{b}_{i}_{nsi}")
                    nc.tensor.matmul(e_ps[:], we1[i][:], rsk[b][:, ns],
                                     start=True, stop=True)
                    if i == 0:
                        nc.vector.tensor_copy(st[b]["end"][:, ns], e_ps[:])
                    else:
                        nc.vector.scalar_tensor_tensor(
                            st[b]["end"][:, ns], e_ps[:], 0.0,
                            st[b]["end"][:, ns], ALU.bypass, ALU.add)

            _, h2t = hop(
                lambda b, k: h1vp[b][k // 2][:, (k % 2) * CL:(k % 2) * CL + CL],
                "H2")

            # -- gconv (block-diag over l); av*xa accumulated in PSUM via a
            # diagonal matmul; bn affine applied on eviction; the residual
            # bns*x is added by the otherwise-idle Pool engine.
            for c in range(2):
                rows = CH[c][1]
                nxs = [ap.tile((rows, N), F32, tag=f"tmp{b}_{c}",
                               name=f"nxs{b}_{i}_{c}") for b in BS]
                nxt = [ap.tile((rows, N), F32R, tag=f"XT{b}_{c}", bufs=2,
                               name=f"XT{b}_{i}_{c}") for b in BS]
                # Pool precomputes pre = bns*x + bnb early (depends only on
                # layer-start x); the PSUM eviction is one DVE op.
                for nsi, ns in enumerate(NS):
                    for b in BS:
                        nc.gpsimd.tensor_scalar(
                            nxs[b][:, ns], xt[b][c][:, ns].bitcast(F32),
                            vcol(f"bns{i}_{c}", rows),
                            vcol(f"bnb{i}_{c}", rows), ALU.mult, ALU.add)
                for nsi, ns in enumerate(NS):
                    gps = []
                    for b in BS:
                        g_ps = pp.tile((rows, 512), F32, tag="pwork", bufs=3,
                                       name=f"gp{b}_{i}_{c}_{nsi}")
                        srcs = (xn[b], h1t[b], h2t[b])
                        for s in range(3):
                            nc.tensor.matmul(g_ps[:], gcw[b][s][c][:],
                                             srcs[s][c][:, ns],
                                             start=(s == 0), stop=False)
                        nc.tensor.matmul(g_ps[:], wav[i][c][:],
                                         xa[b][c][:, ns],
                                         start=False, stop=True)
                        gps.append(g_ps)
                    for b in BS:
                        nc.vector.scalar_tensor_tensor(
                            nxt[b][:, ns], gps[b][:],
                            vcol(f"bns{i}_{c}", rows), nxs[b][:, ns],
                            ALU.mult, ALU.add)
                for b in BS:
                    xt[b][c] = nxt[b]

        # ---------------- end convs ----------------
        def tail(b):
            o1 = ap.tile((64, N), F32R, tag="o1", name=f"o1_{b}")
            ob = ap.tile((12, N), F32, tag="ob", name=f"ob{b}")
            for nsi, ns in enumerate(NS):
                nc.scalar.activation(o1[:, ns], st[b]["end"][:, ns], AF.Relu,
                                     bias=vcol("e1b", 64))
                o2_ps = pp.tile((12, 512), F32, tag="pwork", bufs=3,
                                name=f"o2p{b}_{nsi}")
                nc.tensor.matmul(o2_ps[:], we2[:], o1[:, ns],
                                 start=True, stop=True)
                nc.scalar.activation(ob[:, ns], o2_ps[:], AF.Identity,
                                     bias=vcol("e2b", 12))
            nc.sync.dma_start(out=outp[b], in_=ob[:])

        phase0_pair()
        fillers = []
        for b in range(BPC):
            fillers.extend(start(b))
        phase0_D(fillers)
        for i in range(L):
            layer_pair(i)
        for b in range(BPC):
            tail(b)

    nc.finalize()
    return nc


# ----------------------------------------------------------------------------
# host-side preprocessing
# ----------------------------------------------------------------------------

def _prep_host(inputs):
    f = lambda x: np.asarray(x, dtype=np.float32)
    bf = lambda x: np.ascontiguousarray(x).astype(ml_dtypes.bfloat16)
    x_in = f(inputs["inputs"])
    ind = np.asarray(inputs["ind"]).astype(np.int64)
    p1, p2, p3, pk = f(inputs["p1"]), f(inputs["p2"]), f(inputs["p3"]), f(inputs["pk"])

    xo = np.pad(x_in, ((0, 0), (0, 0), (0, 0), (RF - T, 0)))
    inp_t = np.ascontiguousarray(xo.transpose(0, 1, 3, 2))
    te = p1[ind]
    adp = np.einsum("bi,ijk->bjk", te, pk).astype(np.float32)

    start_w, start_b = f(inputs["start_w"]), f(inputs["start_b"])
    starta_w, starta_b = f(inputs["starta_w"]), f(inputs["starta_b"])
    fc1_w, fc2_w = f(inputs["fc1_w"]), f(inputs["fc2_w"])
    skip_w, skip_b = f(inputs["skip_w"]), f(inputs["skip_b"])
    gconv_w, gconv_b = f(inputs["gconv_w"]), f(inputs["gconv_b"])
    bn_g, bn_b = f(inputs["bn_g"]), f(inputs["bn_b"])
    bna_g, bna_b = f(inputs["bna_g"]), f(inputs["bna_b"])
    end1_w, end1_b = f(inputs["end1_w"]), f(inputs["end1_b"])
    end2_w, end2_b = f(inputs["end2_w"]), f(inputs["end2_b"])

    e8, e5 = np.eye(8, dtype=np.float32), np.eye(5, dtype=np.float32)
    e13 = np.eye(RF, dtype=np.float32)
    kr = lambda e, w: np.kron(e, np.ascontiguousarray(w.T)).astype(np.float32)

    wstart0 = np.stack([np.kron(e13[:, :8], w[:, 0][None, :])
                        for w in (start_w, starta_w)]).astype(np.float32)
    wstart1 = np.stack([np.kron(e13[:, 8:], w[:, 0][None, :])
                        for w in (start_w, starta_w)]).astype(np.float32)
    wgc0 = np.stack([np.stack([kr(e8, gconv_w[i][:, s * 16:(s + 1) * 16])
                               for s in range(3)]) for i in range(L)])
    wgc1 = np.stack([np.stack([kr(e5, gconv_w[i][:, s * 16:(s + 1) * 16])
                               for s in range(3)]) for i in range(L)])
    wskip0 = np.stack([kr(e8, skip_w[i]) for i in range(L)])
    wskip1 = np.stack([kr(e5, skip_w[i]) for i in range(L)])

    # end1 columns: ref skip rows are o*13+l within the (L-1-i)-th block;
    # ours are l*8+o
    we1 = np.zeros((L, SKR, 64), dtype=np.float32)
    ll, oo = np.meshgrid(np.arange(RF), np.arange(SC), indexing="ij")
    src_col = oo.ravel() * RF + ll.ravel()
    for i in range(L):
        we1[i] = end1_w[:, (L - 1 - i) * SKR + src_col].T

    t8 = lambda v: np.tile(v, 8)
    vecs = np.zeros((128, NV_COLS), dtype=np.float32)
    ci = 0
    vecs[:, ci] = t8(start_b); ci += 1
    vecs[:80, ci] = np.tile(start_b, 5); ci += 1
    vecs[:, ci] = t8(starta_b); ci += 1
    vecs[:80, ci] = np.tile(starta_b, 5); ci += 1
    for i in range(L):
        vecs[:SKR, ci] = np.tile(skip_b[i], RF); ci += 1
    bns = (bn_g / np.sqrt(1.0 + BN_EPS)).astype(np.float32)
    bnas = (bna_g / np.sqrt(1.0 + BN_EPS)).astype(np.float32)
    av = np.ones(16, dtype=np.float32)
    bv = np.zeros(16, dtype=np.float32)
    for i in range(L):
        bnb_adj = bn_b[i] + bns[i] * (gconv_b[i] + bv)
        vecs[:, ci] = t8(bns[i]); ci += 1
        vecs[:, ci] = t8(bnb_adj); ci += 1
        vecs[:, ci] = t8(av); ci += 1
        vecs[:80, ci] = np.tile(bns[i], 5); ci += 1
        vecs[:80, ci] = np.tile(bnb_adj, 5); ci += 1
        vecs[:80, ci] = np.tile(av, 5); ci += 1
        av = 2.0 * bnas[i] * av
        bv = 2.0 * bnas[i] * bv + bna_b[i]
    # rebuild per-layer diag(av) for the PE-side xa accumulation
    avs = [np.ones(16, dtype=np.float32)]
    for i in range(L):
        avs.append(2.0 * bnas[i] * avs[-1])
    wav0 = np.stack([np.diag(np.tile(avs[i], 8)) for i in range(L)])
    wav1 = np.stack([np.diag(np.tile(avs[i], 5)) for i in range(L)])
    wav0 = wav0.astype(ml_dtypes.bfloat16)
    wav1 = wav1.astype(ml_dtypes.bfloat16)
    vecs[:64, ci] = end1_b; ci += 1
    vecs[:12, ci] = end2_b; ci += 1
    assert ci == NV_COLS

    shared = {
        "p2T": np.ascontiguousarray(p2.T),
        "p3sT": np.ascontiguousarray(p3[:DIMS, :DIMS].T),
        "wstart0": wstart0, "wstart1": wstart1,
        "wfc1_0": kr(e8, fc1_w), "wfc1_1": kr(e5, fc1_w),
        "wfc2_0": bf(kr(e8, fc2_w)), "wfc2_1": bf(kr(e5, fc2_w)),
        "wskip0": bf(wskip0), "wskip1": bf(wskip1),
        "wgc0": bf(wgc0), "wgc1": bf(wgc1),
        "we1": bf(we1), "we2": np.ascontiguousarray(end2_w.T),
        "idenb": np.eye(128, dtype=ml_dtypes.bfloat16),
        "idenr": np.eye(128, dtype=np.float32),
        "idenh": 0.5 * np.eye(128, dtype=np.float32),
        "wav0": wav0, "wav1": wav1,
        "vecs": vecs,
    }
    in_maps = []
    for c in range(NCORES):
        bs = slice(c * BPC, (c + 1) * BPC)
        m = dict(shared)
        m["inp"] = np.ascontiguousarray(inp_t[bs])
        m["adp"] = np.ascontiguousarray(adp[bs])
        in_maps.append(m)
    return in_maps


def _get_nc():
    global _CACHED
    if _CACHED is None:
        _CACHED = _build_nc()
    return _CACHED


def run(inputs, trace=False):
    nc = _get_nc()
    in_maps = _prep_host(inputs)
    res = run_bass_kernel_spmd(nc, in_maps, core_ids=list(range(NCORES)),
                               trace=trace)
    out = np.stack([res.results[c]["outp"] for c in range(NCORES)])
    out = out.reshape(B, 12, N, 1).astype(np.float32)
    return out, res


def kernel(**inputs):
    out, _ = run(inputs)
    return out
